# revision 2
# baseline (speedup 1.0000x reference)
"""TGN-style GNN message passing + community detection on 8 TRN2 NeuronCores.

Node-sharded SPMD, v2 (engine-balanced rewrite):
- Fused per-group pipeline (events -> agg -> GRU -> feat -> proj -> sim)
  with SBUF-resident intermediates; event tensors host-packed so every
  load is one large contiguous-per-partition DMA.
- Time encoding via a 1-partition PE outer product + Activation Sin with
  per-partition bias, landing in [feat, slot] layout, then xbar DMA
  transpose into the event matrix (no DVE work).
- has-mask folded into the GRU z-gate via a +30*(1-has) rank-1 matmul
  (sigmoid saturates to 1 -> memory passthrough), removing all blend ops.
- cn-sparsemax tau via 2 rounds of multi-probe bracketing (7 nonuniform +
  7 uniform probes, fused sub+relu+sum DVE ops in 4x bf16 mode) + secant
  interpolation: 3 AllReduces instead of 13.
- nc-sparsemax tau via secant from tau0=-2 (8 iterations, per-window
  fused DVE ops); overlapped with the cn AllReduce latency.
- c_memory via relu applied in simT layout then xbar-transposed;
  AllReduce #4. Total 4 collectives.
"""

import os
from contextlib import ExitStack

import numpy as np
import ml_dtypes

import concourse.bass as bass
import concourse.mybir as mybir
import concourse.tile as tile
from concourse.bass_utils import run_bass_kernel_spmd
from concourse.masks import make_identity

FP32 = mybir.dt.float32
BF16 = mybir.dt.bfloat16
AF = mybir.ActivationFunctionType
ALU = mybir.AluOpType
AX = mybir.AxisListType

NCORES = 8
D = 128
C = 256
HALF_PI = float(np.pi / 2)
TWO_PI = float(2 * np.pi)
MAGIC = 12582912.0
GW = 7                      # windows per group
GN = GW * 128               # nodes per group (896)
GS = 2 * GN                 # event slots per group (1792)
GT = GS // 128              # slot-tiles per group (14)

# cn bracket probe offsets below rowmax (round 1, ascending positions)
CN_FR1 = [0.45, 0.25, 0.17, 0.12, 0.08, 0.05, 0.02]
CN_K = len(CN_FR1)          # 7 probes per round
KX = CN_K + 2               # probe array incl. bracket ends
CN_F2 = [(k + 1) / (CN_K + 1) for k in range(CN_K)]
BIGV = 1.0e4
NIT_NC = 8

bfc = lambda x: np.ascontiguousarray(np.asarray(x).astype(ml_dtypes.bfloat16))
f32c = lambda x: np.ascontiguousarray(np.asarray(x).astype(np.float32))


def _bcast_row(dram_tensor, ncols, nparts=128, off=0):
    row = dram_tensor.ap()
    return bass.AP(tensor=row.tensor, offset=row.offset + off,
                   ap=[[0, nparts], [1, ncols]])


def split_waits(nc, sp_limit=1, default_limit=1):
    """This env's walrus rejects >1 sync-wait on SP CTRL instructions:
    move extra waits onto preceding NOPs."""
    limits = {mybir.EngineType.SP: sp_limit}
    for fn in nc.m.functions:
        for bb in fn.blocks:
            out = []
            for ins in bb.instructions:
                si = ins.sync_info
                w = list(si.on_wait) if (si is not None and si.on_wait) else []
                lim = limits.get(ins.engine, default_limit)
                if len(w) > lim:
                    extra, keep = w[:-lim], w[-lim:]
                    for j in range(0, len(extra), lim):
                        out.append(mybir.InstNoOp(
                            name=f"{ins.name}-ws{j}",
                            engine=ins.engine,
                            sync_info=mybir.SyncInfo(
                                on_wait=list(extra[j:j + lim]), on_update=[]),
                        ))
                    ins.sync_info = mybir.SyncInfo(
                        on_wait=list(keep),
                        on_update=list(si.on_update) if si.on_update else [])
                out.append(ins)
            bb.instructions = out
    return nc


def build_program(L, debug=False):
    NG = L // GN            # groups (14 for L=12544)
    NW = L // 128           # windows (98)
    NT = 2 * L // 128       # slot-tiles total (196)

    nc = bass.Bass(num_devices=NCORES)

    # ---- inputs ----
    ev_x = nc.dram_tensor("ev_x", [128, NT, 256], BF16, kind="ExternalInput")
    ev_meta = nc.dram_tensor("ev_meta", [128, 2, NT], FP32, kind="ExternalInput")
    dt_row = nc.dram_tensor("dt_row", [NG, GS], FP32, kind="ExternalInput")
    mem_bf = nc.dram_tensor("mem_bf", [128, L], BF16, kind="ExternalInput")
    nf_bf = nc.dram_tensor("nf_bf", [128, L], BF16, kind="ExternalInput")
    nothas = nc.dram_tensor("nothas", [NG, GN], BF16, kind="ExternalInput")
    W_ihT = nc.dram_tensor("W_ihT", [128, 4, 384], BF16, kind="ExternalInput")
    W_hhT = nc.dram_tensor("W_hhT", [128, 384], BF16, kind="ExternalInput")
    bsum = nc.dram_tensor("bsum", [128, 2], FP32, kind="ExternalInput")
    b_hh2 = nc.dram_tensor("b_hh2", [128, 1], FP32, kind="ExternalInput")
    b_ih2 = nc.dram_tensor("b_ih2", [128, 1], FP32, kind="ExternalInput")
    pWt = nc.dram_tensor("pWt", [128, 128], BF16, kind="ExternalInput")
    pb = nc.dram_tensor("pb", [128, 1], FP32, kind="ExternalInput")
    cenT = nc.dram_tensor("cenT", [128, C], FP32, kind="ExternalInput")
    w_row = nc.dram_tensor("w_row", [1, 128], FP32, kind="ExternalInput")
    bpi_col = nc.dram_tensor("bpi_col", [128, 1], FP32, kind="ExternalInput")
    iota_t = nc.dram_tensor("iota_t", [128, 128], BF16, kind="ExternalInput")

    emb_out = nc.dram_tensor("emb", [L, D], FP32, kind="ExternalOutput")
    dbg = {}
    if debug:
        dbg['newmemT'] = nc.dram_tensor("dbg_newmemT", [128, L], FP32, kind="ExternalOutput")
        dbg['aggT'] = nc.dram_tensor("dbg_aggT", [128, 3, L], FP32, kind="ExternalOutput")
        dbg['xg'] = nc.dram_tensor("dbg_xg", [128, 2 * L // 128, 256], BF16, kind="ExternalOutput")
        dbg['tencT'] = nc.dram_tensor("dbg_tencT", [128, 2 * L], FP32, kind="ExternalOutput")
        dbg['simT'] = nc.dram_tensor("dbg_simT", [128, 2, L], BF16, kind="ExternalOutput")
        dbg['taunc'] = nc.dram_tensor("dbg_taunc", [128, NW], FP32, kind="ExternalOutput")
        dbg['taucn'] = nc.dram_tensor("dbg_taucn", [128, 2], FP32, kind="ExternalOutput")
        dbg['cmem'] = nc.dram_tensor("dbg_cmem", [C, D], FP32, kind="ExternalOutput")

    # ---- staging DRAM ----
    crec_dram = nc.dram_tensor("crec_dram", [1, C], BF16)
    rnorm_dram = nc.dram_tensor("rnorm_dram", [1, L], BF16)
    taunc_dram = nc.dram_tensor("taunc_dram", [NW, 128], BF16)
    taucn_dram = nc.dram_tensor("taucn_dram", [2, 128], BF16)
    rm_l = nc.dram_tensor("rm_l", [128, 4], FP32)
    rm_a = nc.dram_tensor("rm_a", [128, 4], FP32, addr_space="Shared")
    gp_l = [nc.dram_tensor(f"gp_l{r}", [128, 2 * CN_K], FP32) for r in range(2)]
    gp_a = [nc.dram_tensor(f"gp_a{r}", [128, 2 * CN_K], FP32, addr_space="Shared")
            for r in range(2)]
    cm_local = nc.dram_tensor("cm_local", [C, D], FP32)
    cm_all = nc.dram_tensor("cm_all", [C, D], FP32, addr_space="Shared")
    RG = [list(range(NCORES))]

    cc_sem = nc.alloc_semaphore("cc_done")
    ccv = [0]

    def allreduce(alu_op, local_dram, shared_dram, sb_in, sb_out,
                  in_ap=None, out_ap=None):
        """Stage sb_in -> local_dram, AllReduce -> shared_dram, load sb_out."""
        with tc.tile_critical():
            nc.gpsimd.dma_start(
                out=local_dram.ap() if in_ap is None else in_ap,
                in_=sb_in).then_inc(cc_sem, 16)
            ccv[0] += 16
            nc.gpsimd.wait_ge(cc_sem, ccv[0])
            nc.gpsimd.collective_compute(
                "AllReduce", alu_op, replica_groups=RG,
                ins=[local_dram.ap().opt()],
                outs=[shared_dram.ap().opt()]).then_inc(cc_sem)
            ccv[0] += 1
            nc.gpsimd.wait_ge(cc_sem, ccv[0])
            nc.gpsimd.dma_start(
                out=sb_out,
                in_=shared_dram.ap() if out_ap is None else out_ap
            ).then_inc(cc_sem, 16)
            ccv[0] += 16
            nc.gpsimd.wait_ge(cc_sem, ccv[0])

    ctx = ExitStack()
    with tile.TileContext(nc) as tc, ctx:
        const = ctx.enter_context(tc.tile_pool(name="const", bufs=1))
        late = ctx.enter_context(tc.tile_pool(name="late", bufs=1))
        wk = ctx.enter_context(tc.tile_pool(name="wk", bufs=2))
        scr = ctx.enter_context(tc.tile_pool(name="scr", bufs=1))
        psS = ctx.enter_context(tc.tile_pool(name="psS", bufs=1, space="PSUM"))

        # ----- constants -----
        identb = const.tile([128, 128], BF16)
        make_identity(nc, identb)
        iota = const.tile([128, 128], BF16)
        nc.sync.dma_start(out=iota, in_=iota_t[:, :])
        wih = const.tile([128, 4, 384], BF16)
        nc.sync.dma_start(out=wih, in_=W_ihT[:, :, :])
        whh = const.tile([128, 384], BF16)
        nc.sync.dma_start(out=whh, in_=W_hhT[:, :])
        bs = const.tile([128, 2], FP32)
        nc.sync.dma_start(out=bs, in_=bsum[:, :])
        bh2 = const.tile([128, 1], FP32)
        nc.sync.dma_start(out=bh2, in_=b_hh2[:, :])
        bi2 = const.tile([128, 1], FP32)
        nc.sync.dma_start(out=bi2, in_=b_ih2[:, :])
        pw = const.tile([128, 128], BF16)
        nc.sync.dma_start(out=pw, in_=pWt[:, :])
        pbt = const.tile([128, 1], FP32)
        nc.sync.dma_start(out=pbt, in_=pb[:, :])
        wrow = const.tile([1, 128], FP32)
        nc.sync.dma_start(out=wrow, in_=w_row[:, :])
        bpi = const.tile([128, 1], FP32)
        nc.sync.dma_start(out=bpi, in_=bpi_col[:, :])
        meta = const.tile([128, 2, NT], FP32)
        nc.sync.dma_start(out=meta, in_=ev_meta[:, :, :])
        ones_col = const.tile([128, 1], BF16)
        nc.vector.memset(ones_col, 1.0)
        thirty = const.tile([1, 128], BF16)
        nc.vector.memset(thirty, 30.0)
        eps1 = const.tile([1, 1], FP32)
        nc.vector.memset(eps1, 1e-12)

        # centroid norms (device, overlaps with first group loads)
        cen = const.tile([128, C], FP32)
        nc.sync.dma_start(out=cen, in_=cenT[:, :])
        censq = scr.tile([128, C], BF16, tag="censq")
        nc.vector.tensor_mul(censq, cen, cen)
        ps_c = psS.tile([1, 448], FP32, tag="srow")
        nc.tensor.matmul(ps_c[:, 0:C], ones_col, censq, start=True, stop=True)
        cnorm = scr.tile([1, C], FP32, tag="cnorm")
        nc.scalar.activation(cnorm, ps_c[:, 0:C], AF.Sqrt)
        nc.vector.tensor_scalar_add(cnorm, cnorm, 1e-8)
        crecf = scr.tile([1, C], FP32, tag="crecf")
        nc.vector.reciprocal(crecf, cnorm)
        crec = scr.tile([1, C], BF16, tag="crec")
        nc.vector.tensor_copy(crec, crecf)
        nc.sync.dma_start(out=crec_dram[:, :], in_=crec)
        crec_rep = const.tile([128, C], BF16)
        nc.sync.dma_start(out=crec_rep, in_=_bcast_row(crec_dram, C))
        cennT = const.tile([128, C], BF16)
        nc.vector.tensor_mul(cennT, cen, crec_rep)

        # ----- long-lived tensors -----
        simT = late.tile([128, 2, L], BF16)
        rm_part = late.tile([128, 2, NG], FP32)
        nodep = ctx.enter_context(tc.tile_pool(name="nodep", bufs=1))
        nm_node = nodep.tile([128, L // 128, 128], BF16)

        if True:

            # ================= fused group loop =================
            with tc.tile_pool(name="grp", bufs=2) as grp, \
                    tc.tile_pool(name="grpc", bufs=1) as grpc, \
                    tc.tile_pool(name="grp1", bufs=1) as grp1, \
                    tc.tile_pool(name="psA", bufs=1, space="PSUM") as psA, \
                    tc.tile_pool(name="psO", bufs=1, space="PSUM") as psO, \
                    tc.tile_pool(name="psG", bufs=1, space="PSUM") as psG:
                for g in range(NG):
                    t0 = g * GT
                    n0 = g * GN
                    nsl = bass.ds(n0, GN)
                    # --- event loads ---
                    xg = grp.tile([128, GT, 256], BF16, tag="xg")
                    nc.sync.dma_start(out=xg, in_=ev_x[:, t0:t0 + GT, :])
                    xgt = grp.tile([128, GT, 128], BF16, tag="xgt")
                    if debug == 2:
                        nc.sync.dma_start(out=dbg['xg'][:, t0:t0 + GT, :], in_=xg)
                    dtr = grpc.tile([1, GS], FP32, tag="dtr")
                    nc.sync.dma_start(out=dtr, in_=dt_row[g:g + 1, :])
                    # --- time encode: outer product + magic-number range
                    # reduce + Sin + xbar transpose ---
                    tencT = grpc.tile([128, GS], BF16, tag="tencT")
                    for q in range(2):
                        ang = grp1.tile([128, GS // 2], FP32, tag="ang",
                                        name=f"ang{g}{q}")
                        for h in range(2):
                            c0 = q * 896 + h * 448
                            po = psO.tile([128, 448], FP32, tag="po",
                                          name=f"po{g}_{q}{h}")
                            nc.tensor.matmul(po, wrow, dtr[:, c0:c0 + 448],
                                             start=True, stop=True)
                            nc.scalar.activation(ang[:, h * 448:(h + 1) * 448],
                                                 po, AF.Identity, bias=bpi[:, 0:1])
                        m1 = grp1.tile([128, GS // 2], FP32, tag="m1",
                                       name=f"m1_{g}{q}")
                        nc.vector.tensor_scalar(m1, ang, 1.0 / TWO_PI, MAGIC,
                                                op0=ALU.mult, op1=ALU.add)
                        nc.vector.tensor_scalar_add(m1, m1, -MAGIC)
                        nc.vector.scalar_tensor_tensor(ang, m1, -TWO_PI, ang,
                                                       op0=ALU.mult, op1=ALU.add)
                        nc.scalar.activation(tencT[:, q * 896:(q + 1) * 896],
                                             ang, AF.Sin)
                    nc.sync.dma_start_transpose(xgt, tencT)
                    # --- aggregation ---
                    agg = grpc.tile([128, 3, GN], BF16, tag="agg")
                    for w in range(GW):
                        pas = [psA.tile([128, 128], FP32, tag=f"agg{fc}",
                                        name=f"pa{g}_{w}_{fc}")
                               for fc in range(3)]
                        for t_ in range(2):
                            ti = w * 2 + t_
                            oh = grpc.tile([128, 128], BF16, tag="oh")
                            nc.vector.tensor_scalar(
                                oh, iota, meta[:, 0, t0 + ti:t0 + ti + 1],
                                meta[:, 1, t0 + ti:t0 + ti + 1],
                                op0=ALU.is_equal, op1=ALU.mult)
                            for fc in range(3):
                                xsrc = (xg[:, ti, fc * 128:(fc + 1) * 128]
                                        if fc < 2 else xgt[:, ti, :])
                                nc.tensor.matmul(
                                    pas[fc], xsrc,
                                    oh, start=(t_ == 0), stop=(t_ == 1))
                        for fc in range(3):
                            if (w + fc) % 2 == 0:
                                nc.vector.tensor_copy(
                                    agg[:, fc, w * 128:(w + 1) * 128], pas[fc])
                            else:
                                nc.scalar.activation(
                                    agg[:, fc, w * 128:(w + 1) * 128], pas[fc],
                                    AF.Identity)
                    if debug == 2:
                        aggf = grpc.tile([128, 3, GN], FP32, tag="aggf")
                        nc.vector.tensor_copy(aggf, agg)
                        nc.sync.dma_start(out=dbg['aggT'][:, :, nsl], in_=aggf)
                        tencf = grpc.tile([128, GS], FP32, tag="tencf")
                        nc.vector.tensor_copy(tencf, tencT)
                        nc.sync.dma_start(
                            out=dbg['tencT'][:, g * GS:(g + 1) * GS], in_=tencf)
                    # --- GRU ---
                    mg = grp.tile([128, GN], BF16, tag="mg")
                    nc.sync.dma_start(out=mg, in_=mem_bf[:, nsl])
                    nhg = grpc.tile([1, GN], BF16, tag="nhg")
                    nc.sync.dma_start(out=nhg, in_=nothas[g:g + 1, :])
                    rz = grpc.tile([128, 2, GN], BF16, tag="rz")
                    ng_t = grpc.tile([128, GN], BF16, tag="ng")
                    gh2s = grpc.tile([128, GN], BF16, tag="gh2s")
                    for h in range(2):
                        hs = bass.ds(h * 448, 448)
                        gi0 = psG.tile([128, 448], FP32, tag="gi0", name=f"gi0_{g}{h}")
                        gi1 = psG.tile([128, 448], FP32, tag="gi1", name=f"gi1_{g}{h}")
                        gi2 = psG.tile([128, 448], FP32, tag="gi2", name=f"gi2_{g}{h}")
                        gis = [gi0, gi1, gi2]
                        for m in range(3):
                            nc.tensor.matmul(gis[m], wih[:, 0, m * 128:(m + 1) * 128],
                                             mg[:, hs], start=True, stop=False)
                            for k in range(1, 4):
                                nc.tensor.matmul(
                                    gis[m], wih[:, k, m * 128:(m + 1) * 128],
                                    agg[:, k - 1, hs], start=False, stop=False)
                            if m < 2:
                                nc.tensor.matmul(gis[m], whh[:, m * 128:(m + 1) * 128],
                                                 mg[:, hs], start=False,
                                                 stop=(m == 0))
                        # z-gate +30*(1-has) (keeps memory where no events)
                        nc.tensor.matmul(gi1, thirty, nhg[:, hs],
                                         start=False, stop=True)
                        nc.scalar.activation(rz[:, 0, hs], gi0, AF.Sigmoid,
                                             bias=bs[:, 0:1])
                        nc.scalar.activation(rz[:, 1, hs], gi1, AF.Sigmoid,
                                             bias=bs[:, 1:2])
                        gh2 = psG.tile([128, 448], FP32, tag="gi0",
                                       name=f"gh2_{g}{h}")
                        nc.tensor.matmul(gh2, whh[:, 256:384], mg[:, hs],
                                         start=True, stop=True)
                        nc.scalar.activation(gh2s[:, hs], gh2, AF.Identity,
                                             bias=bh2[:, 0:1])
                        nc.vector.tensor_mul(gh2s[:, hs], rz[:, 0, hs],
                                             gh2s[:, hs])
                        nc.tensor.matmul(gi2, identb, gh2s[:, hs],
                                         start=False, stop=True)
                        nc.scalar.activation(ng_t[:, hs], gi2, AF.Tanh,
                                             bias=bi2[:, 0:1])
                    # newmem = n + z*(mem - n)
                    d_t = grpc.tile([128, GN], BF16, tag="d_t")
                    nc.vector.tensor_sub(d_t, mg, ng_t)
                    nc.vector.tensor_mul(d_t, rz[:, 1, :], d_t)
                    nmg = grpc.tile([128, GN], BF16, tag="nmg")
                    nc.vector.tensor_add(nmg, ng_t, d_t)
                    nc.sync.dma_start_transpose(
                        nm_node[:, g * GW:(g + 1) * GW, :], nmg)
                    if debug == 2:
                        nmgf = grpc.tile([128, GN], FP32, tag="nmgf")
                        nc.vector.tensor_copy(nmgf, nmg)
                        nc.sync.dma_start(out=dbg['newmemT'][:, nsl], in_=nmgf)
                    # feat = newmem + node_features (in-place over nfg)
                    nfg = grp.tile([128, GN], BF16, tag="nfg")
                    nc.sync.dma_start(out=nfg, in_=nf_bf[:, nsl])
                    ftg = nfg
                    nc.vector.tensor_add(ftg, nmg, nfg)
                    # --- projection + row norms ---
                    pfc = grp1.tile([128, GN], FP32, tag="pfc")
                    sqb = grpc.tile([128, GN], BF16, tag="sqb")
                    for h in range(2):
                        hs = bass.ds(h * 448, 448)
                        ppf = psG.tile([128, 448], FP32, tag="gi0", name=f"ppf{g}{h}")
                        nc.tensor.matmul(ppf, pw, ftg[:, hs], start=True, stop=True)
                        nc.scalar.activation(pfc[:, hs], ppf, AF.Identity,
                                             bias=pbt[:, 0:1])
                    nc.vector.tensor_mul(sqb, pfc, pfc)
                    rnb = grp1.tile([1, GN], BF16, tag="rnb")
                    for h in range(2):
                        hs = bass.ds(h * 448, 448)
                        pss = psS.tile([1, 448], FP32, tag="srow", name=f"pss{g}{h}")
                        nc.tensor.matmul(pss, ones_col, sqb[:, hs],
                                         start=True, stop=True)
                        rnf = grp1.tile([1, 448], FP32, tag="rnf")
                        nc.scalar.activation(rnf, pss, AF.Sqrt)
                        nc.vector.tensor_scalar_add(rnf, rnf, 1e-8)
                        rnr = grp1.tile([1, 448], FP32, tag="rnr")
                        nc.vector.reciprocal(rnr, rnf)
                        nc.vector.tensor_copy(rnb[:, hs], rnr)
                    nc.sync.dma_start(out=rnorm_dram[0, nsl], in_=rnb)
                    rep = grpc.tile([128, GN], BF16, tag="rep")
                    nc.sync.dma_start(out=rep, in_=_bcast_row(rnorm_dram, GN, off=n0))
                    pfng = grpc.tile([128, GN], BF16, tag="pfn")
                    nc.vector.tensor_mul(pfng, pfc, rep)
                    # --- similarity (448-col PSUM chunks) ---
                    for m in range(2):
                        for h in range(2):
                            hl = bass.ds(h * 448, 448)
                            hs = bass.ds(n0 + h * 448, 448)
                            psm = psG.tile([128, 448], FP32, tag="gi1",
                                           name=f"psm{g}{m}{h}")
                            nc.tensor.matmul(psm, cennT[:, m * 128:(m + 1) * 128],
                                             pfng[:, hl], start=True, stop=True)
                            if m == 0:
                                nc.vector.tensor_copy(simT[:, m, hs], psm)
                            else:
                                nc.scalar.activation(simT[:, m, hs], psm,
                                                     AF.Identity)
                    nc.vector.tensor_reduce(rm_part[:, :, g], simT[:, :, nsl],
                                            axis=AX.X, op=ALU.max)
            # group pools freed
            if debug:
                nc.sync.dma_start(out=dbg['simT'][:, :, :], in_=simT)

            # row max over group partials + AllReduce(max)
            rm4 = scr.tile([128, 4], FP32, tag="rm4")
            nc.vector.tensor_reduce(rm4[:, 0:2], rm_part, axis=AX.X, op=ALU.max)
            nc.vector.tensor_copy(rm4[:, 2:4], rm4[:, 0:2])
            rmg = scr.tile([128, 4], FP32, tag="rmg")
            allreduce(ALU.max, rm_l, rm_a, rm4, rmg)

        # (sim_node/nm_node filled per group above)

        with tc.tile_pool(name="slv", bufs=1) as slv, \
                tc.tile_pool(name="psC", bufs=1, space="PSUM") as psC:
            sim_node = slv.tile([128, L // 128, 256], BF16)
            for m in range(2):
                nc.sync.dma_start_transpose(
                    sim_node[:, :, m * 128:(m + 1) * 128], simT[:, m, :])
            # ===== nc secant (tau over C=256 per node) =====
            # g-eval: relu(x - t) == max(x, t) - t; accum_out reduces with
            # op1 (add) -> per-partition sum.
            junk_n = slv.tile([128, C], BF16)
            junk_n2 = slv.tile([128, C], BF16)
            zer_n = slv.tile([128, C], BF16)
            nc.vector.memset(zer_n, 0.0)
            NW = L // 128
            tau = slv.tile([128, NW], FP32)
            tau_p = slv.tile([128, NW], FP32)
            g_c = slv.tile([128, NW], FP32)
            g_p = slv.tile([128, NW], FP32)

            def nc_eval(tau_tile, g_tile):
                for ch in range(NW):
                    jt = junk_n if ch % 2 == 0 else junk_n2
                    nc.vector.scalar_tensor_tensor(
                        jt, sim_node[:, ch, :], tau_tile[:, ch:ch + 1], zer_n,
                        op0=ALU.subtract, op1=ALU.max,
                        accum_out=g_tile[:, ch:ch + 1])

            nc.vector.memset(tau_p, -2.0)
            nc_eval(tau_p, g_p)
            st1 = wk.tile([128, NW], FP32, tag="st1")
            nc.vector.tensor_scalar(st1, g_p, -1.0, 1.0 / 256.0,
                                    op0=ALU.add, op1=ALU.mult)
            nc.vector.tensor_add(tau, tau_p, st1)

            def secant_update(tt, tp, gg, gp, wtag, shape):
                num = wk.tile(shape, FP32, tag=wtag + "n")
                nc.vector.tensor_sub(num, tt, tp)
                gm1 = wk.tile(shape, FP32, tag=wtag + "g")
                nc.vector.tensor_scalar_add(gm1, gg, -1.0)
                nc.vector.tensor_mul(num, num, gm1)
                den = wk.tile(shape, FP32, tag=wtag + "d")
                nc.vector.tensor_sub(den, gp, gg)
                nc.vector.tensor_scalar_max(den, den, 1e-12)
                rden = wk.tile(shape, FP32, tag=wtag + "r")
                nc.vector.reciprocal(rden, den)
                nc.vector.tensor_copy(tp, tt)
                nc.vector.tensor_copy(gp, gg)
                stp = wk.tile(shape, FP32, tag=wtag + "s")
                nc.vector.tensor_mul(stp, num, rden)
                nc.vector.tensor_scalar(stp, stp, 0.0, 1.0,
                                        op0=ALU.max, op1=ALU.min)
                nc.vector.tensor_add(tt, tt, stp)

            def nc_iter(k):
                nc_eval(tau, g_c)
                secant_update(tau, tau_p, g_c, g_p, "ncs", [128, NW])

            # ===== cn bracket solver =====
            pos = slv.tile([128, 2, KX], FP32)
            gv = slv.tile([128, 2, KX], FP32)
            lo = slv.tile([128, 2], FP32)
            hi = slv.tile([128, 2], FP32)
            glo = slv.tile([128, 2], FP32)
            ghi = slv.tile([128, 2], FP32)
            junkL = slv.tile([128, L], BF16)
            junkL2 = junkL
            neg = slv.tile([128, 2, KX], FP32)

            def cn_probes(round_idx):
                nc.vector.tensor_scalar_mul(neg, pos, -1.0)
                for m in range(2):
                    for k in range(CN_K):
                        jt = junkL if k % 2 == 0 else junkL2
                        nc.scalar.activation(
                            jt, simT[:, m, :], AF.Relu,
                            bias=neg[:, m, 1 + k:2 + k],
                            accum_out=gv[:, m, 1 + k:2 + k])

            def cn_select():
                # shift masked (g>=1) positions by +8; the bracket ends are
                # argmax/argmin in shifted space; g values extracted by
                # bitwise-consistent is_equal one-hots (no magnitude tricks).
                msk = wk.tile([128, 2, KX], FP32, tag="msk")
                nc.vector.tensor_scalar(msk, gv, 1.0, None, op0=ALU.is_ge)
                tsel = wk.tile([128, 2, KX], FP32, tag="tsel")
                nc.vector.scalar_tensor_tensor(tsel, msk, 8.0, pos,
                                               op0=ALU.mult, op1=ALU.add)
                lo8 = wk.tile([128, 2], FP32, tag="lo8")
                nc.vector.tensor_reduce(lo8, tsel, axis=AX.X, op=ALU.max)
                hi8 = wk.tile([128, 2], FP32, tag="hi8")
                nc.vector.tensor_reduce(hi8, tsel, axis=AX.X, op=ALU.min)
                ohl = wk.tile([128, 2, KX], FP32, tag="ohl")
                sel = wk.tile([128, 2, KX], FP32, tag="sel")
                for m in range(2):
                    nc.vector.tensor_scalar(ohl[:, m, :], tsel[:, m, :],
                                            lo8[:, m:m + 1], None,
                                            op0=ALU.is_equal)
                nc.vector.tensor_mul(sel, gv, ohl)
                nc.vector.tensor_reduce(glo, sel, axis=AX.X, op=ALU.max)
                for m in range(2):
                    nc.vector.tensor_scalar(ohl[:, m, :], tsel[:, m, :],
                                            hi8[:, m:m + 1], None,
                                            op0=ALU.is_equal)
                nc.vector.tensor_mul(sel, gv, ohl)
                nc.vector.tensor_reduce(ghi, sel, axis=AX.X, op=ALU.max)
                nc.vector.tensor_scalar_add(lo, lo8, -8.0)
                nc.vector.tensor_copy(hi, hi8)

            # round 1 positions from global row max
            for k in range(CN_K):
                nc.vector.tensor_scalar_add(pos[:, 0, 1 + k:2 + k],
                                            rmg[:, 0:1], -CN_FR1[k])
                nc.vector.tensor_scalar_add(pos[:, 1, 1 + k:2 + k],
                                            rmg[:, 1:2], -CN_FR1[k])
            for m in range(2):
                nc.vector.tensor_scalar_add(pos[:, m, 0:1],
                                            rmg[:, m:m + 1], -1.0)
                nc.vector.tensor_copy(pos[:, m, KX - 1:KX], rmg[:, m:m + 1])
            nc.vector.memset(gv[:, :, 0:1], 2.0)
            nc.vector.memset(gv[:, :, KX - 1:KX], 0.0)

            cn_probes(0)
            nc_iter(0)
            nc_iter(1)
            gvg = wk.tile([128, 2, CN_K], FP32, tag="gvg")
            allreduce(ALU.add, gp_l[0], gp_a[0], gv[:, :, 1:KX - 1], gvg)
            nc.vector.tensor_copy(gv[:, :, 1:KX - 1], gvg)
            cn_select()
            # round 2: uniform probes inside bracket
            w2 = wk.tile([128, 2], FP32, tag="w2")
            nc.vector.tensor_sub(w2, hi, lo)
            for k in range(CN_K):
                nc.vector.scalar_tensor_tensor(
                    pos[:, :, 1 + k:2 + k], w2, CN_F2[k], lo,
                    op0=ALU.mult, op1=ALU.add)
            nc.vector.tensor_copy(pos[:, :, 0:1], lo)
            nc.vector.tensor_copy(pos[:, :, KX - 1:KX], hi)
            nc.vector.tensor_copy(gv[:, :, 0:1], glo)
            nc.vector.tensor_copy(gv[:, :, KX - 1:KX], ghi)

            cn_probes(1)
            nc_iter(2)
            nc_iter(3)
            nc_iter(4)
            allreduce(ALU.add, gp_l[1], gp_a[1], gv[:, :, 1:KX - 1], gvg)
            nc.vector.tensor_copy(gv[:, :, 1:KX - 1], gvg)
            cn_select()
            nc_iter(5)
            nc_iter(6)
            nc_iter(7)
            # secant interpolation: ctau = lo + clip((glo-1)/(glo-ghi)) * (hi-lo)
            ctau = slv.tile([128, 2], FP32)
            num2 = wk.tile([128, 2], FP32, tag="num2")
            nc.vector.tensor_scalar_add(num2, glo, -1.0)
            den2 = wk.tile([128, 2], FP32, tag="den2")
            nc.vector.tensor_sub(den2, glo, ghi)
            nc.vector.tensor_scalar_max(den2, den2, 1e-9)
            rd2 = wk.tile([128, 2], FP32, tag="rd2")
            nc.vector.reciprocal(rd2, den2)
            frac = wk.tile([128, 2], FP32, tag="frac")
            nc.vector.tensor_mul(frac, num2, rd2)
            nc.vector.tensor_scalar(frac, frac, 0.0, 1.0, op0=ALU.max, op1=ALU.min)
            nc.vector.tensor_sub(w2, hi, lo)
            nc.vector.tensor_mul(frac, frac, w2)
            nc.vector.tensor_add(ctau, lo, frac)
            if debug:
                nc.sync.dma_start(out=dbg['taucn'][:, :], in_=ctau)
                nc.sync.dma_start(out=dbg['taunc'][:, :], in_=tau)

            # taunc -> DRAM row for the phase-8 broadcast
            tau_b = wk.tile([128, NW], BF16, tag="tau_b")
            nc.vector.tensor_copy(tau_b, tau)
            nc.sync.dma_start(
                out=taunc_dram.ap().rearrange("w p -> p w"), in_=tau_b)

            # ===== c_memory: relu in simT layout, xbar transpose, matmul =====
            ps_cms = [psC.tile([128, 128], FP32, tag=f"cm{m}", name=f"pscm{m}")
                      for m in range(2)]
            for m in range(2):
                jt = junkL if m == 0 else junkL2
                nc.vector.tensor_scalar(
                    jt, simT[:, m, :], ctau[:, m:m + 1], 0.0,
                    op0=ALU.subtract, op1=ALU.max)
                nc.sync.dma_start_transpose(
                    sim_node[:, :, m * 128:(m + 1) * 128], jt)
            for ch in range(NW):
                for m in range(2):
                    nc.tensor.matmul(
                        ps_cms[m], sim_node[:, ch, m * 128:(m + 1) * 128],
                        nm_node[:, ch, :], start=(ch == 0), stop=(ch == NW - 1))
            cmf = wk.tile([128, 2, 128], FP32, tag="cmf")
            for m in range(2):
                nc.vector.tensor_copy(cmf[:, m, :], ps_cms[m])
            cmgf = wk.tile([128, 2, 128], FP32, tag="cmgf")
            allreduce(ALU.add, cm_local, cm_all, cmf, cmgf,
                      in_ap=cm_local.ap().rearrange("(m p) d -> p m d", p=128),
                      out_ap=cm_all.ap().rearrange("(m p) d -> p m d", p=128))
            cmg = scr.tile([128, 2, 128], BF16, tag="cmg")
            nc.vector.tensor_copy(cmg, cmgf)
            if debug:
                nc.sync.dma_start(
                    out=dbg['cmem'].ap().rearrange("(m p) d -> p m d", p=128),
                    in_=cmgf)
        # sim_node freed after c_memory (slv pool closed; nodep closes below)

        # ===== embedding =====
        with tc.tile_pool(name="embp", bufs=2) as embp, \
                tc.tile_pool(name="psZ", bufs=2, space="PSUM") as psZ:
            NW = L // 128
            tnc = const.tile([128, L], BF16)
            nc.sync.dma_start(out=tnc, in_=_bcast_row(taunc_dram, L))
            wb = 0
            while wb < NW:
                nwin = min(4, NW - wb)
                bsl = bass.ds(wb * 128, nwin * 128)
                ncm = embp.tile([128, 2, 512], BF16, tag="ncm")
                for m in range(2):
                    nc.vector.tensor_sub(ncm[:, m, 0:nwin * 128],
                                         simT[:, m, bsl], tnc[:, bsl])
                nc.vector.tensor_scalar_max(ncm[:, :, 0:nwin * 128],
                                            ncm[:, :, 0:nwin * 128], 0.0)
                ps_z = psZ.tile([128, 4, 128], FP32, tag="z")
                for k in range(nwin):
                    w = wb + k
                    for m in range(2):
                        nc.tensor.matmul(
                            ps_z[:, k, :], ncm[:, m, k * 128:(k + 1) * 128],
                            cmg[:, m, :], start=(m == 0), stop=False)
                    nc.tensor.matmul(ps_z[:, k, :], identb, nm_node[:, w, :],
                                     start=False, stop=True)
                emb_c = embp.tile([128, 4, 128], FP32, tag="emb_c")
                nc.vector.tensor_copy(emb_c[:, 0:nwin, :], ps_z[:, 0:nwin, :])
                nc.sync.dma_start(
                    out=emb_out[wb * 128:(wb + nwin) * 128, :].rearrange(
                        "(k p) d -> p k d", p=128),
                    in_=emb_c[:, 0:nwin, :])
                wb += nwin

    split_waits(nc)
    return nc


# ----------------------------------------------------------------------------
# host side
# ----------------------------------------------------------------------------

_CACHE = {}


def _route(L, src, dst, t):
    idx = np.concatenate([src, dst]).astype(np.int64)
    other = np.concatenate([dst, src]).astype(np.int64)
    tt = np.concatenate([t, t])
    eidx = np.concatenate([np.arange(len(src)), np.arange(len(src))])
    NW = L // 128
    order = np.argsort(idx, kind='stable')
    idx_s, other_s, tt_s, eidx_s = idx[order], other[order], tt[order], eidx[order]
    owner = idx_s // L
    cores = []
    for c in range(NCORES):
        msk = owner == c
        li = idx_s[msk] - c * L
        win = li // 128
        col = li % 128
        wcount = np.bincount(win, minlength=NW)
        assert wcount.max() <= 256, f"window overflow: {wcount.max()}"
        woff = np.zeros(NW + 1, np.int64)
        woff[1:] = np.cumsum(wcount)
        within = np.arange(len(li)) - woff[win]
        slot = win * 256 + within
        cores.append(dict(slot=slot, col=col, li=li, other=other_s[msk],
                          tt=tt_s[msk], eidx=eidx_s[msk]))
    return cores


def kernel(**inputs):
    node_memory = np.asarray(inputs['node_memory'])
    last_update = np.asarray(inputs['last_update'])
    node_features = np.asarray(inputs['node_features'])
    event_feat = np.asarray(inputs['event_feat'])
    t = np.asarray(inputs['t'])
    src = np.asarray(inputs['src']).astype(np.int64)
    dst = np.asarray(inputs['dst']).astype(np.int64)
    time_w = np.asarray(inputs['time_w'])
    time_b = np.asarray(inputs['time_b'])
    W_ih = np.asarray(inputs['W_ih'])
    b_ih = np.asarray(inputs['b_ih'])
    W_hh = np.asarray(inputs['W_hh'])
    b_hh = np.asarray(inputs['b_hh'])
    proj_W = np.asarray(inputs['proj_W'])
    proj_b = np.asarray(inputs['proj_b'])
    centroids = np.asarray(inputs['centroids'])

    Nn = node_memory.shape[0]
    gran = 128 * GW * NCORES          # L must be a multiple of 128*GW
    NP = -(-Nn // gran) * gran
    L = NP // NCORES
    SLOTS = 2 * L
    NT = SLOTS // 128
    NG = L // GN

    nmp = np.zeros((NP, D), np.float32); nmp[:Nn] = node_memory
    nfp = np.zeros((NP, D), np.float32); nfp[:Nn] = node_features
    lup = np.zeros(NP, np.float32); lup[:Nn] = last_update

    idx_full = np.concatenate([src, dst])
    cnt_full = np.bincount(idx_full, minlength=NP).astype(np.float32)
    icnt_full = 1.0 / np.maximum(cnt_full, 1.0)
    nothas_full = (cnt_full == 0).astype(np.float32)

    cores = _route(L, src, dst, t)
    bsum_h = f32c(np.stack([(b_ih + b_hh)[0:128], (b_ih + b_hh)[128:256]], 1))
    wih_h = bfc(W_ih.T.reshape(4, 128, 384).transpose(1, 0, 2))

    in_maps = []
    for c in range(NCORES):
        r = cores[c]
        sl = r['slot']
        p_i = sl % 128
        t_i = sl // 128
        ev_x = np.zeros((128, NT, 256), ml_dtypes.bfloat16)
        ev_x[p_i, t_i, 0:128] = nmp[r['other']].astype(ml_dtypes.bfloat16)
        ev_x[p_i, t_i, 128:256] = event_feat[r['eidx']].astype(ml_dtypes.bfloat16)
        ev_meta = np.zeros((128, 2, NT), np.float32)
        ev_meta[:, 0, :] = -1.0
        ev_meta[p_i, 0, t_i] = r['col'].astype(np.float32)
        ev_meta[p_i, 1, t_i] = icnt_full[r['li'] + c * L]
        dt_flat = np.zeros(SLOTS, np.float32)
        dt_flat[sl] = r['tt'] - lup[r['li'] + c * L]
        nsl = slice(c * L, (c + 1) * L)
        in_maps.append({
            'ev_x': ev_x,
            'ev_meta': ev_meta,
            'dt_row': f32c(dt_flat.reshape(NG, GS)),
            'mem_bf': bfc(nmp[nsl].T),
            'nf_bf': bfc(nfp[nsl].T),
            'nothas': bfc(nothas_full[nsl].reshape(NG, GN)),
            'W_ihT': wih_h,
            'W_hhT': bfc(W_hh.T),
            'bsum': bsum_h,
            'b_hh2': f32c(b_hh[256:384].reshape(128, 1)),
            'b_ih2': f32c(b_ih[256:384].reshape(128, 1)),
            'pWt': bfc(proj_W),
            'pb': f32c(proj_b.reshape(128, 1)),
            'cenT': f32c(centroids.T),
            'w_row': f32c(time_w.reshape(1, 128)),
            'bpi_col': f32c((time_b + HALF_PI).reshape(128, 1)),
            'iota_t': bfc(np.tile(np.arange(128, dtype=np.float32)[None, :],
                                  (128, 1))),
        })

    debug = int(os.environ.get("KERNEL_DEBUG", "0"))
    key = (L, debug)
    if key not in _CACHE:
        _CACHE[key] = build_program(L, debug=debug)
    nc = _CACHE[key]
    res = run_bass_kernel_spmd(nc, in_maps, list(range(NCORES)))
    emb = np.concatenate([res.results[c]['emb'] for c in range(NCORES)], 0)
    kernel._last_exec_ns = getattr(res, 'exec_time_ns', None)
    if debug:
        kernel._last_results = res.results
    return emb[:Nn].astype(np.float32)


# revision 10
# speedup vs baseline: 1.0921x; 1.0921x over previous
"""TGN-style GNN message passing + community detection on 8 TRN2 NeuronCores.

Node-sharded SPMD, v2 (engine-balanced rewrite):
- Fused per-group pipeline (events -> agg -> GRU -> feat -> proj -> sim)
  with SBUF-resident intermediates; event tensors host-packed so every
  load is one large contiguous-per-partition DMA.
- Time encoding via a 1-partition PE outer product + Activation Sin with
  per-partition bias, landing in [feat, slot] layout, then xbar DMA
  transpose into the event matrix (no DVE work).
- has-mask folded into the GRU z-gate via a +30*(1-has) rank-1 matmul
  (sigmoid saturates to 1 -> memory passthrough), removing all blend ops.
- cn-sparsemax tau via 2 rounds of multi-probe bracketing (7 nonuniform +
  7 uniform probes, fused sub+relu+sum DVE ops in 4x bf16 mode) + secant
  interpolation: 3 AllReduces instead of 13.
- nc-sparsemax tau via secant from tau0=-2 (8 iterations, per-window
  fused DVE ops); overlapped with the cn AllReduce latency.
- c_memory via relu applied in simT layout then xbar-transposed;
  AllReduce #4. Total 4 collectives.
"""

import os
from contextlib import ExitStack

import numpy as np
import ml_dtypes

import concourse.bass as bass
import concourse.mybir as mybir
import concourse.tile as tile
from concourse.bass_utils import run_bass_kernel_spmd
from concourse.masks import make_identity

FP32 = mybir.dt.float32
BF16 = mybir.dt.bfloat16
AF = mybir.ActivationFunctionType
ALU = mybir.AluOpType
AX = mybir.AxisListType

NCORES = 8
D = 128
C = 256
HALF_PI = float(np.pi / 2)
TWO_PI = float(2 * np.pi)
MAGIC = 12582912.0
GW = 7                      # windows per group
GN = GW * 128               # nodes per group (896)
GS = 2 * GN                 # event slots per group (1792)
GT = GS // 128              # slot-tiles per group (14)

# cn bracket probe offsets below rowmax (round 1, ascending positions)
CN_FR1 = [0.45, 0.25, 0.17, 0.12, 0.08, 0.05, 0.02]
CN_K = len(CN_FR1)          # 7 probes per round
KX = CN_K + 2               # probe array incl. bracket ends
CN_F2 = [(k + 1) / (CN_K + 1) for k in range(CN_K)]
BIGV = 1.0e4
NIT_NC = 6

bfc = lambda x: np.ascontiguousarray(np.asarray(x).astype(ml_dtypes.bfloat16))
f32c = lambda x: np.ascontiguousarray(np.asarray(x).astype(np.float32))


def _bcast_row(dram_tensor, ncols, nparts=128, off=0):
    row = dram_tensor.ap()
    return bass.AP(tensor=row.tensor, offset=row.offset + off,
                   ap=[[0, nparts], [1, ncols]])


def split_waits(nc, sp_limit=1, default_limit=1):
    """This env's walrus rejects >1 sync-wait on SP CTRL instructions:
    move extra waits onto preceding NOPs."""
    limits = {mybir.EngineType.SP: sp_limit}
    for fn in nc.m.functions:
        for bb in fn.blocks:
            out = []
            for ins in bb.instructions:
                si = ins.sync_info
                w = list(si.on_wait) if (si is not None and si.on_wait) else []
                lim = limits.get(ins.engine, default_limit)
                if len(w) > lim:
                    extra, keep = w[:-lim], w[-lim:]
                    for j in range(0, len(extra), lim):
                        out.append(mybir.InstNoOp(
                            name=f"{ins.name}-ws{j}",
                            engine=ins.engine,
                            sync_info=mybir.SyncInfo(
                                on_wait=list(extra[j:j + lim]), on_update=[]),
                        ))
                    ins.sync_info = mybir.SyncInfo(
                        on_wait=list(keep),
                        on_update=list(si.on_update) if si.on_update else [])
                out.append(ins)
            bb.instructions = out
    return nc


def build_program(L, debug=False):
    NG = L // GN            # groups (14 for L=12544)
    NW = L // 128           # windows (98)
    NT = 2 * L // 128       # slot-tiles total (196)

    nc = bass.Bass(num_devices=NCORES)

    # ---- inputs ----
    ev_x = nc.dram_tensor("ev_x", [128, NT, 256], BF16, kind="ExternalInput")
    ev_meta = nc.dram_tensor("ev_meta", [128, 2, NT], FP32, kind="ExternalInput")
    dt_row = nc.dram_tensor("dt_row", [NG, GS], FP32, kind="ExternalInput")
    mem_bf = nc.dram_tensor("mem_bf", [128, L], BF16, kind="ExternalInput")
    nf_bf = nc.dram_tensor("nf_bf", [128, L], BF16, kind="ExternalInput")
    nothas = nc.dram_tensor("nothas", [NG, GN], BF16, kind="ExternalInput")
    W_ihT = nc.dram_tensor("W_ihT", [128, 4, 384], BF16, kind="ExternalInput")
    W_hhT = nc.dram_tensor("W_hhT", [128, 384], BF16, kind="ExternalInput")
    bsum = nc.dram_tensor("bsum", [128, 2], FP32, kind="ExternalInput")
    b_hh2 = nc.dram_tensor("b_hh2", [128, 1], FP32, kind="ExternalInput")
    b_ih2 = nc.dram_tensor("b_ih2", [128, 1], FP32, kind="ExternalInput")
    pWt = nc.dram_tensor("pWt", [128, 128], BF16, kind="ExternalInput")
    pb = nc.dram_tensor("pb", [128, 1], FP32, kind="ExternalInput")
    cenT = nc.dram_tensor("cenT", [128, C], FP32, kind="ExternalInput")
    w_row = nc.dram_tensor("w_row", [1, 128], FP32, kind="ExternalInput")
    bpi_col = nc.dram_tensor("bpi_col", [128, 1], FP32, kind="ExternalInput")
    iota_t = nc.dram_tensor("iota_t", [128, 128], BF16, kind="ExternalInput")

    emb_out = nc.dram_tensor("emb", [L, D], FP32, kind="ExternalOutput")
    dbg = {}
    if debug:
        dbg['newmemT'] = nc.dram_tensor("dbg_newmemT", [128, L], FP32, kind="ExternalOutput")
        dbg['aggT'] = nc.dram_tensor("dbg_aggT", [128, 3, L], FP32, kind="ExternalOutput")
        dbg['xg'] = nc.dram_tensor("dbg_xg", [128, 2 * L // 128, 256], BF16, kind="ExternalOutput")
        dbg['tencT'] = nc.dram_tensor("dbg_tencT", [128, 2 * L], FP32, kind="ExternalOutput")
        dbg['simT'] = nc.dram_tensor("dbg_simT", [128, 2, L], BF16, kind="ExternalOutput")
        dbg['taunc'] = nc.dram_tensor("dbg_taunc", [128, NW], FP32, kind="ExternalOutput")
        dbg['taucn'] = nc.dram_tensor("dbg_taucn", [128, 2], FP32, kind="ExternalOutput")
        dbg['cmem'] = nc.dram_tensor("dbg_cmem", [C, D], FP32, kind="ExternalOutput")

    # ---- staging DRAM ----
    crec_dram = nc.dram_tensor("crec_dram", [1, C], BF16)
    rnorm_dram = nc.dram_tensor("rnorm_dram", [1, L], BF16)
    taunc_dram = nc.dram_tensor("taunc_dram", [NW, 128], BF16)
    taucn_dram = nc.dram_tensor("taucn_dram", [2, 128], BF16)
    rm_l = nc.dram_tensor("rm_l", [128, 4], FP32)
    rm_a = nc.dram_tensor("rm_a", [128, 4], FP32, addr_space="Shared")
    gp_l = [nc.dram_tensor(f"gp_l{r}", [128, 2 * CN_K], FP32) for r in range(2)]
    gp_a = [nc.dram_tensor(f"gp_a{r}", [128, 2 * CN_K], FP32, addr_space="Shared")
            for r in range(2)]
    cm_local = nc.dram_tensor("cm_local", [C, D], FP32)
    cm_all = nc.dram_tensor("cm_all", [C, D], FP32, addr_space="Shared")
    RG = [list(range(NCORES))]

    cc_sem = nc.alloc_semaphore("cc_done")
    ccv = [0]

    def allreduce(alu_op, local_dram, shared_dram, sb_in, sb_out,
                  in_ap=None, out_ap=None):
        """Stage sb_in -> local_dram, AllReduce -> shared_dram, load sb_out."""
        with tc.tile_critical():
            nc.gpsimd.dma_start(
                out=local_dram.ap() if in_ap is None else in_ap,
                in_=sb_in).then_inc(cc_sem, 16)
            ccv[0] += 16
            nc.gpsimd.wait_ge(cc_sem, ccv[0])
            nc.gpsimd.collective_compute(
                "AllReduce", alu_op, replica_groups=RG,
                ins=[local_dram.ap().opt()],
                outs=[shared_dram.ap().opt()]).then_inc(cc_sem)
            ccv[0] += 1
            nc.gpsimd.wait_ge(cc_sem, ccv[0])
            nc.gpsimd.dma_start(
                out=sb_out,
                in_=shared_dram.ap() if out_ap is None else out_ap
            ).then_inc(cc_sem, 16)
            ccv[0] += 16
            nc.gpsimd.wait_ge(cc_sem, ccv[0])

    ctx = ExitStack()
    with tile.TileContext(nc) as tc, ctx:
        const = ctx.enter_context(tc.tile_pool(name="const", bufs=1))
        late = ctx.enter_context(tc.tile_pool(name="late", bufs=1))
        wk = ctx.enter_context(tc.tile_pool(name="wk", bufs=2))
        scr = ctx.enter_context(tc.tile_pool(name="scr", bufs=1))
        psS = ctx.enter_context(tc.tile_pool(name="psS", bufs=1, space="PSUM"))

        # ----- constants -----
        identb = const.tile([128, 128], BF16)
        make_identity(nc, identb)
        iota = const.tile([128, 128], BF16)
        nc.sync.dma_start(out=iota, in_=iota_t[:, :])
        wih = const.tile([128, 4, 384], BF16)
        nc.sync.dma_start(out=wih, in_=W_ihT[:, :, :])
        whh = const.tile([128, 384], BF16)
        nc.sync.dma_start(out=whh, in_=W_hhT[:, :])
        bs = const.tile([128, 2], FP32)
        nc.sync.dma_start(out=bs, in_=bsum[:, :])
        bh2 = const.tile([128, 1], FP32)
        nc.sync.dma_start(out=bh2, in_=b_hh2[:, :])
        bi2 = const.tile([128, 1], FP32)
        nc.sync.dma_start(out=bi2, in_=b_ih2[:, :])
        pw = const.tile([128, 128], BF16)
        nc.sync.dma_start(out=pw, in_=pWt[:, :])
        pbt = const.tile([128, 1], FP32)
        nc.sync.dma_start(out=pbt, in_=pb[:, :])
        wrow = const.tile([1, 128], FP32)
        nc.sync.dma_start(out=wrow, in_=w_row[:, :])
        bpi = const.tile([128, 1], FP32)
        nc.sync.dma_start(out=bpi, in_=bpi_col[:, :])
        meta = const.tile([128, 2, NT], FP32)
        nc.sync.dma_start(out=meta, in_=ev_meta[:, :, :])
        ones_col = const.tile([128, 1], BF16)
        nc.vector.memset(ones_col, 1.0)
        thirty = const.tile([1, 128], BF16)
        nc.vector.memset(thirty, 30.0)
        eps1 = const.tile([1, 1], FP32)
        nc.vector.memset(eps1, 1e-12)

        # centroid norms (device, overlaps with first group loads)
        cen = const.tile([128, C], FP32)
        nc.sync.dma_start(out=cen, in_=cenT[:, :])
        censq = scr.tile([128, C], BF16, tag="censq")
        nc.vector.tensor_mul(censq, cen, cen)
        ps_c = psS.tile([1, 448], FP32, tag="srow")
        nc.tensor.matmul(ps_c[:, 0:C], ones_col, censq, start=True, stop=True)
        cnorm = scr.tile([1, C], FP32, tag="cnorm")
        nc.scalar.activation(cnorm, ps_c[:, 0:C], AF.Sqrt)
        nc.vector.tensor_scalar_add(cnorm, cnorm, 1e-8)
        crecf = scr.tile([1, C], FP32, tag="crecf")
        nc.vector.reciprocal(crecf, cnorm)
        crec = scr.tile([1, C], BF16, tag="crec")
        nc.vector.tensor_copy(crec, crecf)
        nc.sync.dma_start(out=crec_dram[:, :], in_=crec)
        crec_rep = const.tile([128, C], BF16)
        nc.sync.dma_start(out=crec_rep, in_=_bcast_row(crec_dram, C))
        cennT = const.tile([128, C], BF16)
        nc.vector.tensor_mul(cennT, cen, crec_rep)

        # ----- long-lived tensors -----
        simT = late.tile([128, 2, L], BF16)
        rm_part = late.tile([128, 2, NG], FP32)
        nodep = ctx.enter_context(tc.tile_pool(name="nodep", bufs=1))
        nm_node = nodep.tile([128, L // 128, 128], BF16)

        if True:

            # ================= fused group loop =================
            with tc.tile_pool(name="grp", bufs=2) as grp, \
                    tc.tile_pool(name="grpc", bufs=2) as grpc, \
                    tc.tile_pool(name="grp1", bufs=1) as grp1, \
                    tc.tile_pool(name="psA", bufs=1, space="PSUM") as psA, \
                    tc.tile_pool(name="psO", bufs=1, space="PSUM") as psO, \
                    tc.tile_pool(name="psG", bufs=1, space="PSUM") as psG:
                for g in range(NG):
                    t0 = g * GT
                    n0 = g * GN
                    nsl = bass.ds(n0, GN)
                    # --- event loads ---
                    xg = grp.tile([128, GT, 256], BF16, tag="xg")
                    nc.sync.dma_start(out=xg, in_=ev_x[:, t0:t0 + GT, :])
                    xgt = grp.tile([128, GT, 128], BF16, tag="xgt")
                    if debug == 2:
                        nc.sync.dma_start(out=dbg['xg'][:, t0:t0 + GT, :], in_=xg)
                    dtr = grpc.tile([1, GS], FP32, tag="dtr")
                    nc.sync.dma_start(out=dtr, in_=dt_row[g:g + 1, :])
                    # --- time encode: outer product + magic-number range
                    # reduce + Sin + xbar transpose ---
                    tencT = grp1.tile([128, GS], BF16, tag="tencT")
                    for q in range(4):
                        c0 = q * 448
                        ang = grp1.tile([128, 448], FP32, tag="ang",
                                        name=f"ang{g}{q}")
                        po = psO.tile([128, 448], FP32, tag="po",
                                      name=f"po{g}_{q}")
                        nc.tensor.matmul(po, wrow, dtr[:, c0:c0 + 448],
                                         start=True, stop=True)
                        nc.scalar.activation(ang, po, AF.Identity,
                                             bias=bpi[:, 0:1])
                        m1 = grp1.tile([128, 448], FP32, tag="m1",
                                       name=f"m1_{g}{q}")
                        nc.vector.tensor_scalar(m1, ang, 1.0 / TWO_PI, MAGIC,
                                                op0=ALU.mult, op1=ALU.add)
                        nc.vector.tensor_scalar_add(m1, m1, -MAGIC)
                        nc.vector.scalar_tensor_tensor(ang, m1, -TWO_PI, ang,
                                                       op0=ALU.mult, op1=ALU.add)
                        nc.scalar.activation(tencT[:, c0:c0 + 448],
                                             ang, AF.Sin)
                    nc.sync.dma_start_transpose(xgt, tencT)
                    # --- aggregation ---
                    agg = grp1.tile([128, 3, GN], BF16, tag="agg")
                    for w in range(GW):
                        pas = [psA.tile([128, 128], FP32, tag=f"agg{fc}",
                                        name=f"pa{g}_{w}_{fc}")
                               for fc in range(3)]
                        for t_ in range(2):
                            ti = w * 2 + t_
                            oh = grpc.tile([128, 128], BF16, tag="oh")
                            nc.vector.tensor_scalar(
                                oh, iota, meta[:, 0, t0 + ti:t0 + ti + 1],
                                meta[:, 1, t0 + ti:t0 + ti + 1],
                                op0=ALU.is_equal, op1=ALU.mult)
                            for fc in range(3):
                                xsrc = (xg[:, ti, fc * 128:(fc + 1) * 128]
                                        if fc < 2 else xgt[:, ti, :])
                                nc.tensor.matmul(
                                    pas[fc], xsrc,
                                    oh, start=(t_ == 0), stop=(t_ == 1))
                        for fc in range(3):
                            if (w + fc) % 2 == 0:
                                nc.vector.tensor_copy(
                                    agg[:, fc, w * 128:(w + 1) * 128], pas[fc])
                            else:
                                nc.scalar.activation(
                                    agg[:, fc, w * 128:(w + 1) * 128], pas[fc],
                                    AF.Identity)
                    if debug == 2:
                        aggf = grpc.tile([128, 3, GN], FP32, tag="aggf")
                        nc.vector.tensor_copy(aggf, agg)
                        nc.sync.dma_start(out=dbg['aggT'][:, :, nsl], in_=aggf)
                        tencf = grpc.tile([128, GS], FP32, tag="tencf")
                        nc.vector.tensor_copy(tencf, tencT)
                        nc.sync.dma_start(
                            out=dbg['tencT'][:, g * GS:(g + 1) * GS], in_=tencf)
                    # --- GRU ---
                    mg = grp.tile([128, GN], BF16, tag="mg")
                    nc.sync.dma_start(out=mg, in_=mem_bf[:, nsl])
                    nhg = grp1.tile([1, GN], BF16, tag="nhg")
                    nc.sync.dma_start(out=nhg, in_=nothas[g:g + 1, :])
                    rz = grp1.tile([128, 2, GN], BF16, tag="rz")
                    ng_t = grp1.tile([128, GN], BF16, tag="ng")
                    gh2s = grp1.tile([128, GN], BF16, tag="gh2s")
                    for h in range(2):
                        hs = bass.ds(h * 448, 448)
                        gi0 = psG.tile([128, 448], FP32, tag="gi0", name=f"gi0_{g}{h}")
                        gi1 = psG.tile([128, 448], FP32, tag="gi1", name=f"gi1_{g}{h}")
                        gi2 = psG.tile([128, 448], FP32, tag="gi2", name=f"gi2_{g}{h}")
                        gis = [gi0, gi1, gi2]
                        for m in range(3):
                            nc.tensor.matmul(gis[m], wih[:, 0, m * 128:(m + 1) * 128],
                                             mg[:, hs], start=True, stop=False)
                            for k in range(1, 4):
                                nc.tensor.matmul(
                                    gis[m], wih[:, k, m * 128:(m + 1) * 128],
                                    agg[:, k - 1, hs], start=False, stop=False)
                            if m < 2:
                                nc.tensor.matmul(gis[m], whh[:, m * 128:(m + 1) * 128],
                                                 mg[:, hs], start=False,
                                                 stop=(m == 0))
                        # z-gate +30*(1-has) (keeps memory where no events)
                        nc.tensor.matmul(gi1, thirty, nhg[:, hs],
                                         start=False, stop=True)
                        nc.scalar.activation(rz[:, 0, hs], gi0, AF.Sigmoid,
                                             bias=bs[:, 0:1])
                        nc.scalar.activation(rz[:, 1, hs], gi1, AF.Sigmoid,
                                             bias=bs[:, 1:2])
                        gh2 = psG.tile([128, 448], FP32, tag="gi0",
                                       name=f"gh2_{g}{h}")
                        nc.tensor.matmul(gh2, whh[:, 256:384], mg[:, hs],
                                         start=True, stop=True)
                        nc.scalar.activation(gh2s[:, hs], gh2, AF.Identity,
                                             bias=bh2[:, 0:1])
                        nc.vector.tensor_mul(gh2s[:, hs], rz[:, 0, hs],
                                             gh2s[:, hs])
                        nc.tensor.matmul(gi2, identb, gh2s[:, hs],
                                         start=False, stop=True)
                        nc.scalar.activation(ng_t[:, hs], gi2, AF.Tanh,
                                             bias=bi2[:, 0:1])
                    # newmem = n + z*(mem - n)
                    d_t = grp1.tile([128, GN], BF16, tag="d_t")
                    nc.vector.tensor_sub(d_t, mg, ng_t)
                    nc.vector.tensor_mul(d_t, rz[:, 1, :], d_t)
                    nmg = grpc.tile([128, GN], BF16, tag="nmg")
                    nc.vector.tensor_add(nmg, ng_t, d_t)
                    nc.sync.dma_start_transpose(
                        nm_node[:, g * GW:(g + 1) * GW, :], nmg)
                    if debug == 2:
                        nmgf = grpc.tile([128, GN], FP32, tag="nmgf")
                        nc.vector.tensor_copy(nmgf, nmg)
                        nc.sync.dma_start(out=dbg['newmemT'][:, nsl], in_=nmgf)
                    # feat = newmem + node_features (in-place over nfg)
                    nfg = grp.tile([128, GN], BF16, tag="nfg")
                    nc.sync.dma_start(out=nfg, in_=nf_bf[:, nsl])
                    ftg = nfg
                    nc.vector.tensor_add(ftg, nmg, nfg)
                    # --- projection + row norms ---
                    pfc = grp1.tile([128, GN], FP32, tag="pfc")
                    sqb = grp1.tile([128, GN], BF16, tag="sqb")
                    for h in range(2):
                        hs = bass.ds(h * 448, 448)
                        ppf = psG.tile([128, 448], FP32, tag="gi0", name=f"ppf{g}{h}")
                        nc.tensor.matmul(ppf, pw, ftg[:, hs], start=True, stop=True)
                        nc.scalar.activation(pfc[:, hs], ppf, AF.Identity,
                                             bias=pbt[:, 0:1])
                    nc.vector.tensor_mul(sqb, pfc, pfc)
                    rnb = grp1.tile([1, GN], BF16, tag="rnb")
                    for h in range(2):
                        hs = bass.ds(h * 448, 448)
                        pss = psS.tile([1, 448], FP32, tag="srow", name=f"pss{g}{h}")
                        nc.tensor.matmul(pss, ones_col, sqb[:, hs],
                                         start=True, stop=True)
                        rnf = grp1.tile([1, 448], FP32, tag="rnf")
                        nc.scalar.activation(rnf, pss, AF.Sqrt)
                        nc.vector.tensor_scalar_add(rnf, rnf, 1e-8)
                        rnr = grp1.tile([1, 448], FP32, tag="rnr")
                        nc.vector.reciprocal(rnr, rnf)
                        nc.vector.tensor_copy(rnb[:, hs], rnr)
                    nc.sync.dma_start(out=rnorm_dram[0, nsl], in_=rnb)
                    rep = grp1.tile([128, GN], BF16, tag="rep")
                    nc.sync.dma_start(out=rep, in_=_bcast_row(rnorm_dram, GN, off=n0))
                    pfng = grpc.tile([128, GN], BF16, tag="pfn")
                    nc.vector.tensor_mul(pfng, pfc, rep)
                    # --- similarity (448-col PSUM chunks) ---
                    for m in range(2):
                        for h in range(2):
                            hl = bass.ds(h * 448, 448)
                            hs = bass.ds(n0 + h * 448, 448)
                            psm = psG.tile([128, 448], FP32, tag="gi1",
                                           name=f"psm{g}{m}{h}")
                            nc.tensor.matmul(psm, cennT[:, m * 128:(m + 1) * 128],
                                             pfng[:, hl], start=True, stop=True)
                            if m == 0:
                                nc.vector.tensor_copy(simT[:, m, hs], psm)
                            else:
                                nc.scalar.activation(simT[:, m, hs], psm,
                                                     AF.Identity)
                    nc.vector.tensor_reduce(rm_part[:, :, g], simT[:, :, nsl],
                                            axis=AX.X, op=ALU.max)
            # group pools freed
            if debug:
                nc.sync.dma_start(out=dbg['simT'][:, :, :], in_=simT)

            # row max over group partials + AllReduce(max)
            rm4 = scr.tile([128, 4], FP32, tag="rm4")
            nc.vector.tensor_reduce(rm4[:, 0:2], rm_part, axis=AX.X, op=ALU.max)
            nc.vector.tensor_copy(rm4[:, 2:4], rm4[:, 0:2])
            rmg = scr.tile([128, 4], FP32, tag="rmg")
            allreduce(ALU.max, rm_l, rm_a, rm4, rmg)

        # (sim_node/nm_node filled per group above)

        with tc.tile_pool(name="slv", bufs=1) as slv, \
                tc.tile_pool(name="psC", bufs=1, space="PSUM") as psC:
            sim_node = slv.tile([128, L // 128, 256], BF16)
            for m in range(2):
                nc.sync.dma_start_transpose(
                    sim_node[:, :, m * 128:(m + 1) * 128], simT[:, m, :])
            # ===== nc secant (tau over C=256 per node) =====
            # g-eval: relu(x - t) == max(x, t) - t; accum_out reduces with
            # op1 (add) -> per-partition sum.
            junk_n = slv.tile([128, C], BF16)
            junk_n2 = slv.tile([128, C], BF16)
            zer_n = slv.tile([128, C], BF16)
            nc.vector.memset(zer_n, 0.0)
            NW = L // 128
            tau = slv.tile([128, NW], FP32)
            tau_p = slv.tile([128, NW], FP32)
            g_c = slv.tile([128, NW], FP32)
            g_p = slv.tile([128, NW], FP32)

            def nc_eval(tau_tile, g_tile):
                for ch in range(NW):
                    jt = junk_n if ch % 2 == 0 else junk_n2
                    nc.vector.scalar_tensor_tensor(
                        jt, sim_node[:, ch, :], tau_tile[:, ch:ch + 1], zer_n,
                        op0=ALU.subtract, op1=ALU.max,
                        accum_out=g_tile[:, ch:ch + 1])

            nc.vector.memset(tau_p, -2.0)
            nc_eval(tau_p, g_p)
            st1 = wk.tile([128, NW], FP32, tag="st1")
            nc.vector.tensor_scalar(st1, g_p, -1.0, 1.0 / 256.0,
                                    op0=ALU.add, op1=ALU.mult)
            nc.vector.tensor_add(tau, tau_p, st1)

            def secant_update(tt, tp, gg, gp, wtag, shape):
                num = wk.tile(shape, FP32, tag=wtag + "n")
                nc.vector.tensor_sub(num, tt, tp)
                gm1 = wk.tile(shape, FP32, tag=wtag + "g")
                nc.vector.tensor_scalar_add(gm1, gg, -1.0)
                nc.vector.tensor_mul(num, num, gm1)
                den = wk.tile(shape, FP32, tag=wtag + "d")
                nc.vector.tensor_sub(den, gp, gg)
                nc.vector.tensor_scalar_max(den, den, 1e-12)
                rden = wk.tile(shape, FP32, tag=wtag + "r")
                nc.vector.reciprocal(rden, den)
                nc.vector.tensor_copy(tp, tt)
                nc.vector.tensor_copy(gp, gg)
                stp = wk.tile(shape, FP32, tag=wtag + "s")
                nc.vector.tensor_mul(stp, num, rden)
                nc.vector.tensor_scalar(stp, stp, 0.0, 1.0,
                                        op0=ALU.max, op1=ALU.min)
                nc.vector.tensor_add(tt, tt, stp)

            def nc_iter(k):
                nc_eval(tau, g_c)
                secant_update(tau, tau_p, g_c, g_p, "ncs", [128, NW])

            # ===== cn bracket solver =====
            pos = slv.tile([128, 2, KX], FP32)
            gv = slv.tile([128, 2, KX], FP32)
            lo = slv.tile([128, 2], FP32)
            hi = slv.tile([128, 2], FP32)
            glo = slv.tile([128, 2], FP32)
            ghi = slv.tile([128, 2], FP32)
            junkL = slv.tile([128, L], BF16)
            junkL2 = junkL
            neg = slv.tile([128, 2, KX], FP32)

            def cn_probes(round_idx, nk=CN_K):
                nc.vector.tensor_scalar_mul(neg, pos, -1.0)
                for m in range(2):
                    for k in range(nk):
                        jt = junkL if k % 2 == 0 else junkL2
                        nc.scalar.activation(
                            jt, simT[:, m, :], AF.Relu,
                            bias=neg[:, m, 1 + k:2 + k],
                            accum_out=gv[:, m, 1 + k:2 + k])

            def cn_select():
                # shift masked (g>=1) positions by +8; the bracket ends are
                # argmax/argmin in shifted space; g values extracted by
                # bitwise-consistent is_equal one-hots (no magnitude tricks).
                msk = wk.tile([128, 2, KX], FP32, tag="msk")
                nc.vector.tensor_scalar(msk, gv, 1.0, None, op0=ALU.is_ge)
                tsel = wk.tile([128, 2, KX], FP32, tag="tsel")
                nc.vector.scalar_tensor_tensor(tsel, msk, 8.0, pos,
                                               op0=ALU.mult, op1=ALU.add)
                lo8 = wk.tile([128, 2], FP32, tag="lo8")
                nc.vector.tensor_reduce(lo8, tsel, axis=AX.X, op=ALU.max)
                hi8 = wk.tile([128, 2], FP32, tag="hi8")
                nc.vector.tensor_reduce(hi8, tsel, axis=AX.X, op=ALU.min)
                ohl = wk.tile([128, 2, KX], FP32, tag="ohl")
                sel = wk.tile([128, 2, KX], FP32, tag="sel")
                for m in range(2):
                    nc.vector.tensor_scalar(ohl[:, m, :], tsel[:, m, :],
                                            lo8[:, m:m + 1], None,
                                            op0=ALU.is_equal)
                nc.vector.tensor_mul(sel, gv, ohl)
                nc.vector.tensor_reduce(glo, sel, axis=AX.X, op=ALU.max)
                for m in range(2):
                    nc.vector.tensor_scalar(ohl[:, m, :], tsel[:, m, :],
                                            hi8[:, m:m + 1], None,
                                            op0=ALU.is_equal)
                nc.vector.tensor_mul(sel, gv, ohl)
                nc.vector.tensor_reduce(ghi, sel, axis=AX.X, op=ALU.max)
                nc.vector.tensor_scalar_add(lo, lo8, -8.0)
                nc.vector.tensor_copy(hi, hi8)

            # round 1 positions from global row max
            for k in range(CN_K):
                nc.vector.tensor_scalar_add(pos[:, 0, 1 + k:2 + k],
                                            rmg[:, 0:1], -CN_FR1[k])
                nc.vector.tensor_scalar_add(pos[:, 1, 1 + k:2 + k],
                                            rmg[:, 1:2], -CN_FR1[k])
            for m in range(2):
                nc.vector.tensor_scalar_add(pos[:, m, 0:1],
                                            rmg[:, m:m + 1], -1.0)
                nc.vector.tensor_copy(pos[:, m, KX - 1:KX], rmg[:, m:m + 1])
            nc.vector.memset(gv[:, :, 0:1], 2.0)
            nc.vector.memset(gv[:, :, KX - 1:KX], 0.0)

            cn_probes(0)
            nc_iter(0)
            nc_iter(1)
            gvg = wk.tile([128, 2, CN_K], FP32, tag="gvg")
            allreduce(ALU.add, gp_l[0], gp_a[0], gv[:, :, 1:KX - 1], gvg)
            nc.vector.tensor_copy(gv[:, :, 1:KX - 1], gvg)
            cn_select()
            # round 2: 5 uniform probes; entries 6,7 duplicate hi (g=0,
            # is_equal ties in the select resolve via max)
            K2 = 5
            w2 = wk.tile([128, 2], FP32, tag="w2")
            nc.vector.tensor_sub(w2, hi, lo)
            for k in range(K2):
                nc.vector.scalar_tensor_tensor(
                    pos[:, :, 1 + k:2 + k], w2, (k + 1.0) / (K2 + 1), lo,
                    op0=ALU.mult, op1=ALU.add)
            for k in range(K2, CN_K):
                nc.vector.tensor_copy(pos[:, :, 1 + k:2 + k], hi)
            nc.vector.memset(gv[:, :, 1 + K2:KX - 1], 0.0)
            nc.vector.tensor_copy(pos[:, :, 0:1], lo)
            nc.vector.tensor_copy(pos[:, :, KX - 1:KX], hi)
            nc.vector.tensor_copy(gv[:, :, 0:1], glo)
            nc.vector.tensor_copy(gv[:, :, KX - 1:KX], ghi)

            cn_probes(1, nk=K2)
            nc_iter(2)
            nc_iter(3)
            allreduce(ALU.add, gp_l[1], gp_a[1], gv[:, :, 1:KX - 1], gvg)
            nc.vector.tensor_copy(gv[:, :, 1:KX - 1], gvg)
            cn_select()
            nc_iter(4)
            nc_iter(5)
            # secant interpolation: ctau = lo + clip((glo-1)/(glo-ghi)) * (hi-lo)
            ctau = slv.tile([128, 2], FP32)
            num2 = wk.tile([128, 2], FP32, tag="num2")
            nc.vector.tensor_scalar_add(num2, glo, -1.0)
            den2 = wk.tile([128, 2], FP32, tag="den2")
            nc.vector.tensor_sub(den2, glo, ghi)
            nc.vector.tensor_scalar_max(den2, den2, 1e-9)
            rd2 = wk.tile([128, 2], FP32, tag="rd2")
            nc.vector.reciprocal(rd2, den2)
            frac = wk.tile([128, 2], FP32, tag="frac")
            nc.vector.tensor_mul(frac, num2, rd2)
            nc.vector.tensor_scalar(frac, frac, 0.0, 1.0, op0=ALU.max, op1=ALU.min)
            nc.vector.tensor_sub(w2, hi, lo)
            nc.vector.tensor_mul(frac, frac, w2)
            nc.vector.tensor_add(ctau, lo, frac)
            if debug:
                nc.sync.dma_start(out=dbg['taucn'][:, :], in_=ctau)
                nc.sync.dma_start(out=dbg['taunc'][:, :], in_=tau)

            # taunc -> DRAM row for the phase-8 broadcast
            tau_b = wk.tile([128, NW], BF16, tag="tau_b")
            nc.vector.tensor_copy(tau_b, tau)
            nc.sync.dma_start(
                out=taunc_dram.ap().rearrange("w p -> p w"), in_=tau_b)

            # ===== c_memory: relu in simT layout, xbar transpose, matmul =====
            ps_cms = [psC.tile([128, 128], FP32, tag=f"cm{m}", name=f"pscm{m}")
                      for m in range(2)]
            for m in range(2):
                jt = junkL if m == 0 else junkL2
                nc.vector.tensor_scalar(
                    jt, simT[:, m, :], ctau[:, m:m + 1], 0.0,
                    op0=ALU.subtract, op1=ALU.max)
                nc.sync.dma_start_transpose(
                    sim_node[:, :, m * 128:(m + 1) * 128], jt)
            for ch in range(NW):
                for m in range(2):
                    nc.tensor.matmul(
                        ps_cms[m], sim_node[:, ch, m * 128:(m + 1) * 128],
                        nm_node[:, ch, :], start=(ch == 0), stop=(ch == NW - 1))
            cmf = wk.tile([128, 2, 128], FP32, tag="cmf")
            for m in range(2):
                nc.vector.tensor_copy(cmf[:, m, :], ps_cms[m])
            cmgf = wk.tile([128, 2, 128], FP32, tag="cmgf")
            allreduce(ALU.add, cm_local, cm_all, cmf, cmgf,
                      in_ap=cm_local.ap().rearrange("(m p) d -> p m d", p=128),
                      out_ap=cm_all.ap().rearrange("(m p) d -> p m d", p=128))
            cmg = scr.tile([128, 2, 128], BF16, tag="cmg")
            nc.vector.tensor_copy(cmg, cmgf)
            if debug:
                nc.sync.dma_start(
                    out=dbg['cmem'].ap().rearrange("(m p) d -> p m d", p=128),
                    in_=cmgf)
        # sim_node freed after c_memory (slv pool closed; nodep closes below)

        # ===== embedding =====
        with tc.tile_pool(name="embp", bufs=2) as embp, \
                tc.tile_pool(name="psZ", bufs=2, space="PSUM") as psZ:
            NW = L // 128
            tnc = const.tile([128, L], BF16)
            nc.sync.dma_start(out=tnc, in_=_bcast_row(taunc_dram, L))
            wb = 0
            while wb < NW:
                nwin = min(4, NW - wb)
                bsl = bass.ds(wb * 128, nwin * 128)
                ncm = embp.tile([128, 2, 512], BF16, tag="ncm")
                for m in range(2):
                    nc.vector.tensor_sub(ncm[:, m, 0:nwin * 128],
                                         simT[:, m, bsl], tnc[:, bsl])
                nc.vector.tensor_scalar_max(ncm[:, :, 0:nwin * 128],
                                            ncm[:, :, 0:nwin * 128], 0.0)
                ps_z = psZ.tile([128, 4, 128], FP32, tag="z")
                for k in range(nwin):
                    w = wb + k
                    for m in range(2):
                        nc.tensor.matmul(
                            ps_z[:, k, :], ncm[:, m, k * 128:(k + 1) * 128],
                            cmg[:, m, :], start=(m == 0), stop=False)
                    nc.tensor.matmul(ps_z[:, k, :], identb, nm_node[:, w, :],
                                     start=False, stop=True)
                emb_c = embp.tile([128, 4, 128], FP32, tag="emb_c")
                nc.vector.tensor_copy(emb_c[:, 0:nwin, :], ps_z[:, 0:nwin, :])
                nc.sync.dma_start(
                    out=emb_out[wb * 128:(wb + nwin) * 128, :].rearrange(
                        "(k p) d -> p k d", p=128),
                    in_=emb_c[:, 0:nwin, :])
                wb += nwin

    split_waits(nc)
    return nc


# ----------------------------------------------------------------------------
# host side
# ----------------------------------------------------------------------------

_CACHE = {}


def _route(L, src, dst, t):
    idx = np.concatenate([src, dst]).astype(np.int64)
    other = np.concatenate([dst, src]).astype(np.int64)
    tt = np.concatenate([t, t])
    eidx = np.concatenate([np.arange(len(src)), np.arange(len(src))])
    NW = L // 128
    order = np.argsort(idx, kind='stable')
    idx_s, other_s, tt_s, eidx_s = idx[order], other[order], tt[order], eidx[order]
    owner = idx_s // L
    cores = []
    for c in range(NCORES):
        msk = owner == c
        li = idx_s[msk] - c * L
        win = li // 128
        col = li % 128
        wcount = np.bincount(win, minlength=NW)
        assert wcount.max() <= 256, f"window overflow: {wcount.max()}"
        woff = np.zeros(NW + 1, np.int64)
        woff[1:] = np.cumsum(wcount)
        within = np.arange(len(li)) - woff[win]
        slot = win * 256 + within
        cores.append(dict(slot=slot, col=col, li=li, other=other_s[msk],
                          tt=tt_s[msk], eidx=eidx_s[msk]))
    return cores


def kernel(**inputs):
    node_memory = np.asarray(inputs['node_memory'])
    last_update = np.asarray(inputs['last_update'])
    node_features = np.asarray(inputs['node_features'])
    event_feat = np.asarray(inputs['event_feat'])
    t = np.asarray(inputs['t'])
    src = np.asarray(inputs['src']).astype(np.int64)
    dst = np.asarray(inputs['dst']).astype(np.int64)
    time_w = np.asarray(inputs['time_w'])
    time_b = np.asarray(inputs['time_b'])
    W_ih = np.asarray(inputs['W_ih'])
    b_ih = np.asarray(inputs['b_ih'])
    W_hh = np.asarray(inputs['W_hh'])
    b_hh = np.asarray(inputs['b_hh'])
    proj_W = np.asarray(inputs['proj_W'])
    proj_b = np.asarray(inputs['proj_b'])
    centroids = np.asarray(inputs['centroids'])

    Nn = node_memory.shape[0]
    gran = 128 * GW * NCORES          # L must be a multiple of 128*GW
    NP = -(-Nn // gran) * gran
    L = NP // NCORES
    SLOTS = 2 * L
    NT = SLOTS // 128
    NG = L // GN

    nmp = np.zeros((NP, D), np.float32); nmp[:Nn] = node_memory
    nfp = np.zeros((NP, D), np.float32); nfp[:Nn] = node_features
    lup = np.zeros(NP, np.float32); lup[:Nn] = last_update

    idx_full = np.concatenate([src, dst])
    cnt_full = np.bincount(idx_full, minlength=NP).astype(np.float32)
    icnt_full = 1.0 / np.maximum(cnt_full, 1.0)
    nothas_full = (cnt_full == 0).astype(np.float32)

    cores = _route(L, src, dst, t)
    bsum_h = f32c(np.stack([(b_ih + b_hh)[0:128], (b_ih + b_hh)[128:256]], 1))
    wih_h = bfc(W_ih.T.reshape(4, 128, 384).transpose(1, 0, 2))

    in_maps = []
    for c in range(NCORES):
        r = cores[c]
        sl = r['slot']
        p_i = sl % 128
        t_i = sl // 128
        ev_x = np.zeros((128, NT, 256), ml_dtypes.bfloat16)
        ev_x[p_i, t_i, 0:128] = nmp[r['other']].astype(ml_dtypes.bfloat16)
        ev_x[p_i, t_i, 128:256] = event_feat[r['eidx']].astype(ml_dtypes.bfloat16)
        ev_meta = np.zeros((128, 2, NT), np.float32)
        ev_meta[:, 0, :] = -1.0
        ev_meta[p_i, 0, t_i] = r['col'].astype(np.float32)
        ev_meta[p_i, 1, t_i] = icnt_full[r['li'] + c * L]
        dt_flat = np.zeros(SLOTS, np.float32)
        dt_flat[sl] = r['tt'] - lup[r['li'] + c * L]
        nsl = slice(c * L, (c + 1) * L)
        in_maps.append({
            'ev_x': ev_x,
            'ev_meta': ev_meta,
            'dt_row': f32c(dt_flat.reshape(NG, GS)),
            'mem_bf': bfc(nmp[nsl].T),
            'nf_bf': bfc(nfp[nsl].T),
            'nothas': bfc(nothas_full[nsl].reshape(NG, GN)),
            'W_ihT': wih_h,
            'W_hhT': bfc(W_hh.T),
            'bsum': bsum_h,
            'b_hh2': f32c(b_hh[256:384].reshape(128, 1)),
            'b_ih2': f32c(b_ih[256:384].reshape(128, 1)),
            'pWt': bfc(proj_W),
            'pb': f32c(proj_b.reshape(128, 1)),
            'cenT': f32c(centroids.T),
            'w_row': f32c(time_w.reshape(1, 128)),
            'bpi_col': f32c((time_b + HALF_PI).reshape(128, 1)),
            'iota_t': bfc(np.tile(np.arange(128, dtype=np.float32)[None, :],
                                  (128, 1))),
        })

    debug = int(os.environ.get("KERNEL_DEBUG", "0"))
    key = (L, debug)
    if key not in _CACHE:
        _CACHE[key] = build_program(L, debug=debug)
    nc = _CACHE[key]
    res = run_bass_kernel_spmd(nc, in_maps, list(range(NCORES)))
    emb = np.concatenate([res.results[c]['emb'] for c in range(NCORES)], 0)
    kernel._last_exec_ns = getattr(res, 'exec_time_ns', None)
    if debug:
        kernel._last_results = res.results
    return emb[:Nn].astype(np.float32)


# revision 14
# speedup vs baseline: 1.1008x; 1.0080x over previous
"""TGN-style GNN message passing + community detection on 8 TRN2 NeuronCores.

Node-sharded SPMD, v2 (engine-balanced rewrite):
- Fused per-group pipeline (events -> agg -> GRU -> feat -> proj -> sim)
  with SBUF-resident intermediates; event tensors host-packed so every
  load is one large contiguous-per-partition DMA.
- Time encoding via a 1-partition PE outer product + Activation Sin with
  per-partition bias, landing in [feat, slot] layout, then xbar DMA
  transpose into the event matrix (no DVE work).
- has-mask folded into the GRU z-gate via a +30*(1-has) rank-1 matmul
  (sigmoid saturates to 1 -> memory passthrough), removing all blend ops.
- cn-sparsemax tau via 2 rounds of multi-probe bracketing (7 nonuniform +
  7 uniform probes, fused sub+relu+sum DVE ops in 4x bf16 mode) + secant
  interpolation: 3 AllReduces instead of 13.
- nc-sparsemax tau via secant from tau0=-2 (8 iterations, per-window
  fused DVE ops); overlapped with the cn AllReduce latency.
- c_memory via relu applied in simT layout then xbar-transposed;
  AllReduce #4. Total 4 collectives.
"""

import os
from contextlib import ExitStack

import numpy as np
import ml_dtypes

import concourse.bass as bass
import concourse.mybir as mybir
import concourse.tile as tile
from concourse.bass_utils import run_bass_kernel_spmd
from concourse.masks import make_identity

FP32 = mybir.dt.float32
BF16 = mybir.dt.bfloat16
AF = mybir.ActivationFunctionType
ALU = mybir.AluOpType
AX = mybir.AxisListType

NCORES = 8
D = 128
C = 256
HALF_PI = float(np.pi / 2)
TWO_PI = float(2 * np.pi)
MAGIC = 12582912.0
GW = 7                      # windows per group
GN = GW * 128               # nodes per group (896)
GS = 2 * GN                 # event slots per group (1792)
GT = GS // 128              # slot-tiles per group (14)

# cn bracket probe offsets below rowmax (round 1, ascending positions)
CN_FR1 = [0.25, 0.17, 0.12, 0.08, 0.05, 0.02]
CN_K = len(CN_FR1)          # 7 probes per round
KX = CN_K + 2               # probe array incl. bracket ends
CN_F2 = [(k + 1) / (CN_K + 1) for k in range(CN_K)]
BIGV = 1.0e4
NIT_NC = 6

bfc = lambda x: np.ascontiguousarray(np.asarray(x).astype(ml_dtypes.bfloat16))
f32c = lambda x: np.ascontiguousarray(np.asarray(x).astype(np.float32))


def _bcast_row(dram_tensor, ncols, nparts=128, off=0):
    row = dram_tensor.ap()
    return bass.AP(tensor=row.tensor, offset=row.offset + off,
                   ap=[[0, nparts], [1, ncols]])


def split_waits(nc, sp_limit=1, default_limit=1):
    """This env's walrus rejects >1 sync-wait on SP CTRL instructions:
    move extra waits onto preceding NOPs."""
    limits = {mybir.EngineType.SP: sp_limit}
    for fn in nc.m.functions:
        for bb in fn.blocks:
            out = []
            for ins in bb.instructions:
                si = ins.sync_info
                w = list(si.on_wait) if (si is not None and si.on_wait) else []
                lim = limits.get(ins.engine, default_limit)
                if len(w) > lim:
                    extra, keep = w[:-lim], w[-lim:]
                    for j in range(0, len(extra), lim):
                        out.append(mybir.InstNoOp(
                            name=f"{ins.name}-ws{j}",
                            engine=ins.engine,
                            sync_info=mybir.SyncInfo(
                                on_wait=list(extra[j:j + lim]), on_update=[]),
                        ))
                    ins.sync_info = mybir.SyncInfo(
                        on_wait=list(keep),
                        on_update=list(si.on_update) if si.on_update else [])
                out.append(ins)
            bb.instructions = out
    return nc


def build_program(L, debug=False):
    NG = L // GN            # groups (14 for L=12544)
    NW = L // 128           # windows (98)
    NT = 2 * L // 128       # slot-tiles total (196)

    nc = bass.Bass(num_devices=NCORES)

    # ---- inputs ----
    ev_x = nc.dram_tensor("ev_x", [128, NT, 256], BF16, kind="ExternalInput")
    ev_meta = nc.dram_tensor("ev_meta", [128, 2, NT], FP32, kind="ExternalInput")
    dt_row = nc.dram_tensor("dt_row", [NG, GS], FP32, kind="ExternalInput")
    mem_bf = nc.dram_tensor("mem_bf", [128, L], BF16, kind="ExternalInput")
    nf_bf = nc.dram_tensor("nf_bf", [128, L], BF16, kind="ExternalInput")
    nothas = nc.dram_tensor("nothas", [NG, GN], BF16, kind="ExternalInput")
    W_ihT = nc.dram_tensor("W_ihT", [128, 4, 384], BF16, kind="ExternalInput")
    W_hhT = nc.dram_tensor("W_hhT", [128, 384], BF16, kind="ExternalInput")
    bsum = nc.dram_tensor("bsum", [128, 2], FP32, kind="ExternalInput")
    b_hh2 = nc.dram_tensor("b_hh2", [128, 1], FP32, kind="ExternalInput")
    b_ih2 = nc.dram_tensor("b_ih2", [128, 1], FP32, kind="ExternalInput")
    pWt = nc.dram_tensor("pWt", [128, 128], BF16, kind="ExternalInput")
    pb = nc.dram_tensor("pb", [128, 1], FP32, kind="ExternalInput")
    cenT = nc.dram_tensor("cenT", [128, C], FP32, kind="ExternalInput")
    w_row = nc.dram_tensor("w_row", [1, 128], FP32, kind="ExternalInput")
    bpi_col = nc.dram_tensor("bpi_col", [128, 1], FP32, kind="ExternalInput")
    iota_t = nc.dram_tensor("iota_t", [128, 128], BF16, kind="ExternalInput")

    emb_out = nc.dram_tensor("emb", [L, D], FP32, kind="ExternalOutput")
    dbg = {}
    if debug:
        dbg['newmemT'] = nc.dram_tensor("dbg_newmemT", [128, L], FP32, kind="ExternalOutput")
        dbg['aggT'] = nc.dram_tensor("dbg_aggT", [128, 3, L], FP32, kind="ExternalOutput")
        dbg['xg'] = nc.dram_tensor("dbg_xg", [128, 2 * L // 128, 256], BF16, kind="ExternalOutput")
        dbg['tencT'] = nc.dram_tensor("dbg_tencT", [128, 2 * L], FP32, kind="ExternalOutput")
        dbg['simT'] = nc.dram_tensor("dbg_simT", [128, 2, L], BF16, kind="ExternalOutput")
        dbg['taunc'] = nc.dram_tensor("dbg_taunc", [128, NW], FP32, kind="ExternalOutput")
        dbg['taucn'] = nc.dram_tensor("dbg_taucn", [128, 2], FP32, kind="ExternalOutput")
        dbg['cmem'] = nc.dram_tensor("dbg_cmem", [C, D], FP32, kind="ExternalOutput")

    # ---- staging DRAM ----
    crec_dram = nc.dram_tensor("crec_dram", [1, C], BF16)
    rnorm_dram = nc.dram_tensor("rnorm_dram", [1, L], BF16)
    taunc_dram = nc.dram_tensor("taunc_dram", [NW, 128], BF16)
    taucn_dram = nc.dram_tensor("taucn_dram", [2, 128], BF16)
    rm_l = nc.dram_tensor("rm_l", [128, 4], FP32)
    rm_a = nc.dram_tensor("rm_a", [128, 4], FP32, addr_space="Shared")
    gp_l = [nc.dram_tensor(f"gp_l{r}", [128, 2 * CN_K], FP32) for r in range(2)]
    gp_a = [nc.dram_tensor(f"gp_a{r}", [128, 2 * CN_K], FP32, addr_space="Shared")
            for r in range(2)]
    cm_local = nc.dram_tensor("cm_local", [C, D], FP32)
    cm_all = nc.dram_tensor("cm_all", [C, D], FP32, addr_space="Shared")
    RG = [list(range(NCORES))]

    cc_sem = nc.alloc_semaphore("cc_done")
    ccv = [0]

    def allreduce(alu_op, local_dram, shared_dram, sb_in, sb_out,
                  in_ap=None, out_ap=None):
        """Stage sb_in -> local_dram, AllReduce -> shared_dram, load sb_out."""
        with tc.tile_critical():
            nc.gpsimd.dma_start(
                out=local_dram.ap() if in_ap is None else in_ap,
                in_=sb_in).then_inc(cc_sem, 16)
            ccv[0] += 16
            nc.gpsimd.wait_ge(cc_sem, ccv[0])
            nc.gpsimd.collective_compute(
                "AllReduce", alu_op, replica_groups=RG,
                ins=[local_dram.ap().opt()],
                outs=[shared_dram.ap().opt()]).then_inc(cc_sem)
            ccv[0] += 1
            nc.gpsimd.wait_ge(cc_sem, ccv[0])
            nc.gpsimd.dma_start(
                out=sb_out,
                in_=shared_dram.ap() if out_ap is None else out_ap
            ).then_inc(cc_sem, 16)
            ccv[0] += 16
            nc.gpsimd.wait_ge(cc_sem, ccv[0])

    ctx = ExitStack()
    with tile.TileContext(nc) as tc, ctx:
        const = ctx.enter_context(tc.tile_pool(name="const", bufs=1))
        late = ctx.enter_context(tc.tile_pool(name="late", bufs=1))
        wk = ctx.enter_context(tc.tile_pool(name="wk", bufs=2))
        scr = ctx.enter_context(tc.tile_pool(name="scr", bufs=1))
        psS = ctx.enter_context(tc.tile_pool(name="psS", bufs=1, space="PSUM"))

        # ----- constants -----
        identb = const.tile([128, 128], BF16)
        make_identity(nc, identb)
        iota = const.tile([128, 128], BF16)
        nc.sync.dma_start(out=iota, in_=iota_t[:, :])
        wih = const.tile([128, 4, 384], BF16)
        nc.sync.dma_start(out=wih, in_=W_ihT[:, :, :])
        whh = const.tile([128, 384], BF16)
        nc.sync.dma_start(out=whh, in_=W_hhT[:, :])
        bs = const.tile([128, 2], FP32)
        nc.sync.dma_start(out=bs, in_=bsum[:, :])
        bh2 = const.tile([128, 1], FP32)
        nc.sync.dma_start(out=bh2, in_=b_hh2[:, :])
        bi2 = const.tile([128, 1], FP32)
        nc.sync.dma_start(out=bi2, in_=b_ih2[:, :])
        pw = const.tile([128, 128], BF16)
        nc.sync.dma_start(out=pw, in_=pWt[:, :])
        pbt = const.tile([128, 1], FP32)
        nc.sync.dma_start(out=pbt, in_=pb[:, :])
        wrow = const.tile([1, 128], FP32)
        nc.sync.dma_start(out=wrow, in_=w_row[:, :])
        bpi = const.tile([128, 1], FP32)
        nc.sync.dma_start(out=bpi, in_=bpi_col[:, :])
        meta = const.tile([128, 2, NT], FP32)
        nc.sync.dma_start(out=meta, in_=ev_meta[:, :, :])
        ones_col = const.tile([128, 1], BF16)
        nc.vector.memset(ones_col, 1.0)
        thirty = const.tile([1, 128], BF16)
        nc.vector.memset(thirty, 30.0)
        eps1 = const.tile([1, 1], FP32)
        nc.vector.memset(eps1, 1e-12)

        # centroid norms (device, overlaps with first group loads)
        cen = const.tile([128, C], FP32)
        nc.sync.dma_start(out=cen, in_=cenT[:, :])
        censq = scr.tile([128, C], BF16, tag="censq")
        nc.vector.tensor_mul(censq, cen, cen)
        ps_c = psS.tile([1, 448], FP32, tag="srow")
        nc.tensor.matmul(ps_c[:, 0:C], ones_col, censq, start=True, stop=True)
        cnorm = scr.tile([1, C], FP32, tag="cnorm")
        nc.scalar.activation(cnorm, ps_c[:, 0:C], AF.Sqrt)
        nc.vector.tensor_scalar_add(cnorm, cnorm, 1e-8)
        crecf = scr.tile([1, C], FP32, tag="crecf")
        nc.vector.reciprocal(crecf, cnorm)
        crec = scr.tile([1, C], BF16, tag="crec")
        nc.vector.tensor_copy(crec, crecf)
        nc.sync.dma_start(out=crec_dram[:, :], in_=crec)
        crec_rep = const.tile([128, C], BF16)
        nc.sync.dma_start(out=crec_rep, in_=_bcast_row(crec_dram, C))
        cennT = const.tile([128, C], BF16)
        nc.vector.tensor_mul(cennT, cen, crec_rep)

        # ----- long-lived tensors -----
        simT = late.tile([128, 2, L], BF16)
        rm_part = late.tile([128, 2, NG], FP32)
        nodep = ctx.enter_context(tc.tile_pool(name="nodep", bufs=1))
        nm_node = nodep.tile([128, L // 128, 128], BF16)

        if True:

            # ================= fused group loop =================
            with tc.tile_pool(name="grp", bufs=2) as grp, \
                    tc.tile_pool(name="grpc", bufs=2) as grpc, \
                    tc.tile_pool(name="grp1", bufs=1) as grp1, \
                    tc.tile_pool(name="psA", bufs=1, space="PSUM") as psA, \
                    tc.tile_pool(name="psO", bufs=1, space="PSUM") as psO, \
                    tc.tile_pool(name="psG", bufs=1, space="PSUM") as psG:
                for g in range(NG):
                    t0 = g * GT
                    n0 = g * GN
                    nsl = bass.ds(n0, GN)
                    # --- event loads ---
                    xg = grp.tile([128, GT, 256], BF16, tag="xg")
                    nc.sync.dma_start(out=xg, in_=ev_x[:, t0:t0 + GT, :])
                    xgt = grp.tile([128, GT, 128], BF16, tag="xgt")
                    if debug == 2:
                        nc.sync.dma_start(out=dbg['xg'][:, t0:t0 + GT, :], in_=xg)
                    dtr = grpc.tile([1, GS], FP32, tag="dtr")
                    nc.sync.dma_start(out=dtr, in_=dt_row[g:g + 1, :])
                    # --- time encode: outer product + magic-number range
                    # reduce + Sin + xbar transpose ---
                    tencT = grp1.tile([128, GS], BF16, tag="tencT")
                    for q in range(4):
                        c0 = q * 448
                        ang = grp1.tile([128, 448], FP32, tag="ang",
                                        name=f"ang{g}{q}")
                        po = psO.tile([128, 448], FP32, tag="po",
                                      name=f"po{g}_{q}")
                        nc.tensor.matmul(po, wrow, dtr[:, c0:c0 + 448],
                                         start=True, stop=True)
                        nc.scalar.activation(ang, po, AF.Identity,
                                             bias=bpi[:, 0:1])
                        m1 = grp1.tile([128, 448], FP32, tag="m1",
                                       name=f"m1_{g}{q}")
                        nc.vector.tensor_scalar(m1, ang, 1.0 / TWO_PI, MAGIC,
                                                op0=ALU.mult, op1=ALU.add)
                        nc.vector.tensor_scalar_add(m1, m1, -MAGIC)
                        nc.vector.scalar_tensor_tensor(ang, m1, -TWO_PI, ang,
                                                       op0=ALU.mult, op1=ALU.add)
                        nc.scalar.activation(tencT[:, c0:c0 + 448],
                                             ang, AF.Sin)
                    nc.sync.dma_start_transpose(xgt, tencT)
                    # --- aggregation ---
                    agg = grp1.tile([128, 3, GN], BF16, tag="agg")
                    for w in range(GW):
                        pas = [psA.tile([128, 128], FP32, tag=f"agg{fc}",
                                        name=f"pa{g}_{w}_{fc}")
                               for fc in range(3)]
                        oh2 = []
                        for t_ in range(2):
                            ti = w * 2 + t_
                            oh = grpc.tile([128, 128], BF16, tag="oh",
                                           name=f"oh{g}_{w}_{t_}")
                            nc.vector.tensor_scalar(
                                oh, iota, meta[:, 0, t0 + ti:t0 + ti + 1],
                                meta[:, 1, t0 + ti:t0 + ti + 1],
                                op0=ALU.is_equal, op1=ALU.mult)
                            oh2.append(oh)
                        # tenc-independent chains first (overlap tenc tail)
                        for fc in range(2):
                            for t_ in range(2):
                                ti = w * 2 + t_
                                nc.tensor.matmul(
                                    pas[fc], xg[:, ti, fc * 128:(fc + 1) * 128],
                                    oh2[t_], start=(t_ == 0), stop=(t_ == 1))
                        for t_ in range(2):
                            ti = w * 2 + t_
                            nc.tensor.matmul(pas[2], xgt[:, ti, :], oh2[t_],
                                             start=(t_ == 0), stop=(t_ == 1))
                        for fc in range(3):
                            if (w + fc) % 2 == 0:
                                nc.vector.tensor_copy(
                                    agg[:, fc, w * 128:(w + 1) * 128], pas[fc])
                            else:
                                nc.scalar.activation(
                                    agg[:, fc, w * 128:(w + 1) * 128], pas[fc],
                                    AF.Identity)
                    if debug == 2:
                        aggf = grpc.tile([128, 3, GN], FP32, tag="aggf")
                        nc.vector.tensor_copy(aggf, agg)
                        nc.sync.dma_start(out=dbg['aggT'][:, :, nsl], in_=aggf)
                        tencf = grpc.tile([128, GS], FP32, tag="tencf")
                        nc.vector.tensor_copy(tencf, tencT)
                        nc.sync.dma_start(
                            out=dbg['tencT'][:, g * GS:(g + 1) * GS], in_=tencf)
                    # --- GRU ---
                    mg = grp.tile([128, GN], BF16, tag="mg")
                    nc.sync.dma_start(out=mg, in_=mem_bf[:, nsl])
                    nhg = grp1.tile([1, GN], BF16, tag="nhg")
                    nc.sync.dma_start(out=nhg, in_=nothas[g:g + 1, :])
                    rz = grp1.tile([128, 2, GN], BF16, tag="rz")
                    ng_t = grp1.tile([128, GN], BF16, tag="ng")
                    gh2s = grp1.tile([128, GN], BF16, tag="gh2s")
                    for h in range(2):
                        hs = bass.ds(h * 448, 448)
                        gi0 = psG.tile([128, 448], FP32, tag="gi0", name=f"gi0_{g}{h}")
                        gi1 = psG.tile([128, 448], FP32, tag="gi1", name=f"gi1_{g}{h}")
                        gi2 = psG.tile([128, 448], FP32, tag="gi2", name=f"gi2_{g}{h}")
                        gis = [gi0, gi1, gi2]
                        for m in range(3):
                            nc.tensor.matmul(gis[m], wih[:, 0, m * 128:(m + 1) * 128],
                                             mg[:, hs], start=True, stop=False)
                            for k in range(1, 4):
                                nc.tensor.matmul(
                                    gis[m], wih[:, k, m * 128:(m + 1) * 128],
                                    agg[:, k - 1, hs], start=False, stop=False)
                            if m < 2:
                                nc.tensor.matmul(gis[m], whh[:, m * 128:(m + 1) * 128],
                                                 mg[:, hs], start=False,
                                                 stop=(m == 0))
                        # z-gate +30*(1-has) (keeps memory where no events)
                        nc.tensor.matmul(gi1, thirty, nhg[:, hs],
                                         start=False, stop=True)
                        nc.scalar.activation(rz[:, 0, hs], gi0, AF.Sigmoid,
                                             bias=bs[:, 0:1])
                        nc.scalar.activation(rz[:, 1, hs], gi1, AF.Sigmoid,
                                             bias=bs[:, 1:2])
                        gh2 = psG.tile([128, 448], FP32, tag="gi0",
                                       name=f"gh2_{g}{h}")
                        nc.tensor.matmul(gh2, whh[:, 256:384], mg[:, hs],
                                         start=True, stop=True)
                        nc.scalar.activation(gh2s[:, hs], gh2, AF.Identity,
                                             bias=bh2[:, 0:1])
                        nc.vector.tensor_mul(gh2s[:, hs], rz[:, 0, hs],
                                             gh2s[:, hs])
                        nc.tensor.matmul(gi2, identb, gh2s[:, hs],
                                         start=False, stop=True)
                        nc.scalar.activation(ng_t[:, hs], gi2, AF.Tanh,
                                             bias=bi2[:, 0:1])
                    # newmem = n + z*(mem - n)
                    d_t = grp1.tile([128, GN], BF16, tag="d_t")
                    nc.vector.tensor_sub(d_t, mg, ng_t)
                    nc.vector.tensor_mul(d_t, rz[:, 1, :], d_t)
                    nmg = grpc.tile([128, GN], BF16, tag="nmg")
                    nc.vector.tensor_add(nmg, ng_t, d_t)
                    nc.sync.dma_start_transpose(
                        nm_node[:, g * GW:(g + 1) * GW, :], nmg)
                    if debug == 2:
                        nmgf = grpc.tile([128, GN], FP32, tag="nmgf")
                        nc.vector.tensor_copy(nmgf, nmg)
                        nc.sync.dma_start(out=dbg['newmemT'][:, nsl], in_=nmgf)
                    # feat = newmem + node_features (in-place over nfg)
                    nfg = grp.tile([128, GN], BF16, tag="nfg")
                    nc.sync.dma_start(out=nfg, in_=nf_bf[:, nsl])
                    ftg = nfg
                    nc.vector.tensor_add(ftg, nmg, nfg)
                    # --- projection + row norms ---
                    pfc = grp1.tile([128, GN], FP32, tag="pfc")
                    sqb = grp1.tile([128, GN], BF16, tag="sqb")
                    for h in range(2):
                        hs = bass.ds(h * 448, 448)
                        ppf = psG.tile([128, 448], FP32, tag="gi0", name=f"ppf{g}{h}")
                        nc.tensor.matmul(ppf, pw, ftg[:, hs], start=True, stop=True)
                        nc.scalar.activation(pfc[:, hs], ppf, AF.Identity,
                                             bias=pbt[:, 0:1])
                    nc.vector.tensor_mul(sqb, pfc, pfc)
                    rnb = grp1.tile([1, GN], BF16, tag="rnb")
                    for h in range(2):
                        hs = bass.ds(h * 448, 448)
                        pss = psS.tile([1, 448], FP32, tag="srow", name=f"pss{g}{h}")
                        nc.tensor.matmul(pss, ones_col, sqb[:, hs],
                                         start=True, stop=True)
                        rnf = grp1.tile([1, 448], FP32, tag="rnf")
                        nc.scalar.activation(rnf, pss, AF.Sqrt)
                        nc.vector.tensor_scalar_add(rnf, rnf, 1e-8)
                        rnr = grp1.tile([1, 448], FP32, tag="rnr")
                        nc.vector.reciprocal(rnr, rnf)
                        nc.vector.tensor_copy(rnb[:, hs], rnr)
                    nc.sync.dma_start(out=rnorm_dram[0, nsl], in_=rnb)
                    rep = grp1.tile([128, GN], BF16, tag="rep")
                    nc.sync.dma_start(out=rep, in_=_bcast_row(rnorm_dram, GN, off=n0))
                    pfng = grpc.tile([128, GN], BF16, tag="pfn")
                    nc.vector.tensor_mul(pfng, pfc, rep)
                    # --- similarity (448-col PSUM chunks) ---
                    for m in range(2):
                        for h in range(2):
                            hl = bass.ds(h * 448, 448)
                            hs = bass.ds(n0 + h * 448, 448)
                            psm = psG.tile([128, 448], FP32, tag="gi1",
                                           name=f"psm{g}{m}{h}")
                            nc.tensor.matmul(psm, cennT[:, m * 128:(m + 1) * 128],
                                             pfng[:, hl], start=True, stop=True)
                            if m == 0:
                                nc.vector.tensor_copy(simT[:, m, hs], psm)
                            else:
                                nc.scalar.activation(simT[:, m, hs], psm,
                                                     AF.Identity)
                    nc.vector.tensor_reduce(rm_part[:, :, g], simT[:, :, nsl],
                                            axis=AX.X, op=ALU.max)
            # group pools freed
            if debug:
                nc.sync.dma_start(out=dbg['simT'][:, :, :], in_=simT)

            # row max over group partials + AllReduce(max)
            rm4 = scr.tile([128, 4], FP32, tag="rm4")
            nc.vector.tensor_reduce(rm4[:, 0:2], rm_part, axis=AX.X, op=ALU.max)
            nc.vector.tensor_copy(rm4[:, 2:4], rm4[:, 0:2])
            rmg = scr.tile([128, 4], FP32, tag="rmg")
            allreduce(ALU.max, rm_l, rm_a, rm4, rmg)

        # (sim_node/nm_node filled per group above)

        with tc.tile_pool(name="slv", bufs=1) as slv, \
                tc.tile_pool(name="psC", bufs=1, space="PSUM") as psC:
            sim_node = slv.tile([128, L // 128, 256], BF16)
            for m in range(2):
                nc.sync.dma_start_transpose(
                    sim_node[:, :, m * 128:(m + 1) * 128], simT[:, m, :])
            # ===== nc secant (tau over C=256 per node) =====
            # g-eval: relu(x - t) == max(x, t) - t; accum_out reduces with
            # op1 (add) -> per-partition sum.
            junk_n = slv.tile([128, C], BF16)
            junk_n2 = slv.tile([128, C], BF16)
            zer_n = slv.tile([128, C], BF16)
            nc.vector.memset(zer_n, 0.0)
            NW = L // 128
            tau = slv.tile([128, NW], FP32)
            tau_p = slv.tile([128, NW], FP32)
            g_c = slv.tile([128, NW], FP32)
            g_p = slv.tile([128, NW], FP32)

            def nc_eval(tau_tile, g_tile):
                for ch in range(NW):
                    jt = junk_n if ch % 2 == 0 else junk_n2
                    nc.vector.scalar_tensor_tensor(
                        jt, sim_node[:, ch, :], tau_tile[:, ch:ch + 1], zer_n,
                        op0=ALU.subtract, op1=ALU.max,
                        accum_out=g_tile[:, ch:ch + 1])

            nc.vector.memset(tau_p, -2.0)
            nc_eval(tau_p, g_p)
            st1 = wk.tile([128, NW], FP32, tag="st1")
            nc.vector.tensor_scalar(st1, g_p, -1.0, 1.0 / 256.0,
                                    op0=ALU.add, op1=ALU.mult)
            nc.vector.tensor_add(tau, tau_p, st1)

            def secant_update(tt, tp, gg, gp, wtag, shape):
                num = wk.tile(shape, FP32, tag=wtag + "n")
                nc.vector.tensor_sub(num, tt, tp)
                gm1 = wk.tile(shape, FP32, tag=wtag + "g")
                nc.vector.tensor_scalar_add(gm1, gg, -1.0)
                nc.vector.tensor_mul(num, num, gm1)
                den = wk.tile(shape, FP32, tag=wtag + "d")
                nc.vector.tensor_sub(den, gp, gg)
                nc.vector.tensor_scalar_max(den, den, 1e-12)
                rden = wk.tile(shape, FP32, tag=wtag + "r")
                nc.vector.reciprocal(rden, den)
                nc.vector.tensor_copy(tp, tt)
                nc.vector.tensor_copy(gp, gg)
                stp = wk.tile(shape, FP32, tag=wtag + "s")
                nc.vector.tensor_mul(stp, num, rden)
                nc.vector.tensor_scalar(stp, stp, 0.0, 1.0,
                                        op0=ALU.max, op1=ALU.min)
                nc.vector.tensor_add(tt, tt, stp)

            def nc_iter(k):
                nc_eval(tau, g_c)
                secant_update(tau, tau_p, g_c, g_p, "ncs", [128, NW])

            # ===== cn bracket solver =====
            pos = slv.tile([128, 2, KX], FP32)
            gv = slv.tile([128, 2, KX], FP32)
            lo = slv.tile([128, 2], FP32)
            hi = slv.tile([128, 2], FP32)
            glo = slv.tile([128, 2], FP32)
            ghi = slv.tile([128, 2], FP32)
            junkL = slv.tile([128, L], BF16)
            junkL2 = junkL
            neg = slv.tile([128, 2, KX], FP32)

            def cn_probes(round_idx, nk=CN_K):
                nc.vector.tensor_scalar_mul(neg, pos, -1.0)
                for m in range(2):
                    for k in range(nk):
                        jt = junkL if k % 2 == 0 else junkL2
                        nc.scalar.activation(
                            jt, simT[:, m, :], AF.Relu,
                            bias=neg[:, m, 1 + k:2 + k],
                            accum_out=gv[:, m, 1 + k:2 + k])

            def cn_select():
                # shift masked (g>=1) positions by +8; the bracket ends are
                # argmax/argmin in shifted space; g values extracted by
                # bitwise-consistent is_equal one-hots (no magnitude tricks).
                msk = wk.tile([128, 2, KX], FP32, tag="msk")
                nc.vector.tensor_scalar(msk, gv, 1.0, None, op0=ALU.is_ge)
                tsel = wk.tile([128, 2, KX], FP32, tag="tsel")
                nc.vector.scalar_tensor_tensor(tsel, msk, 8.0, pos,
                                               op0=ALU.mult, op1=ALU.add)
                lo8 = wk.tile([128, 2], FP32, tag="lo8")
                nc.vector.tensor_reduce(lo8, tsel, axis=AX.X, op=ALU.max)
                hi8 = wk.tile([128, 2], FP32, tag="hi8")
                nc.vector.tensor_reduce(hi8, tsel, axis=AX.X, op=ALU.min)
                ohl = wk.tile([128, 2, KX], FP32, tag="ohl")
                sel = wk.tile([128, 2, KX], FP32, tag="sel")
                for m in range(2):
                    nc.vector.tensor_scalar(ohl[:, m, :], tsel[:, m, :],
                                            lo8[:, m:m + 1], None,
                                            op0=ALU.is_equal)
                nc.vector.tensor_mul(sel, gv, ohl)
                nc.vector.tensor_reduce(glo, sel, axis=AX.X, op=ALU.max)
                for m in range(2):
                    nc.vector.tensor_scalar(ohl[:, m, :], tsel[:, m, :],
                                            hi8[:, m:m + 1], None,
                                            op0=ALU.is_equal)
                nc.vector.tensor_mul(sel, gv, ohl)
                nc.vector.tensor_reduce(ghi, sel, axis=AX.X, op=ALU.max)
                nc.vector.tensor_scalar_add(lo, lo8, -8.0)
                nc.vector.tensor_copy(hi, hi8)

            # round 1 positions from global row max
            for k in range(CN_K):
                nc.vector.tensor_scalar_add(pos[:, 0, 1 + k:2 + k],
                                            rmg[:, 0:1], -CN_FR1[k])
                nc.vector.tensor_scalar_add(pos[:, 1, 1 + k:2 + k],
                                            rmg[:, 1:2], -CN_FR1[k])
            for m in range(2):
                nc.vector.tensor_scalar_add(pos[:, m, 0:1],
                                            rmg[:, m:m + 1], -1.0)
                nc.vector.tensor_copy(pos[:, m, KX - 1:KX], rmg[:, m:m + 1])
            nc.vector.memset(gv[:, :, 0:1], 2.0)
            nc.vector.memset(gv[:, :, KX - 1:KX], 0.0)

            cn_probes(0)
            nc_iter(0)
            nc_iter(1)
            gvg = wk.tile([128, 2, CN_K], FP32, tag="gvg")
            allreduce(ALU.add, gp_l[0], gp_a[0], gv[:, :, 1:KX - 1], gvg)
            nc.vector.tensor_copy(gv[:, :, 1:KX - 1], gvg)
            cn_select()
            # round 2: 5 uniform probes; entries 6,7 duplicate hi (g=0,
            # is_equal ties in the select resolve via max)
            K2 = 5
            w2 = wk.tile([128, 2], FP32, tag="w2")
            nc.vector.tensor_sub(w2, hi, lo)
            for k in range(K2):
                nc.vector.scalar_tensor_tensor(
                    pos[:, :, 1 + k:2 + k], w2, (k + 1.0) / (K2 + 1), lo,
                    op0=ALU.mult, op1=ALU.add)
            for k in range(K2, CN_K):
                nc.vector.tensor_copy(pos[:, :, 1 + k:2 + k], hi)
            nc.vector.memset(gv[:, :, 1 + K2:KX - 1], 0.0)
            nc.vector.tensor_copy(pos[:, :, 0:1], lo)
            nc.vector.tensor_copy(pos[:, :, KX - 1:KX], hi)
            nc.vector.tensor_copy(gv[:, :, 0:1], glo)
            nc.vector.tensor_copy(gv[:, :, KX - 1:KX], ghi)

            cn_probes(1, nk=K2)
            nc_iter(2)
            nc_iter(3)
            allreduce(ALU.add, gp_l[1], gp_a[1], gv[:, :, 1:KX - 1], gvg)
            nc.vector.tensor_copy(gv[:, :, 1:KX - 1], gvg)
            cn_select()
            nc_iter(4)
            nc_iter(5)
            # secant interpolation: ctau = lo + clip((glo-1)/(glo-ghi)) * (hi-lo)
            ctau = slv.tile([128, 2], FP32)
            num2 = wk.tile([128, 2], FP32, tag="num2")
            nc.vector.tensor_scalar_add(num2, glo, -1.0)
            den2 = wk.tile([128, 2], FP32, tag="den2")
            nc.vector.tensor_sub(den2, glo, ghi)
            nc.vector.tensor_scalar_max(den2, den2, 1e-9)
            rd2 = wk.tile([128, 2], FP32, tag="rd2")
            nc.vector.reciprocal(rd2, den2)
            frac = wk.tile([128, 2], FP32, tag="frac")
            nc.vector.tensor_mul(frac, num2, rd2)
            nc.vector.tensor_scalar(frac, frac, 0.0, 1.0, op0=ALU.max, op1=ALU.min)
            nc.vector.tensor_sub(w2, hi, lo)
            nc.vector.tensor_mul(frac, frac, w2)
            nc.vector.tensor_add(ctau, lo, frac)
            if debug:
                nc.sync.dma_start(out=dbg['taucn'][:, :], in_=ctau)
                nc.sync.dma_start(out=dbg['taunc'][:, :], in_=tau)

            # taunc -> DRAM row for the phase-8 broadcast
            tau_b = wk.tile([128, NW], BF16, tag="tau_b")
            nc.vector.tensor_copy(tau_b, tau)
            nc.sync.dma_start(
                out=taunc_dram.ap().rearrange("w p -> p w"), in_=tau_b)

            # ===== c_memory: relu in simT layout, xbar transpose, matmul =====
            ps_cms = [psC.tile([128, 128], FP32, tag=f"cm{m}", name=f"pscm{m}")
                      for m in range(2)]
            for m in range(2):
                jt = junkL if m == 0 else junkL2
                nc.vector.tensor_scalar(
                    jt, simT[:, m, :], ctau[:, m:m + 1], 0.0,
                    op0=ALU.subtract, op1=ALU.max)
                nc.sync.dma_start_transpose(
                    sim_node[:, :, m * 128:(m + 1) * 128], jt)
            for ch in range(NW):
                for m in range(2):
                    nc.tensor.matmul(
                        ps_cms[m], sim_node[:, ch, m * 128:(m + 1) * 128],
                        nm_node[:, ch, :], start=(ch == 0), stop=(ch == NW - 1))
            cmf = wk.tile([128, 2, 128], FP32, tag="cmf")
            for m in range(2):
                nc.vector.tensor_copy(cmf[:, m, :], ps_cms[m])
            cmgf = wk.tile([128, 2, 128], FP32, tag="cmgf")
            allreduce(ALU.add, cm_local, cm_all, cmf, cmgf,
                      in_ap=cm_local.ap().rearrange("(m p) d -> p m d", p=128),
                      out_ap=cm_all.ap().rearrange("(m p) d -> p m d", p=128))
            cmg = scr.tile([128, 2, 128], BF16, tag="cmg")
            nc.vector.tensor_copy(cmg, cmgf)
            if debug:
                nc.sync.dma_start(
                    out=dbg['cmem'].ap().rearrange("(m p) d -> p m d", p=128),
                    in_=cmgf)
        # sim_node freed after c_memory (slv pool closed; nodep closes below)

        # ===== embedding =====
        with tc.tile_pool(name="embp", bufs=2) as embp, \
                tc.tile_pool(name="psZ", bufs=2, space="PSUM") as psZ:
            NW = L // 128
            tnc = const.tile([128, L], BF16)
            nc.sync.dma_start(out=tnc, in_=_bcast_row(taunc_dram, L))
            wb = 0
            while wb < NW:
                nwin = min(4, NW - wb)
                bsl = bass.ds(wb * 128, nwin * 128)
                ncm = embp.tile([128, 2, 512], BF16, tag="ncm")
                for m in range(2):
                    nc.vector.tensor_sub(ncm[:, m, 0:nwin * 128],
                                         simT[:, m, bsl], tnc[:, bsl])
                nc.vector.tensor_scalar_max(ncm[:, :, 0:nwin * 128],
                                            ncm[:, :, 0:nwin * 128], 0.0)
                ps_z = psZ.tile([128, 4, 128], FP32, tag="z")
                for k in range(nwin):
                    w = wb + k
                    for m in range(2):
                        nc.tensor.matmul(
                            ps_z[:, k, :], ncm[:, m, k * 128:(k + 1) * 128],
                            cmg[:, m, :], start=(m == 0), stop=False)
                    nc.tensor.matmul(ps_z[:, k, :], identb, nm_node[:, w, :],
                                     start=False, stop=True)
                emb_c = embp.tile([128, 4, 128], FP32, tag="emb_c")
                nc.vector.tensor_copy(emb_c[:, 0:nwin, :], ps_z[:, 0:nwin, :])
                nc.sync.dma_start(
                    out=emb_out[wb * 128:(wb + nwin) * 128, :].rearrange(
                        "(k p) d -> p k d", p=128),
                    in_=emb_c[:, 0:nwin, :])
                wb += nwin

    split_waits(nc)
    return nc


# ----------------------------------------------------------------------------
# host side
# ----------------------------------------------------------------------------

_CACHE = {}


def _route(L, src, dst, t):
    idx = np.concatenate([src, dst]).astype(np.int64)
    other = np.concatenate([dst, src]).astype(np.int64)
    tt = np.concatenate([t, t])
    eidx = np.concatenate([np.arange(len(src)), np.arange(len(src))])
    NW = L // 128
    order = np.argsort(idx, kind='stable')
    idx_s, other_s, tt_s, eidx_s = idx[order], other[order], tt[order], eidx[order]
    owner = idx_s // L
    cores = []
    for c in range(NCORES):
        msk = owner == c
        li = idx_s[msk] - c * L
        win = li // 128
        col = li % 128
        wcount = np.bincount(win, minlength=NW)
        assert wcount.max() <= 256, f"window overflow: {wcount.max()}"
        woff = np.zeros(NW + 1, np.int64)
        woff[1:] = np.cumsum(wcount)
        within = np.arange(len(li)) - woff[win]
        slot = win * 256 + within
        cores.append(dict(slot=slot, col=col, li=li, other=other_s[msk],
                          tt=tt_s[msk], eidx=eidx_s[msk]))
    return cores


def kernel(**inputs):
    node_memory = np.asarray(inputs['node_memory'])
    last_update = np.asarray(inputs['last_update'])
    node_features = np.asarray(inputs['node_features'])
    event_feat = np.asarray(inputs['event_feat'])
    t = np.asarray(inputs['t'])
    src = np.asarray(inputs['src']).astype(np.int64)
    dst = np.asarray(inputs['dst']).astype(np.int64)
    time_w = np.asarray(inputs['time_w'])
    time_b = np.asarray(inputs['time_b'])
    W_ih = np.asarray(inputs['W_ih'])
    b_ih = np.asarray(inputs['b_ih'])
    W_hh = np.asarray(inputs['W_hh'])
    b_hh = np.asarray(inputs['b_hh'])
    proj_W = np.asarray(inputs['proj_W'])
    proj_b = np.asarray(inputs['proj_b'])
    centroids = np.asarray(inputs['centroids'])

    Nn = node_memory.shape[0]
    gran = 128 * GW * NCORES          # L must be a multiple of 128*GW
    NP = -(-Nn // gran) * gran
    L = NP // NCORES
    SLOTS = 2 * L
    NT = SLOTS // 128
    NG = L // GN

    nmp = np.zeros((NP, D), np.float32); nmp[:Nn] = node_memory
    nfp = np.zeros((NP, D), np.float32); nfp[:Nn] = node_features
    lup = np.zeros(NP, np.float32); lup[:Nn] = last_update

    idx_full = np.concatenate([src, dst])
    cnt_full = np.bincount(idx_full, minlength=NP).astype(np.float32)
    icnt_full = 1.0 / np.maximum(cnt_full, 1.0)
    nothas_full = (cnt_full == 0).astype(np.float32)

    cores = _route(L, src, dst, t)
    bsum_h = f32c(np.stack([(b_ih + b_hh)[0:128], (b_ih + b_hh)[128:256]], 1))
    wih_h = bfc(W_ih.T.reshape(4, 128, 384).transpose(1, 0, 2))

    in_maps = []
    for c in range(NCORES):
        r = cores[c]
        sl = r['slot']
        p_i = sl % 128
        t_i = sl // 128
        ev_x = np.zeros((128, NT, 256), ml_dtypes.bfloat16)
        ev_x[p_i, t_i, 0:128] = nmp[r['other']].astype(ml_dtypes.bfloat16)
        ev_x[p_i, t_i, 128:256] = event_feat[r['eidx']].astype(ml_dtypes.bfloat16)
        ev_meta = np.zeros((128, 2, NT), np.float32)
        ev_meta[:, 0, :] = -1.0
        ev_meta[p_i, 0, t_i] = r['col'].astype(np.float32)
        ev_meta[p_i, 1, t_i] = icnt_full[r['li'] + c * L]
        dt_flat = np.zeros(SLOTS, np.float32)
        dt_flat[sl] = r['tt'] - lup[r['li'] + c * L]
        nsl = slice(c * L, (c + 1) * L)
        in_maps.append({
            'ev_x': ev_x,
            'ev_meta': ev_meta,
            'dt_row': f32c(dt_flat.reshape(NG, GS)),
            'mem_bf': bfc(nmp[nsl].T),
            'nf_bf': bfc(nfp[nsl].T),
            'nothas': bfc(nothas_full[nsl].reshape(NG, GN)),
            'W_ihT': wih_h,
            'W_hhT': bfc(W_hh.T),
            'bsum': bsum_h,
            'b_hh2': f32c(b_hh[256:384].reshape(128, 1)),
            'b_ih2': f32c(b_ih[256:384].reshape(128, 1)),
            'pWt': bfc(proj_W),
            'pb': f32c(proj_b.reshape(128, 1)),
            'cenT': f32c(centroids.T),
            'w_row': f32c(time_w.reshape(1, 128)),
            'bpi_col': f32c((time_b + HALF_PI).reshape(128, 1)),
            'iota_t': bfc(np.tile(np.arange(128, dtype=np.float32)[None, :],
                                  (128, 1))),
        })

    debug = int(os.environ.get("KERNEL_DEBUG", "0"))
    key = (L, debug)
    if key not in _CACHE:
        _CACHE[key] = build_program(L, debug=debug)
    nc = _CACHE[key]
    res = run_bass_kernel_spmd(nc, in_maps, list(range(NCORES)))
    emb = np.concatenate([res.results[c]['emb'] for c in range(NCORES)], 0)
    kernel._last_exec_ns = getattr(res, 'exec_time_ns', None)
    if debug:
        kernel._last_results = res.results
    return emb[:Nn].astype(np.float32)


# revision 17
# speedup vs baseline: 1.1009x; 1.0001x over previous
"""TGN-style GNN message passing + community detection on 8 TRN2 NeuronCores.

Node-sharded SPMD, v2 (engine-balanced rewrite):
- Fused per-group pipeline (events -> agg -> GRU -> feat -> proj -> sim)
  with SBUF-resident intermediates; event tensors host-packed so every
  load is one large contiguous-per-partition DMA.
- Time encoding via a 1-partition PE outer product + Activation Sin with
  per-partition bias, landing in [feat, slot] layout, then xbar DMA
  transpose into the event matrix (no DVE work).
- has-mask folded into the GRU z-gate via a +30*(1-has) rank-1 matmul
  (sigmoid saturates to 1 -> memory passthrough), removing all blend ops.
- cn-sparsemax tau via 2 rounds of multi-probe bracketing (7 nonuniform +
  7 uniform probes, fused sub+relu+sum DVE ops in 4x bf16 mode) + secant
  interpolation: 3 AllReduces instead of 13.
- nc-sparsemax tau via secant from tau0=-2 (8 iterations, per-window
  fused DVE ops); overlapped with the cn AllReduce latency.
- c_memory via relu applied in simT layout then xbar-transposed;
  AllReduce #4. Total 4 collectives.
"""

import os
from contextlib import ExitStack

import numpy as np
import ml_dtypes

import concourse.bass as bass
import concourse.mybir as mybir
import concourse.tile as tile
from concourse.bass_utils import run_bass_kernel_spmd
from concourse.masks import make_identity

FP32 = mybir.dt.float32
BF16 = mybir.dt.bfloat16
AF = mybir.ActivationFunctionType
ALU = mybir.AluOpType
AX = mybir.AxisListType

NCORES = 8
D = 128
C = 256
HALF_PI = float(np.pi / 2)
TWO_PI = float(2 * np.pi)
MAGIC = 12582912.0
GW = 7                      # windows per group
GN = GW * 128               # nodes per group (896)
GS = 2 * GN                 # event slots per group (1792)
GT = GS // 128              # slot-tiles per group (14)

# cn bracket probe offsets below rowmax (round 1, ascending positions)
CN_FR1 = [0.25, 0.17, 0.12, 0.08, 0.05, 0.02]
CN_K = len(CN_FR1)          # 7 probes per round
KX = CN_K + 2               # probe array incl. bracket ends
CN_F2 = [(k + 1) / (CN_K + 1) for k in range(CN_K)]
BIGV = 1.0e4
NIT_NC = 6

bfc = lambda x: np.ascontiguousarray(np.asarray(x).astype(ml_dtypes.bfloat16))
f32c = lambda x: np.ascontiguousarray(np.asarray(x).astype(np.float32))


def _bcast_row(dram_tensor, ncols, nparts=128, off=0):
    row = dram_tensor.ap()
    return bass.AP(tensor=row.tensor, offset=row.offset + off,
                   ap=[[0, nparts], [1, ncols]])


def split_waits(nc, sp_limit=1, default_limit=1):
    """This env's walrus rejects >1 sync-wait on SP CTRL instructions:
    move extra waits onto preceding NOPs."""
    limits = {mybir.EngineType.SP: sp_limit}
    for fn in nc.m.functions:
        for bb in fn.blocks:
            out = []
            for ins in bb.instructions:
                si = ins.sync_info
                w = list(si.on_wait) if (si is not None and si.on_wait) else []
                lim = limits.get(ins.engine, default_limit)
                if len(w) > lim:
                    extra, keep = w[:-lim], w[-lim:]
                    for j in range(0, len(extra), lim):
                        out.append(mybir.InstNoOp(
                            name=f"{ins.name}-ws{j}",
                            engine=ins.engine,
                            sync_info=mybir.SyncInfo(
                                on_wait=list(extra[j:j + lim]), on_update=[]),
                        ))
                    ins.sync_info = mybir.SyncInfo(
                        on_wait=list(keep),
                        on_update=list(si.on_update) if si.on_update else [])
                out.append(ins)
            bb.instructions = out
    return nc


def build_program(L, debug=False):
    NG = L // GN            # groups (14 for L=12544)
    NW = L // 128           # windows (98)
    NT = 2 * L // 128       # slot-tiles total (196)

    nc = bass.Bass(num_devices=NCORES)

    # ---- inputs ----
    ev_x = nc.dram_tensor("ev_x", [128, NT, 256], BF16, kind="ExternalInput")
    ev_meta = nc.dram_tensor("ev_meta", [128, 2, NT], FP32, kind="ExternalInput")
    dt_row = nc.dram_tensor("dt_row", [NG, GS], FP32, kind="ExternalInput")
    mem_bf = nc.dram_tensor("mem_bf", [128, L], BF16, kind="ExternalInput")
    nf_bf = nc.dram_tensor("nf_bf", [128, L], BF16, kind="ExternalInput")
    nothas = nc.dram_tensor("nothas", [NG, GN], BF16, kind="ExternalInput")
    W_ihT = nc.dram_tensor("W_ihT", [128, 4, 384], BF16, kind="ExternalInput")
    W_hhT = nc.dram_tensor("W_hhT", [128, 384], BF16, kind="ExternalInput")
    bsum = nc.dram_tensor("bsum", [128, 2], FP32, kind="ExternalInput")
    b_hh2 = nc.dram_tensor("b_hh2", [128, 1], FP32, kind="ExternalInput")
    b_ih2 = nc.dram_tensor("b_ih2", [128, 1], FP32, kind="ExternalInput")
    pWt = nc.dram_tensor("pWt", [128, 128], BF16, kind="ExternalInput")
    pb = nc.dram_tensor("pb", [128, 1], FP32, kind="ExternalInput")
    cenT = nc.dram_tensor("cenT", [128, C], FP32, kind="ExternalInput")
    w_row = nc.dram_tensor("w_row", [1, 128], FP32, kind="ExternalInput")
    bpi_col = nc.dram_tensor("bpi_col", [128, 1], FP32, kind="ExternalInput")
    iota_t = nc.dram_tensor("iota_t", [128, 128], BF16, kind="ExternalInput")

    emb_out = nc.dram_tensor("emb", [L, D], FP32, kind="ExternalOutput")
    dbg = {}
    if debug:
        dbg['newmemT'] = nc.dram_tensor("dbg_newmemT", [128, L], FP32, kind="ExternalOutput")
        dbg['aggT'] = nc.dram_tensor("dbg_aggT", [128, 3, L], FP32, kind="ExternalOutput")
        dbg['xg'] = nc.dram_tensor("dbg_xg", [128, 2 * L // 128, 256], BF16, kind="ExternalOutput")
        dbg['tencT'] = nc.dram_tensor("dbg_tencT", [128, 2 * L], FP32, kind="ExternalOutput")
        dbg['simT'] = nc.dram_tensor("dbg_simT", [128, 2, L], BF16, kind="ExternalOutput")
        dbg['taunc'] = nc.dram_tensor("dbg_taunc", [128, NW], FP32, kind="ExternalOutput")
        dbg['taucn'] = nc.dram_tensor("dbg_taucn", [128, 2], FP32, kind="ExternalOutput")
        dbg['cmem'] = nc.dram_tensor("dbg_cmem", [C, D], FP32, kind="ExternalOutput")

    # ---- staging DRAM ----
    crec_dram = nc.dram_tensor("crec_dram", [1, C], BF16)
    rnorm_dram = nc.dram_tensor("rnorm_dram", [1, L], BF16)
    taunc_dram = nc.dram_tensor("taunc_dram", [NW, 128], BF16)
    taucn_dram = nc.dram_tensor("taucn_dram", [2, 128], BF16)
    rm_l = nc.dram_tensor("rm_l", [128, 4], FP32)
    rm_a = nc.dram_tensor("rm_a", [128, 4], FP32, addr_space="Shared")
    gp_l = [nc.dram_tensor(f"gp_l{r}", [128, 2 * CN_K], FP32) for r in range(2)]
    gp_a = [nc.dram_tensor(f"gp_a{r}", [128, 2 * CN_K], FP32, addr_space="Shared")
            for r in range(2)]
    cm_local = nc.dram_tensor("cm_local", [C, D], FP32)
    cm_all = nc.dram_tensor("cm_all", [C, D], FP32, addr_space="Shared")
    RG = [list(range(NCORES))]

    cc_sem = nc.alloc_semaphore("cc_done")
    ccv = [0]

    def allreduce(alu_op, local_dram, shared_dram, sb_in, sb_out,
                  in_ap=None, out_ap=None):
        """Stage sb_in -> local_dram, AllReduce -> shared_dram, load sb_out."""
        with tc.tile_critical():
            nc.gpsimd.dma_start(
                out=local_dram.ap() if in_ap is None else in_ap,
                in_=sb_in).then_inc(cc_sem, 16)
            ccv[0] += 16
            nc.gpsimd.wait_ge(cc_sem, ccv[0])
            nc.gpsimd.collective_compute(
                "AllReduce", alu_op, replica_groups=RG,
                ins=[local_dram.ap().opt()],
                outs=[shared_dram.ap().opt()]).then_inc(cc_sem)
            ccv[0] += 1
            nc.gpsimd.wait_ge(cc_sem, ccv[0])
            nc.gpsimd.dma_start(
                out=sb_out,
                in_=shared_dram.ap() if out_ap is None else out_ap
            ).then_inc(cc_sem, 16)
            ccv[0] += 16
            nc.gpsimd.wait_ge(cc_sem, ccv[0])

    ctx = ExitStack()
    with tile.TileContext(nc) as tc, ctx:
        const = ctx.enter_context(tc.tile_pool(name="const", bufs=1))
        late = ctx.enter_context(tc.tile_pool(name="late", bufs=1))
        wk = ctx.enter_context(tc.tile_pool(name="wk", bufs=2))
        scr = ctx.enter_context(tc.tile_pool(name="scr", bufs=1))
        psS = ctx.enter_context(tc.tile_pool(name="psS", bufs=1, space="PSUM"))

        # ----- constants -----
        identb = const.tile([128, 128], BF16)
        make_identity(nc, identb)
        iota = const.tile([128, 128], BF16)
        nc.sync.dma_start(out=iota, in_=iota_t[:, :])
        wih = const.tile([128, 4, 384], BF16)
        nc.sync.dma_start(out=wih, in_=W_ihT[:, :, :])
        whh = const.tile([128, 384], BF16)
        nc.sync.dma_start(out=whh, in_=W_hhT[:, :])
        bs = const.tile([128, 2], FP32)
        nc.sync.dma_start(out=bs, in_=bsum[:, :])
        bh2 = const.tile([128, 1], FP32)
        nc.sync.dma_start(out=bh2, in_=b_hh2[:, :])
        bi2 = const.tile([128, 1], FP32)
        nc.sync.dma_start(out=bi2, in_=b_ih2[:, :])
        pw = const.tile([128, 128], BF16)
        nc.sync.dma_start(out=pw, in_=pWt[:, :])
        pbt = const.tile([128, 1], FP32)
        nc.sync.dma_start(out=pbt, in_=pb[:, :])
        wrow = const.tile([1, 128], FP32)
        nc.sync.dma_start(out=wrow, in_=w_row[:, :])
        bpi = const.tile([128, 1], FP32)
        nc.sync.dma_start(out=bpi, in_=bpi_col[:, :])
        meta = const.tile([128, 2, NT], FP32)
        nc.sync.dma_start(out=meta, in_=ev_meta[:, :, :])
        ones_col = const.tile([128, 1], BF16)
        nc.vector.memset(ones_col, 1.0)
        thirty = const.tile([1, 128], BF16)
        nc.vector.memset(thirty, 30.0)
        eps1 = const.tile([1, 1], FP32)
        nc.vector.memset(eps1, 1e-12)

        # centroid norms (device, overlaps with first group loads)
        cen = const.tile([128, C], FP32)
        nc.sync.dma_start(out=cen, in_=cenT[:, :])
        censq = scr.tile([128, C], BF16, tag="censq")
        nc.vector.tensor_mul(censq, cen, cen)
        ps_c = psS.tile([1, 448], FP32, tag="srow")
        nc.tensor.matmul(ps_c[:, 0:C], ones_col, censq, start=True, stop=True)
        cnorm = scr.tile([1, C], FP32, tag="cnorm")
        nc.scalar.activation(cnorm, ps_c[:, 0:C], AF.Sqrt)
        nc.vector.tensor_scalar_add(cnorm, cnorm, 1e-8)
        crecf = scr.tile([1, C], FP32, tag="crecf")
        nc.vector.reciprocal(crecf, cnorm)
        crec = scr.tile([1, C], BF16, tag="crec")
        nc.vector.tensor_copy(crec, crecf)
        nc.sync.dma_start(out=crec_dram[:, :], in_=crec)
        crec_rep = const.tile([128, C], BF16)
        nc.sync.dma_start(out=crec_rep, in_=_bcast_row(crec_dram, C))
        cennT = const.tile([128, C], BF16)
        nc.vector.tensor_mul(cennT, cen, crec_rep)

        # ----- long-lived tensors -----
        simT = late.tile([128, 2, L], BF16)
        rm_part = late.tile([128, 2, NG], FP32)
        nodep = ctx.enter_context(tc.tile_pool(name="nodep", bufs=1))
        nm_node = nodep.tile([128, L // 128, 128], BF16)

        if True:

            # ================= fused group loop =================
            with tc.tile_pool(name="grp", bufs=2) as grp, \
                    tc.tile_pool(name="grpc", bufs=2) as grpc, \
                    tc.tile_pool(name="grp1", bufs=1) as grp1, \
                    tc.tile_pool(name="psA", bufs=1, space="PSUM") as psA, \
                    tc.tile_pool(name="psO", bufs=1, space="PSUM") as psO, \
                    tc.tile_pool(name="psG", bufs=1, space="PSUM") as psG:
                for g in range(NG):
                    t0 = g * GT
                    n0 = g * GN
                    nsl = bass.ds(n0, GN)
                    # --- event loads ---
                    xg = grp.tile([128, GT, 256], BF16, tag="xg")
                    nc.sync.dma_start(out=xg, in_=ev_x[:, t0:t0 + GT, :])
                    xgt = grp.tile([128, GT, 128], BF16, tag="xgt")
                    if debug == 2:
                        nc.sync.dma_start(out=dbg['xg'][:, t0:t0 + GT, :], in_=xg)
                    dtr = grpc.tile([1, GS], FP32, tag="dtr")
                    nc.sync.dma_start(out=dtr, in_=dt_row[g:g + 1, :])
                    # --- time encode: outer product + magic-number range
                    # reduce + Sin + xbar transpose ---
                    tencT = grp1.tile([128, GS], BF16, tag="tencT")
                    for q in range(4):
                        c0 = q * 448
                        ang = grp1.tile([128, 448], FP32, tag="ang",
                                        name=f"ang{g}{q}")
                        po = psO.tile([128, 448], FP32, tag="po",
                                      name=f"po{g}_{q}")
                        nc.tensor.matmul(po, wrow, dtr[:, c0:c0 + 448],
                                         start=True, stop=True)
                        nc.scalar.activation(ang, po, AF.Identity,
                                             bias=bpi[:, 0:1])
                        m1 = grp1.tile([128, 448], FP32, tag="m1",
                                       name=f"m1_{g}{q}")
                        nc.vector.tensor_scalar(m1, ang, 1.0 / TWO_PI, MAGIC,
                                                op0=ALU.mult, op1=ALU.add)
                        nc.vector.tensor_scalar_add(m1, m1, -MAGIC)
                        nc.vector.scalar_tensor_tensor(ang, m1, -TWO_PI, ang,
                                                       op0=ALU.mult, op1=ALU.add)
                        nc.scalar.activation(tencT[:, c0:c0 + 448],
                                             ang, AF.Sin)
                    nc.sync.dma_start_transpose(xgt, tencT)
                    # --- aggregation ---
                    agg = grp1.tile([128, 3, GN], BF16, tag="agg")
                    for w in range(GW):
                        pas = [psA.tile([128, 128], FP32, tag=f"agg{fc}",
                                        name=f"pa{g}_{w}_{fc}")
                               for fc in range(3)]
                        oh2 = []
                        for t_ in range(2):
                            ti = w * 2 + t_
                            oh = grpc.tile([128, 128], BF16, tag="oh",
                                           name=f"oh{g}_{w}_{t_}")
                            nc.vector.tensor_scalar(
                                oh, iota, meta[:, 0, t0 + ti:t0 + ti + 1],
                                meta[:, 1, t0 + ti:t0 + ti + 1],
                                op0=ALU.is_equal, op1=ALU.mult)
                            oh2.append(oh)
                        # tenc-independent chains first (overlap tenc tail)
                        for fc in range(2):
                            for t_ in range(2):
                                ti = w * 2 + t_
                                nc.tensor.matmul(
                                    pas[fc], xg[:, ti, fc * 128:(fc + 1) * 128],
                                    oh2[t_], start=(t_ == 0), stop=(t_ == 1))
                        for t_ in range(2):
                            ti = w * 2 + t_
                            nc.tensor.matmul(pas[2], xgt[:, ti, :], oh2[t_],
                                             start=(t_ == 0), stop=(t_ == 1))
                        for fc in range(3):
                            if (w + fc) % 2 == 0:
                                nc.vector.tensor_copy(
                                    agg[:, fc, w * 128:(w + 1) * 128], pas[fc])
                            else:
                                nc.scalar.activation(
                                    agg[:, fc, w * 128:(w + 1) * 128], pas[fc],
                                    AF.Identity)
                    if debug == 2:
                        aggf = grpc.tile([128, 3, GN], FP32, tag="aggf")
                        nc.vector.tensor_copy(aggf, agg)
                        nc.sync.dma_start(out=dbg['aggT'][:, :, nsl], in_=aggf)
                        tencf = grpc.tile([128, GS], FP32, tag="tencf")
                        nc.vector.tensor_copy(tencf, tencT)
                        nc.sync.dma_start(
                            out=dbg['tencT'][:, g * GS:(g + 1) * GS], in_=tencf)
                    # --- GRU ---
                    mg = grp.tile([128, GN], BF16, tag="mg")
                    nc.sync.dma_start(out=mg, in_=mem_bf[:, nsl])
                    nhg = grp1.tile([1, GN], BF16, tag="nhg")
                    nc.sync.dma_start(out=nhg, in_=nothas[g:g + 1, :])
                    rz = grp1.tile([128, 2, GN], BF16, tag="rz")
                    ng_t = grp1.tile([128, GN], BF16, tag="ng")
                    gh2s = grp1.tile([128, GN], BF16, tag="gh2s")
                    for h in range(2):
                        hs = bass.ds(h * 448, 448)
                        gi0 = psG.tile([128, 448], FP32, tag="gi0", name=f"gi0_{g}{h}")
                        gi1 = psG.tile([128, 448], FP32, tag="gi1", name=f"gi1_{g}{h}")
                        gi2 = psG.tile([128, 448], FP32, tag="gi2", name=f"gi2_{g}{h}")
                        gis = [gi0, gi1, gi2]
                        for m in range(3):
                            nc.tensor.matmul(gis[m], wih[:, 0, m * 128:(m + 1) * 128],
                                             mg[:, hs], start=True, stop=False)
                            for k in range(1, 4):
                                nc.tensor.matmul(
                                    gis[m], wih[:, k, m * 128:(m + 1) * 128],
                                    agg[:, k - 1, hs], start=False, stop=False)
                            if m < 2:
                                nc.tensor.matmul(gis[m], whh[:, m * 128:(m + 1) * 128],
                                                 mg[:, hs], start=False,
                                                 stop=(m == 0))
                        # z-gate +30*(1-has) (keeps memory where no events)
                        nc.tensor.matmul(gi1, thirty, nhg[:, hs],
                                         start=False, stop=True)
                        nc.scalar.activation(rz[:, 0, hs], gi0, AF.Sigmoid,
                                             bias=bs[:, 0:1])
                        nc.scalar.activation(rz[:, 1, hs], gi1, AF.Sigmoid,
                                             bias=bs[:, 1:2])
                        gh2 = psG.tile([128, 448], FP32, tag="gi0",
                                       name=f"gh2_{g}{h}")
                        nc.tensor.matmul(gh2, whh[:, 256:384], mg[:, hs],
                                         start=True, stop=True)
                        nc.scalar.activation(gh2s[:, hs], gh2, AF.Identity,
                                             bias=bh2[:, 0:1])
                        nc.vector.tensor_mul(gh2s[:, hs], rz[:, 0, hs],
                                             gh2s[:, hs])
                        nc.tensor.matmul(gi2, identb, gh2s[:, hs],
                                         start=False, stop=True)
                        nc.scalar.activation(ng_t[:, hs], gi2, AF.Tanh,
                                             bias=bi2[:, 0:1])
                    # newmem = n + z*(mem - n)
                    d_t = grp1.tile([128, GN], BF16, tag="d_t")
                    nc.vector.tensor_sub(d_t, mg, ng_t)
                    nc.vector.tensor_mul(d_t, rz[:, 1, :], d_t)
                    nmg = grpc.tile([128, GN], BF16, tag="nmg")
                    nc.vector.tensor_add(nmg, ng_t, d_t)
                    nc.sync.dma_start_transpose(
                        nm_node[:, g * GW:(g + 1) * GW, :], nmg)
                    if debug == 2:
                        nmgf = grpc.tile([128, GN], FP32, tag="nmgf")
                        nc.vector.tensor_copy(nmgf, nmg)
                        nc.sync.dma_start(out=dbg['newmemT'][:, nsl], in_=nmgf)
                    # feat = newmem + node_features (in-place over nfg)
                    nfg = grp.tile([128, GN], BF16, tag="nfg")
                    nc.sync.dma_start(out=nfg, in_=nf_bf[:, nsl])
                    ftg = nfg
                    nc.vector.tensor_add(ftg, nmg, nfg)
                    # --- projection + row norms ---
                    pfc = grp1.tile([128, GN], FP32, tag="pfc")
                    sqb = grp1.tile([128, GN], BF16, tag="sqb")
                    for h in range(2):
                        hs = bass.ds(h * 448, 448)
                        ppf = psG.tile([128, 448], FP32, tag="gi0", name=f"ppf{g}{h}")
                        nc.tensor.matmul(ppf, pw, ftg[:, hs], start=True, stop=True)
                        nc.scalar.activation(pfc[:, hs], ppf, AF.Identity,
                                             bias=pbt[:, 0:1])
                    nc.vector.tensor_mul(sqb, pfc, pfc)
                    rnb = grp1.tile([1, GN], BF16, tag="rnb")
                    for h in range(2):
                        hs = bass.ds(h * 448, 448)
                        pss = psS.tile([1, 448], FP32, tag="srow", name=f"pss{g}{h}")
                        nc.tensor.matmul(pss, ones_col, sqb[:, hs],
                                         start=True, stop=True)
                        rnf = grp1.tile([1, 448], FP32, tag="rnf")
                        nc.scalar.activation(rnf, pss, AF.Sqrt)
                        nc.vector.tensor_scalar_add(rnf, rnf, 1e-8)
                        rnr = grp1.tile([1, 448], FP32, tag="rnr")
                        nc.vector.reciprocal(rnr, rnf)
                        nc.vector.tensor_copy(rnb[:, hs], rnr)
                    nc.sync.dma_start(out=rnorm_dram[0, nsl], in_=rnb)
                    rep = grp1.tile([128, GN], BF16, tag="rep")
                    nc.sync.dma_start(out=rep, in_=_bcast_row(rnorm_dram, GN, off=n0))
                    pfng = grpc.tile([128, GN], BF16, tag="pfn")
                    nc.vector.tensor_mul(pfng, pfc, rep)
                    # --- similarity (448-col PSUM chunks) ---
                    for m in range(2):
                        for h in range(2):
                            hl = bass.ds(h * 448, 448)
                            hs = bass.ds(n0 + h * 448, 448)
                            psm = psG.tile([128, 448], FP32, tag="gi1",
                                           name=f"psm{g}{m}{h}")
                            nc.tensor.matmul(psm, cennT[:, m * 128:(m + 1) * 128],
                                             pfng[:, hl], start=True, stop=True)
                            if m == 0:
                                nc.vector.tensor_copy(simT[:, m, hs], psm)
                            else:
                                nc.scalar.activation(simT[:, m, hs], psm,
                                                     AF.Identity)
                    nc.vector.tensor_reduce(rm_part[:, :, g], simT[:, :, nsl],
                                            axis=AX.X, op=ALU.max)
            # group pools freed
            if debug:
                nc.sync.dma_start(out=dbg['simT'][:, :, :], in_=simT)

            # row max over group partials + AllReduce(max)
            rm4 = scr.tile([128, 4], FP32, tag="rm4")
            nc.vector.tensor_reduce(rm4[:, 0:2], rm_part, axis=AX.X, op=ALU.max)
            nc.vector.tensor_copy(rm4[:, 2:4], rm4[:, 0:2])
            rmg = scr.tile([128, 4], FP32, tag="rmg")
            allreduce(ALU.max, rm_l, rm_a, rm4, rmg)

        # (sim_node/nm_node filled per group above)

        with tc.tile_pool(name="slv", bufs=1) as slv, \
                tc.tile_pool(name="psC", bufs=1, space="PSUM") as psC:
            sim_node = slv.tile([128, L // 128, 256], BF16)
            for m in range(2):
                nc.sync.dma_start_transpose(
                    sim_node[:, :, m * 128:(m + 1) * 128], simT[:, m, :])
            # ===== nc secant (tau over C=256 per node) =====
            # g-eval: relu(x - t) == max(x, t) - t; accum_out reduces with
            # op1 (add) -> per-partition sum.
            junk_n = slv.tile([128, C], BF16)
            junk_n2 = slv.tile([128, C], BF16)
            zer_n = slv.tile([128, C], BF16)
            nc.vector.memset(zer_n, 0.0)
            NW = L // 128
            tau = slv.tile([128, NW], FP32)
            tau_p = slv.tile([128, NW], FP32)
            g_c = slv.tile([128, NW], FP32)
            g_p = slv.tile([128, NW], FP32)

            def nc_eval(tau_tile, g_tile):
                for ch in range(NW):
                    jt = junk_n if ch % 2 == 0 else junk_n2
                    nc.vector.scalar_tensor_tensor(
                        jt, sim_node[:, ch, :], tau_tile[:, ch:ch + 1], zer_n,
                        op0=ALU.subtract, op1=ALU.max,
                        accum_out=g_tile[:, ch:ch + 1])

            nc.vector.memset(tau_p, -2.0)
            nc_eval(tau_p, g_p)
            st1 = wk.tile([128, NW], FP32, tag="st1")
            nc.vector.tensor_scalar(st1, g_p, -1.0, 1.0 / 256.0,
                                    op0=ALU.add, op1=ALU.mult)
            nc.vector.tensor_add(tau, tau_p, st1)

            def secant_update(tt, tp, gg, gp, wtag, shape):
                num = wk.tile(shape, FP32, tag=wtag + "n")
                nc.vector.tensor_sub(num, tt, tp)
                gm1 = wk.tile(shape, FP32, tag=wtag + "g")
                nc.vector.tensor_scalar_add(gm1, gg, -1.0)
                nc.vector.tensor_mul(num, num, gm1)
                den = wk.tile(shape, FP32, tag=wtag + "d")
                nc.vector.tensor_sub(den, gp, gg)
                nc.vector.tensor_scalar_max(den, den, 1e-12)
                rden = wk.tile(shape, FP32, tag=wtag + "r")
                nc.vector.reciprocal(rden, den)
                nc.vector.tensor_copy(tp, tt)
                nc.vector.tensor_copy(gp, gg)
                stp = wk.tile(shape, FP32, tag=wtag + "s")
                nc.vector.tensor_mul(stp, num, rden)
                nc.vector.tensor_scalar(stp, stp, 0.0, 1.0,
                                        op0=ALU.max, op1=ALU.min)
                nc.vector.tensor_add(tt, tt, stp)

            def nc_iter(k):
                nc_eval(tau, g_c)
                secant_update(tau, tau_p, g_c, g_p, "ncs", [128, NW])

            # ===== cn bracket solver =====
            pos = slv.tile([128, 2, KX], FP32)
            gv = slv.tile([128, 2, KX], FP32)
            lo = slv.tile([128, 2], FP32)
            hi = slv.tile([128, 2], FP32)
            glo = slv.tile([128, 2], FP32)
            ghi = slv.tile([128, 2], FP32)
            junkL = slv.tile([128, L], BF16)
            junkL2 = junkL
            neg = slv.tile([128, 2, KX], FP32)

            def cn_probes(round_idx, nk=CN_K):
                nc.vector.tensor_scalar_mul(neg, pos, -1.0)
                for m in range(2):
                    for k in range(nk):
                        jt = junkL if k % 2 == 0 else junkL2
                        nc.scalar.activation(
                            jt, simT[:, m, :], AF.Relu,
                            bias=neg[:, m, 1 + k:2 + k],
                            accum_out=gv[:, m, 1 + k:2 + k])

            def cn_select():
                # shift masked (g>=1) positions by +8; the bracket ends are
                # argmax/argmin in shifted space; g values extracted by
                # bitwise-consistent is_equal one-hots (no magnitude tricks).
                msk = wk.tile([128, 2, KX], FP32, tag="msk")
                nc.vector.tensor_scalar(msk, gv, 1.0, None, op0=ALU.is_ge)
                tsel = wk.tile([128, 2, KX], FP32, tag="tsel")
                nc.vector.scalar_tensor_tensor(tsel, msk, 8.0, pos,
                                               op0=ALU.mult, op1=ALU.add)
                lo8 = wk.tile([128, 2], FP32, tag="lo8")
                nc.vector.tensor_reduce(lo8, tsel, axis=AX.X, op=ALU.max)
                hi8 = wk.tile([128, 2], FP32, tag="hi8")
                nc.vector.tensor_reduce(hi8, tsel, axis=AX.X, op=ALU.min)
                ohl = wk.tile([128, 2, KX], FP32, tag="ohl")
                sel = wk.tile([128, 2, KX], FP32, tag="sel")
                for m in range(2):
                    nc.vector.tensor_scalar(ohl[:, m, :], tsel[:, m, :],
                                            lo8[:, m:m + 1], None,
                                            op0=ALU.is_equal)
                nc.vector.tensor_mul(sel, gv, ohl)
                nc.vector.tensor_reduce(glo, sel, axis=AX.X, op=ALU.max)
                for m in range(2):
                    nc.vector.tensor_scalar(ohl[:, m, :], tsel[:, m, :],
                                            hi8[:, m:m + 1], None,
                                            op0=ALU.is_equal)
                nc.vector.tensor_mul(sel, gv, ohl)
                nc.vector.tensor_reduce(ghi, sel, axis=AX.X, op=ALU.max)
                nc.vector.tensor_scalar_add(lo, lo8, -8.0)
                nc.vector.tensor_copy(hi, hi8)

            # round 1 positions from global row max
            for k in range(CN_K):
                nc.vector.tensor_scalar_add(pos[:, 0, 1 + k:2 + k],
                                            rmg[:, 0:1], -CN_FR1[k])
                nc.vector.tensor_scalar_add(pos[:, 1, 1 + k:2 + k],
                                            rmg[:, 1:2], -CN_FR1[k])
            for m in range(2):
                nc.vector.tensor_scalar_add(pos[:, m, 0:1],
                                            rmg[:, m:m + 1], -1.0)
                nc.vector.tensor_copy(pos[:, m, KX - 1:KX], rmg[:, m:m + 1])
            nc.vector.memset(gv[:, :, 0:1], 2.0)
            nc.vector.memset(gv[:, :, KX - 1:KX], 0.0)

            cn_probes(0)
            nc_iter(0)
            nc_iter(1)
            gvg = wk.tile([128, 2, CN_K], FP32, tag="gvg")
            allreduce(ALU.add, gp_l[0], gp_a[0], gv[:, :, 1:KX - 1], gvg)
            nc.vector.tensor_copy(gv[:, :, 1:KX - 1], gvg)
            cn_select()
            # round 2: 5 uniform probes; entries 6,7 duplicate hi (g=0,
            # is_equal ties in the select resolve via max)
            K2 = 5
            w2 = wk.tile([128, 2], FP32, tag="w2")
            nc.vector.tensor_sub(w2, hi, lo)
            for k in range(K2):
                nc.vector.scalar_tensor_tensor(
                    pos[:, :, 1 + k:2 + k], w2, (k + 1.0) / (K2 + 1), lo,
                    op0=ALU.mult, op1=ALU.add)
            for k in range(K2, CN_K):
                nc.vector.tensor_copy(pos[:, :, 1 + k:2 + k], hi)
            nc.vector.memset(gv[:, :, 1 + K2:KX - 1], 0.0)
            nc.vector.tensor_copy(pos[:, :, 0:1], lo)
            nc.vector.tensor_copy(pos[:, :, KX - 1:KX], hi)
            nc.vector.tensor_copy(gv[:, :, 0:1], glo)
            nc.vector.tensor_copy(gv[:, :, KX - 1:KX], ghi)

            cn_probes(1, nk=K2)
            nc_iter(2)
            nc_iter(3)
            allreduce(ALU.add, gp_l[1], gp_a[1], gv[:, :, 1:KX - 1], gvg)
            nc.vector.tensor_copy(gv[:, :, 1:KX - 1], gvg)
            cn_select()
            nc_iter(4)
            nc_iter(5)
            # secant interpolation: ctau = lo + clip((glo-1)/(glo-ghi)) * (hi-lo)
            ctau = slv.tile([128, 2], FP32)
            num2 = wk.tile([128, 2], FP32, tag="num2")
            nc.vector.tensor_scalar_add(num2, glo, -1.0)
            den2 = wk.tile([128, 2], FP32, tag="den2")
            nc.vector.tensor_sub(den2, glo, ghi)
            nc.vector.tensor_scalar_max(den2, den2, 1e-9)
            rd2 = wk.tile([128, 2], FP32, tag="rd2")
            nc.vector.reciprocal(rd2, den2)
            frac = wk.tile([128, 2], FP32, tag="frac")
            nc.vector.tensor_mul(frac, num2, rd2)
            nc.vector.tensor_scalar(frac, frac, 0.0, 1.0, op0=ALU.max, op1=ALU.min)
            nc.vector.tensor_sub(w2, hi, lo)
            nc.vector.tensor_mul(frac, frac, w2)
            nc.vector.tensor_add(ctau, lo, frac)
            if debug:
                nc.sync.dma_start(out=dbg['taucn'][:, :], in_=ctau)
                nc.sync.dma_start(out=dbg['taunc'][:, :], in_=tau)

            # taunc -> DRAM row for the phase-8 broadcast
            tau_b = wk.tile([128, NW], BF16, tag="tau_b")
            nc.vector.tensor_copy(tau_b, tau)
            nc.sync.dma_start(
                out=taunc_dram.ap().rearrange("w p -> p w"), in_=tau_b)

            # ===== c_memory: relu in simT layout, xbar transpose, matmul =====
            ps_cms = [psC.tile([128, 128], FP32, tag=f"cm{m}", name=f"pscm{m}")
                      for m in range(2)]
            for m in range(2):
                jt = junkL if m == 0 else junkL2
                nc.vector.tensor_scalar(
                    jt, simT[:, m, :], ctau[:, m:m + 1], 0.0,
                    op0=ALU.subtract, op1=ALU.max)
                nc.sync.dma_start_transpose(
                    sim_node[:, :, m * 128:(m + 1) * 128], jt)
            for ch in range(NW):
                for m in range(2):
                    nc.tensor.matmul(
                        ps_cms[m], sim_node[:, ch, m * 128:(m + 1) * 128],
                        nm_node[:, ch, :], start=(ch == 0), stop=(ch == NW - 1))
            cmf = wk.tile([128, 2, 128], FP32, tag="cmf")
            for m in range(2):
                nc.vector.tensor_copy(cmf[:, m, :], ps_cms[m])
            cmgf = wk.tile([128, 2, 128], FP32, tag="cmgf")
            allreduce(ALU.add, cm_local, cm_all, cmf, cmgf,
                      in_ap=cm_local.ap().rearrange("(m p) d -> p m d", p=128),
                      out_ap=cm_all.ap().rearrange("(m p) d -> p m d", p=128))
            cmg = scr.tile([128, 2, 128], BF16, tag="cmg")
            nc.vector.tensor_copy(cmg, cmgf)
            if debug:
                nc.sync.dma_start(
                    out=dbg['cmem'].ap().rearrange("(m p) d -> p m d", p=128),
                    in_=cmgf)
        # sim_node freed after c_memory (slv pool closed; nodep closes below)

        # ===== embedding =====
        with tc.tile_pool(name="embp", bufs=2) as embp, \
                tc.tile_pool(name="psZ", bufs=2, space="PSUM") as psZ:
            NW = L // 128
            tnc = const.tile([128, L], BF16)
            nc.sync.dma_start(out=tnc, in_=_bcast_row(taunc_dram, L))
            batches = []
            wb = 0
            while wb < NW:
                nwin = min(4, NW - wb)
                batches.append((wb, nwin))
                wb += nwin
            ncm_t = {}

            def emit_ncm(i):
                wbi, nwi = batches[i]
                bsl = bass.ds(wbi * 128, nwi * 128)
                ncm = embp.tile([128, 2, 512], BF16, tag="ncm",
                                name=f"ncm{i}")
                for m in range(2):
                    nc.vector.tensor_sub(ncm[:, m, 0:nwi * 128],
                                         simT[:, m, bsl], tnc[:, bsl])
                nc.vector.tensor_scalar_max(ncm[:, :, 0:nwi * 128],
                                            ncm[:, :, 0:nwi * 128], 0.0)
                ncm_t[i] = ncm

            emit_ncm(0)
            for i, (wbi, nwin) in enumerate(batches):
                if i + 1 < len(batches):
                    emit_ncm(i + 1)
                ncm = ncm_t.pop(i)
                ps_z = psZ.tile([128, 4, 128], FP32, tag="z")
                for k in range(nwin):
                    w = wbi + k
                    for m in range(2):
                        nc.tensor.matmul(
                            ps_z[:, k, :], ncm[:, m, k * 128:(k + 1) * 128],
                            cmg[:, m, :], start=(m == 0), stop=False)
                    nc.tensor.matmul(ps_z[:, k, :], identb, nm_node[:, w, :],
                                     start=False, stop=True)
                emb_c = embp.tile([128, 4, 128], FP32, tag="emb_c")
                nc.vector.tensor_copy(emb_c[:, 0:nwin, :], ps_z[:, 0:nwin, :])
                nc.sync.dma_start(
                    out=emb_out[wbi * 128:(wbi + nwin) * 128, :].rearrange(
                        "(k p) d -> p k d", p=128),
                    in_=emb_c[:, 0:nwin, :])

    split_waits(nc)
    return nc


# ----------------------------------------------------------------------------
# host side
# ----------------------------------------------------------------------------

_CACHE = {}


def _route(L, src, dst, t):
    idx = np.concatenate([src, dst]).astype(np.int64)
    other = np.concatenate([dst, src]).astype(np.int64)
    tt = np.concatenate([t, t])
    eidx = np.concatenate([np.arange(len(src)), np.arange(len(src))])
    NW = L // 128
    order = np.argsort(idx, kind='stable')
    idx_s, other_s, tt_s, eidx_s = idx[order], other[order], tt[order], eidx[order]
    owner = idx_s // L
    cores = []
    for c in range(NCORES):
        msk = owner == c
        li = idx_s[msk] - c * L
        win = li // 128
        col = li % 128
        wcount = np.bincount(win, minlength=NW)
        assert wcount.max() <= 256, f"window overflow: {wcount.max()}"
        woff = np.zeros(NW + 1, np.int64)
        woff[1:] = np.cumsum(wcount)
        within = np.arange(len(li)) - woff[win]
        slot = win * 256 + within
        cores.append(dict(slot=slot, col=col, li=li, other=other_s[msk],
                          tt=tt_s[msk], eidx=eidx_s[msk]))
    return cores


def kernel(**inputs):
    node_memory = np.asarray(inputs['node_memory'])
    last_update = np.asarray(inputs['last_update'])
    node_features = np.asarray(inputs['node_features'])
    event_feat = np.asarray(inputs['event_feat'])
    t = np.asarray(inputs['t'])
    src = np.asarray(inputs['src']).astype(np.int64)
    dst = np.asarray(inputs['dst']).astype(np.int64)
    time_w = np.asarray(inputs['time_w'])
    time_b = np.asarray(inputs['time_b'])
    W_ih = np.asarray(inputs['W_ih'])
    b_ih = np.asarray(inputs['b_ih'])
    W_hh = np.asarray(inputs['W_hh'])
    b_hh = np.asarray(inputs['b_hh'])
    proj_W = np.asarray(inputs['proj_W'])
    proj_b = np.asarray(inputs['proj_b'])
    centroids = np.asarray(inputs['centroids'])

    Nn = node_memory.shape[0]
    gran = 128 * GW * NCORES          # L must be a multiple of 128*GW
    NP = -(-Nn // gran) * gran
    L = NP // NCORES
    SLOTS = 2 * L
    NT = SLOTS // 128
    NG = L // GN

    nmp = np.zeros((NP, D), np.float32); nmp[:Nn] = node_memory
    nfp = np.zeros((NP, D), np.float32); nfp[:Nn] = node_features
    lup = np.zeros(NP, np.float32); lup[:Nn] = last_update

    idx_full = np.concatenate([src, dst])
    cnt_full = np.bincount(idx_full, minlength=NP).astype(np.float32)
    icnt_full = 1.0 / np.maximum(cnt_full, 1.0)
    nothas_full = (cnt_full == 0).astype(np.float32)

    cores = _route(L, src, dst, t)
    bsum_h = f32c(np.stack([(b_ih + b_hh)[0:128], (b_ih + b_hh)[128:256]], 1))
    wih_h = bfc(W_ih.T.reshape(4, 128, 384).transpose(1, 0, 2))

    in_maps = []
    for c in range(NCORES):
        r = cores[c]
        sl = r['slot']
        p_i = sl % 128
        t_i = sl // 128
        ev_x = np.zeros((128, NT, 256), ml_dtypes.bfloat16)
        ev_x[p_i, t_i, 0:128] = nmp[r['other']].astype(ml_dtypes.bfloat16)
        ev_x[p_i, t_i, 128:256] = event_feat[r['eidx']].astype(ml_dtypes.bfloat16)
        ev_meta = np.zeros((128, 2, NT), np.float32)
        ev_meta[:, 0, :] = -1.0
        ev_meta[p_i, 0, t_i] = r['col'].astype(np.float32)
        ev_meta[p_i, 1, t_i] = icnt_full[r['li'] + c * L]
        dt_flat = np.zeros(SLOTS, np.float32)
        dt_flat[sl] = r['tt'] - lup[r['li'] + c * L]
        nsl = slice(c * L, (c + 1) * L)
        in_maps.append({
            'ev_x': ev_x,
            'ev_meta': ev_meta,
            'dt_row': f32c(dt_flat.reshape(NG, GS)),
            'mem_bf': bfc(nmp[nsl].T),
            'nf_bf': bfc(nfp[nsl].T),
            'nothas': bfc(nothas_full[nsl].reshape(NG, GN)),
            'W_ihT': wih_h,
            'W_hhT': bfc(W_hh.T),
            'bsum': bsum_h,
            'b_hh2': f32c(b_hh[256:384].reshape(128, 1)),
            'b_ih2': f32c(b_ih[256:384].reshape(128, 1)),
            'pWt': bfc(proj_W),
            'pb': f32c(proj_b.reshape(128, 1)),
            'cenT': f32c(centroids.T),
            'w_row': f32c(time_w.reshape(1, 128)),
            'bpi_col': f32c((time_b + HALF_PI).reshape(128, 1)),
            'iota_t': bfc(np.tile(np.arange(128, dtype=np.float32)[None, :],
                                  (128, 1))),
        })

    debug = int(os.environ.get("KERNEL_DEBUG", "0"))
    key = (L, debug)
    if key not in _CACHE:
        _CACHE[key] = build_program(L, debug=debug)
    nc = _CACHE[key]
    res = run_bass_kernel_spmd(nc, in_maps, list(range(NCORES)))
    emb = np.concatenate([res.results[c]['emb'] for c in range(NCORES)], 0)
    kernel._last_exec_ns = getattr(res, 'exec_time_ns', None)
    if debug:
        kernel._last_results = res.results
    return emb[:Nn].astype(np.float32)


# revision 18
# speedup vs baseline: 1.1106x; 1.0088x over previous
"""TGN-style GNN message passing + community detection on 8 TRN2 NeuronCores.

Node-sharded SPMD, v2 (engine-balanced rewrite):
- Fused per-group pipeline (events -> agg -> GRU -> feat -> proj -> sim)
  with SBUF-resident intermediates; event tensors host-packed so every
  load is one large contiguous-per-partition DMA.
- Time encoding via a 1-partition PE outer product + Activation Sin with
  per-partition bias, landing in [feat, slot] layout, then xbar DMA
  transpose into the event matrix (no DVE work).
- has-mask folded into the GRU z-gate via a +30*(1-has) rank-1 matmul
  (sigmoid saturates to 1 -> memory passthrough), removing all blend ops.
- cn-sparsemax tau via 2 rounds of multi-probe bracketing (7 nonuniform +
  7 uniform probes, fused sub+relu+sum DVE ops in 4x bf16 mode) + secant
  interpolation: 3 AllReduces instead of 13.
- nc-sparsemax tau via secant from tau0=-2 (8 iterations, per-window
  fused DVE ops); overlapped with the cn AllReduce latency.
- c_memory via relu applied in simT layout then xbar-transposed;
  AllReduce #4. Total 4 collectives.
"""

import os
from contextlib import ExitStack

import numpy as np
import ml_dtypes

import concourse.bass as bass
import concourse.mybir as mybir
import concourse.tile as tile
from concourse.bass_utils import run_bass_kernel_spmd
from concourse.masks import make_identity

FP32 = mybir.dt.float32
BF16 = mybir.dt.bfloat16
AF = mybir.ActivationFunctionType
ALU = mybir.AluOpType
AX = mybir.AxisListType

NCORES = 8
D = 128
C = 256
HALF_PI = float(np.pi / 2)
TWO_PI = float(2 * np.pi)
MAGIC = 12582912.0
GW = 7                      # windows per group
GN = GW * 128               # nodes per group (896)
GS = 2 * GN                 # event slots per group (1792)
GT = GS // 128              # slot-tiles per group (14)

# cn bracket probe offsets below rowmax (round 1, ascending positions)
CN_P1 = [0.10, 0.15, 0.19, 0.22, 0.26, 0.35]  # absolute round-1 probes
CN_FR1 = CN_P1
CN_K = len(CN_FR1)          # 7 probes per round
KX = CN_K + 2               # probe array incl. bracket ends
CN_F2 = [(k + 1) / (CN_K + 1) for k in range(CN_K)]
BIGV = 1.0e4
NIT_NC = 6

bfc = lambda x: np.ascontiguousarray(np.asarray(x).astype(ml_dtypes.bfloat16))
f32c = lambda x: np.ascontiguousarray(np.asarray(x).astype(np.float32))


def _bcast_row(dram_tensor, ncols, nparts=128, off=0):
    row = dram_tensor.ap()
    return bass.AP(tensor=row.tensor, offset=row.offset + off,
                   ap=[[0, nparts], [1, ncols]])


def split_waits(nc, sp_limit=1, default_limit=1):
    """This env's walrus rejects >1 sync-wait on SP CTRL instructions:
    move extra waits onto preceding NOPs."""
    limits = {mybir.EngineType.SP: sp_limit}
    for fn in nc.m.functions:
        for bb in fn.blocks:
            out = []
            for ins in bb.instructions:
                si = ins.sync_info
                w = list(si.on_wait) if (si is not None and si.on_wait) else []
                lim = limits.get(ins.engine, default_limit)
                if len(w) > lim:
                    extra, keep = w[:-lim], w[-lim:]
                    for j in range(0, len(extra), lim):
                        out.append(mybir.InstNoOp(
                            name=f"{ins.name}-ws{j}",
                            engine=ins.engine,
                            sync_info=mybir.SyncInfo(
                                on_wait=list(extra[j:j + lim]), on_update=[]),
                        ))
                    ins.sync_info = mybir.SyncInfo(
                        on_wait=list(keep),
                        on_update=list(si.on_update) if si.on_update else [])
                out.append(ins)
            bb.instructions = out
    return nc


def build_program(L, debug=False):
    NG = L // GN            # groups (14 for L=12544)
    NW = L // 128           # windows (98)
    NT = 2 * L // 128       # slot-tiles total (196)

    nc = bass.Bass(num_devices=NCORES)

    # ---- inputs ----
    ev_x = nc.dram_tensor("ev_x", [128, NT, 256], BF16, kind="ExternalInput")
    ev_meta = nc.dram_tensor("ev_meta", [128, 2, NT], FP32, kind="ExternalInput")
    dt_row = nc.dram_tensor("dt_row", [NG, GS], FP32, kind="ExternalInput")
    mem_bf = nc.dram_tensor("mem_bf", [128, L], BF16, kind="ExternalInput")
    nf_bf = nc.dram_tensor("nf_bf", [128, L], BF16, kind="ExternalInput")
    nothas = nc.dram_tensor("nothas", [NG, GN], BF16, kind="ExternalInput")
    W_ihT = nc.dram_tensor("W_ihT", [128, 4, 384], BF16, kind="ExternalInput")
    W_hhT = nc.dram_tensor("W_hhT", [128, 384], BF16, kind="ExternalInput")
    bsum = nc.dram_tensor("bsum", [128, 2], FP32, kind="ExternalInput")
    b_hh2 = nc.dram_tensor("b_hh2", [128, 1], FP32, kind="ExternalInput")
    b_ih2 = nc.dram_tensor("b_ih2", [128, 1], FP32, kind="ExternalInput")
    pWt = nc.dram_tensor("pWt", [128, 128], BF16, kind="ExternalInput")
    pb = nc.dram_tensor("pb", [128, 1], FP32, kind="ExternalInput")
    cenT = nc.dram_tensor("cenT", [128, C], FP32, kind="ExternalInput")
    w_row = nc.dram_tensor("w_row", [1, 128], FP32, kind="ExternalInput")
    bpi_col = nc.dram_tensor("bpi_col", [128, 1], FP32, kind="ExternalInput")
    iota_t = nc.dram_tensor("iota_t", [128, 128], BF16, kind="ExternalInput")

    emb_out = nc.dram_tensor("emb", [L, D], FP32, kind="ExternalOutput")
    dbg = {}
    if debug:
        dbg['newmemT'] = nc.dram_tensor("dbg_newmemT", [128, L], FP32, kind="ExternalOutput")
        dbg['aggT'] = nc.dram_tensor("dbg_aggT", [128, 3, L], FP32, kind="ExternalOutput")
        dbg['xg'] = nc.dram_tensor("dbg_xg", [128, 2 * L // 128, 256], BF16, kind="ExternalOutput")
        dbg['tencT'] = nc.dram_tensor("dbg_tencT", [128, 2 * L], FP32, kind="ExternalOutput")
        dbg['simT'] = nc.dram_tensor("dbg_simT", [128, 2, L], BF16, kind="ExternalOutput")
        dbg['taunc'] = nc.dram_tensor("dbg_taunc", [128, NW], FP32, kind="ExternalOutput")
        dbg['taucn'] = nc.dram_tensor("dbg_taucn", [128, 2], FP32, kind="ExternalOutput")
        dbg['cmem'] = nc.dram_tensor("dbg_cmem", [C, D], FP32, kind="ExternalOutput")

    # ---- staging DRAM ----
    crec_dram = nc.dram_tensor("crec_dram", [1, C], BF16)
    rnorm_dram = nc.dram_tensor("rnorm_dram", [1, L], BF16)
    taunc_dram = nc.dram_tensor("taunc_dram", [NW, 128], BF16)
    taucn_dram = nc.dram_tensor("taucn_dram", [2, 128], BF16)
    rm_l = nc.dram_tensor("rm_l", [128, 4], FP32)
    rm_a = nc.dram_tensor("rm_a", [128, 4], FP32, addr_space="Shared")
    gp_l = [nc.dram_tensor(f"gp_l{r}", [128, 2 * CN_K], FP32) for r in range(2)]
    gp_a = [nc.dram_tensor(f"gp_a{r}", [128, 2 * CN_K], FP32, addr_space="Shared")
            for r in range(2)]
    cm_local = nc.dram_tensor("cm_local", [C, D], FP32)
    cm_all = nc.dram_tensor("cm_all", [C, D], FP32, addr_space="Shared")
    RG = [list(range(NCORES))]

    cc_sem = nc.alloc_semaphore("cc_done")
    ccv = [0]

    def allreduce(alu_op, local_dram, shared_dram, sb_in, sb_out,
                  in_ap=None, out_ap=None):
        """Stage sb_in -> local_dram, AllReduce -> shared_dram, load sb_out."""
        with tc.tile_critical():
            nc.gpsimd.dma_start(
                out=local_dram.ap() if in_ap is None else in_ap,
                in_=sb_in).then_inc(cc_sem, 16)
            ccv[0] += 16
            nc.gpsimd.wait_ge(cc_sem, ccv[0])
            nc.gpsimd.collective_compute(
                "AllReduce", alu_op, replica_groups=RG,
                ins=[local_dram.ap().opt()],
                outs=[shared_dram.ap().opt()]).then_inc(cc_sem)
            ccv[0] += 1
            nc.gpsimd.wait_ge(cc_sem, ccv[0])
            nc.gpsimd.dma_start(
                out=sb_out,
                in_=shared_dram.ap() if out_ap is None else out_ap
            ).then_inc(cc_sem, 16)
            ccv[0] += 16
            nc.gpsimd.wait_ge(cc_sem, ccv[0])

    ctx = ExitStack()
    with tile.TileContext(nc) as tc, ctx:
        const = ctx.enter_context(tc.tile_pool(name="const", bufs=1))
        late = ctx.enter_context(tc.tile_pool(name="late", bufs=1))
        wk = ctx.enter_context(tc.tile_pool(name="wk", bufs=2))
        scr = ctx.enter_context(tc.tile_pool(name="scr", bufs=1))
        psS = ctx.enter_context(tc.tile_pool(name="psS", bufs=1, space="PSUM"))

        # ----- constants -----
        identb = const.tile([128, 128], BF16)
        make_identity(nc, identb)
        iota = const.tile([128, 128], BF16)
        nc.sync.dma_start(out=iota, in_=iota_t[:, :])
        wih = const.tile([128, 4, 384], BF16)
        nc.sync.dma_start(out=wih, in_=W_ihT[:, :, :])
        whh = const.tile([128, 384], BF16)
        nc.sync.dma_start(out=whh, in_=W_hhT[:, :])
        bs = const.tile([128, 2], FP32)
        nc.sync.dma_start(out=bs, in_=bsum[:, :])
        bh2 = const.tile([128, 1], FP32)
        nc.sync.dma_start(out=bh2, in_=b_hh2[:, :])
        bi2 = const.tile([128, 1], FP32)
        nc.sync.dma_start(out=bi2, in_=b_ih2[:, :])
        pw = const.tile([128, 128], BF16)
        nc.sync.dma_start(out=pw, in_=pWt[:, :])
        pbt = const.tile([128, 1], FP32)
        nc.sync.dma_start(out=pbt, in_=pb[:, :])
        wrow = const.tile([1, 128], FP32)
        nc.sync.dma_start(out=wrow, in_=w_row[:, :])
        bpi = const.tile([128, 1], FP32)
        nc.sync.dma_start(out=bpi, in_=bpi_col[:, :])
        meta = const.tile([128, 2, NT], FP32)
        nc.sync.dma_start(out=meta, in_=ev_meta[:, :, :])
        ones_col = const.tile([128, 1], BF16)
        nc.vector.memset(ones_col, 1.0)
        thirty = const.tile([1, 128], BF16)
        nc.vector.memset(thirty, 30.0)
        eps1 = const.tile([1, 1], FP32)
        nc.vector.memset(eps1, 1e-12)

        # centroid norms (device, overlaps with first group loads)
        cen = const.tile([128, C], FP32)
        nc.sync.dma_start(out=cen, in_=cenT[:, :])
        censq = scr.tile([128, C], BF16, tag="censq")
        nc.vector.tensor_mul(censq, cen, cen)
        ps_c = psS.tile([1, 448], FP32, tag="srow")
        nc.tensor.matmul(ps_c[:, 0:C], ones_col, censq, start=True, stop=True)
        cnorm = scr.tile([1, C], FP32, tag="cnorm")
        nc.scalar.activation(cnorm, ps_c[:, 0:C], AF.Sqrt)
        nc.vector.tensor_scalar_add(cnorm, cnorm, 1e-8)
        crecf = scr.tile([1, C], FP32, tag="crecf")
        nc.vector.reciprocal(crecf, cnorm)
        crec = scr.tile([1, C], BF16, tag="crec")
        nc.vector.tensor_copy(crec, crecf)
        nc.sync.dma_start(out=crec_dram[:, :], in_=crec)
        crec_rep = const.tile([128, C], BF16)
        nc.sync.dma_start(out=crec_rep, in_=_bcast_row(crec_dram, C))
        cennT = const.tile([128, C], BF16)
        nc.vector.tensor_mul(cennT, cen, crec_rep)

        # ----- long-lived tensors -----
        simT = late.tile([128, 2, L], BF16)
        nodep = ctx.enter_context(tc.tile_pool(name="nodep", bufs=1))
        nm_node = nodep.tile([128, L // 128, 128], BF16)

        if True:

            # ================= fused group loop =================
            with tc.tile_pool(name="grp", bufs=2) as grp, \
                    tc.tile_pool(name="grpc", bufs=2) as grpc, \
                    tc.tile_pool(name="grp1", bufs=1) as grp1, \
                    tc.tile_pool(name="psA", bufs=1, space="PSUM") as psA, \
                    tc.tile_pool(name="psO", bufs=1, space="PSUM") as psO, \
                    tc.tile_pool(name="psG", bufs=1, space="PSUM") as psG:
                for g in range(NG):
                    t0 = g * GT
                    n0 = g * GN
                    nsl = bass.ds(n0, GN)
                    # --- event loads ---
                    xg = grp.tile([128, GT, 256], BF16, tag="xg")
                    nc.sync.dma_start(out=xg, in_=ev_x[:, t0:t0 + GT, :])
                    xgt = grp.tile([128, GT, 128], BF16, tag="xgt")
                    if debug == 2:
                        nc.sync.dma_start(out=dbg['xg'][:, t0:t0 + GT, :], in_=xg)
                    dtr = grpc.tile([1, GS], FP32, tag="dtr")
                    nc.sync.dma_start(out=dtr, in_=dt_row[g:g + 1, :])
                    # --- time encode: outer product + magic-number range
                    # reduce + Sin + xbar transpose ---
                    tencT = grp1.tile([128, GS], BF16, tag="tencT")
                    for q in range(4):
                        c0 = q * 448
                        ang = grp1.tile([128, 448], FP32, tag="ang",
                                        name=f"ang{g}{q}")
                        po = psO.tile([128, 448], FP32, tag="po",
                                      name=f"po{g}_{q}")
                        nc.tensor.matmul(po, wrow, dtr[:, c0:c0 + 448],
                                         start=True, stop=True)
                        nc.scalar.activation(ang, po, AF.Identity,
                                             bias=bpi[:, 0:1])
                        m1 = grp1.tile([128, 448], FP32, tag="m1",
                                       name=f"m1_{g}{q}")
                        nc.vector.tensor_scalar(m1, ang, 1.0 / TWO_PI, MAGIC,
                                                op0=ALU.mult, op1=ALU.add)
                        nc.vector.tensor_scalar_add(m1, m1, -MAGIC)
                        nc.vector.scalar_tensor_tensor(ang, m1, -TWO_PI, ang,
                                                       op0=ALU.mult, op1=ALU.add)
                        nc.scalar.activation(tencT[:, c0:c0 + 448],
                                             ang, AF.Sin)
                    nc.sync.dma_start_transpose(xgt, tencT)
                    # --- aggregation ---
                    agg = grp1.tile([128, 3, GN], BF16, tag="agg")
                    for w in range(GW):
                        pas = [psA.tile([128, 128], FP32, tag=f"agg{fc}",
                                        name=f"pa{g}_{w}_{fc}")
                               for fc in range(3)]
                        oh2 = []
                        for t_ in range(2):
                            ti = w * 2 + t_
                            oh = grpc.tile([128, 128], BF16, tag="oh",
                                           name=f"oh{g}_{w}_{t_}")
                            nc.vector.tensor_scalar(
                                oh, iota, meta[:, 0, t0 + ti:t0 + ti + 1],
                                meta[:, 1, t0 + ti:t0 + ti + 1],
                                op0=ALU.is_equal, op1=ALU.mult)
                            oh2.append(oh)
                        # tenc-independent chains first (overlap tenc tail)
                        for fc in range(2):
                            for t_ in range(2):
                                ti = w * 2 + t_
                                nc.tensor.matmul(
                                    pas[fc], xg[:, ti, fc * 128:(fc + 1) * 128],
                                    oh2[t_], start=(t_ == 0), stop=(t_ == 1))
                        for t_ in range(2):
                            ti = w * 2 + t_
                            nc.tensor.matmul(pas[2], xgt[:, ti, :], oh2[t_],
                                             start=(t_ == 0), stop=(t_ == 1))
                        for fc in range(3):
                            if (w + fc) % 2 == 0:
                                nc.vector.tensor_copy(
                                    agg[:, fc, w * 128:(w + 1) * 128], pas[fc])
                            else:
                                nc.scalar.activation(
                                    agg[:, fc, w * 128:(w + 1) * 128], pas[fc],
                                    AF.Identity)
                    if debug == 2:
                        aggf = grpc.tile([128, 3, GN], FP32, tag="aggf")
                        nc.vector.tensor_copy(aggf, agg)
                        nc.sync.dma_start(out=dbg['aggT'][:, :, nsl], in_=aggf)
                        tencf = grpc.tile([128, GS], FP32, tag="tencf")
                        nc.vector.tensor_copy(tencf, tencT)
                        nc.sync.dma_start(
                            out=dbg['tencT'][:, g * GS:(g + 1) * GS], in_=tencf)
                    # --- GRU ---
                    mg = grp.tile([128, GN], BF16, tag="mg")
                    nc.sync.dma_start(out=mg, in_=mem_bf[:, nsl])
                    nhg = grp1.tile([1, GN], BF16, tag="nhg")
                    nc.sync.dma_start(out=nhg, in_=nothas[g:g + 1, :])
                    rz = grp1.tile([128, 2, GN], BF16, tag="rz")
                    ng_t = grp1.tile([128, GN], BF16, tag="ng")
                    gh2s = grp1.tile([128, GN], BF16, tag="gh2s")
                    for h in range(2):
                        hs = bass.ds(h * 448, 448)
                        gi0 = psG.tile([128, 448], FP32, tag="gi0", name=f"gi0_{g}{h}")
                        gi1 = psG.tile([128, 448], FP32, tag="gi1", name=f"gi1_{g}{h}")
                        gi2 = psG.tile([128, 448], FP32, tag="gi2", name=f"gi2_{g}{h}")
                        gis = [gi0, gi1, gi2]
                        for m in range(3):
                            nc.tensor.matmul(gis[m], wih[:, 0, m * 128:(m + 1) * 128],
                                             mg[:, hs], start=True, stop=False)
                            for k in range(1, 4):
                                nc.tensor.matmul(
                                    gis[m], wih[:, k, m * 128:(m + 1) * 128],
                                    agg[:, k - 1, hs], start=False, stop=False)
                            if m < 2:
                                nc.tensor.matmul(gis[m], whh[:, m * 128:(m + 1) * 128],
                                                 mg[:, hs], start=False,
                                                 stop=(m == 0))
                        # z-gate +30*(1-has) (keeps memory where no events)
                        nc.tensor.matmul(gi1, thirty, nhg[:, hs],
                                         start=False, stop=True)
                        nc.scalar.activation(rz[:, 0, hs], gi0, AF.Sigmoid,
                                             bias=bs[:, 0:1])
                        nc.scalar.activation(rz[:, 1, hs], gi1, AF.Sigmoid,
                                             bias=bs[:, 1:2])
                        gh2 = psG.tile([128, 448], FP32, tag="gi0",
                                       name=f"gh2_{g}{h}")
                        nc.tensor.matmul(gh2, whh[:, 256:384], mg[:, hs],
                                         start=True, stop=True)
                        nc.scalar.activation(gh2s[:, hs], gh2, AF.Identity,
                                             bias=bh2[:, 0:1])
                        nc.vector.tensor_mul(gh2s[:, hs], rz[:, 0, hs],
                                             gh2s[:, hs])
                        nc.tensor.matmul(gi2, identb, gh2s[:, hs],
                                         start=False, stop=True)
                        nc.scalar.activation(ng_t[:, hs], gi2, AF.Tanh,
                                             bias=bi2[:, 0:1])
                    # newmem = n + z*(mem - n)
                    d_t = grp1.tile([128, GN], BF16, tag="d_t")
                    nc.vector.tensor_sub(d_t, mg, ng_t)
                    nc.vector.tensor_mul(d_t, rz[:, 1, :], d_t)
                    nmg = grpc.tile([128, GN], BF16, tag="nmg")
                    nc.vector.tensor_add(nmg, ng_t, d_t)
                    nc.sync.dma_start_transpose(
                        nm_node[:, g * GW:(g + 1) * GW, :], nmg)
                    if debug == 2:
                        nmgf = grpc.tile([128, GN], FP32, tag="nmgf")
                        nc.vector.tensor_copy(nmgf, nmg)
                        nc.sync.dma_start(out=dbg['newmemT'][:, nsl], in_=nmgf)
                    # feat = newmem + node_features (in-place over nfg)
                    nfg = grp.tile([128, GN], BF16, tag="nfg")
                    nc.sync.dma_start(out=nfg, in_=nf_bf[:, nsl])
                    ftg = nfg
                    nc.vector.tensor_add(ftg, nmg, nfg)
                    # --- projection + row norms ---
                    pfc = grp1.tile([128, GN], FP32, tag="pfc")
                    sqb = grp1.tile([128, GN], BF16, tag="sqb")
                    for h in range(2):
                        hs = bass.ds(h * 448, 448)
                        ppf = psG.tile([128, 448], FP32, tag="gi0", name=f"ppf{g}{h}")
                        nc.tensor.matmul(ppf, pw, ftg[:, hs], start=True, stop=True)
                        nc.scalar.activation(pfc[:, hs], ppf, AF.Identity,
                                             bias=pbt[:, 0:1])
                    nc.vector.tensor_mul(sqb, pfc, pfc)
                    rnb = grp1.tile([1, GN], BF16, tag="rnb")
                    for h in range(2):
                        hs = bass.ds(h * 448, 448)
                        pss = psS.tile([1, 448], FP32, tag="srow", name=f"pss{g}{h}")
                        nc.tensor.matmul(pss, ones_col, sqb[:, hs],
                                         start=True, stop=True)
                        rnf = grp1.tile([1, 448], FP32, tag="rnf")
                        nc.scalar.activation(rnf, pss, AF.Sqrt)
                        nc.vector.tensor_scalar_add(rnf, rnf, 1e-8)
                        rnr = grp1.tile([1, 448], FP32, tag="rnr")
                        nc.vector.reciprocal(rnr, rnf)
                        nc.vector.tensor_copy(rnb[:, hs], rnr)
                    nc.sync.dma_start(out=rnorm_dram[0, nsl], in_=rnb)
                    rep = grp1.tile([128, GN], BF16, tag="rep")
                    nc.sync.dma_start(out=rep, in_=_bcast_row(rnorm_dram, GN, off=n0))
                    pfng = grpc.tile([128, GN], BF16, tag="pfn")
                    nc.vector.tensor_mul(pfng, pfc, rep)
                    # --- similarity (448-col PSUM chunks) ---
                    for m in range(2):
                        for h in range(2):
                            hl = bass.ds(h * 448, 448)
                            hs = bass.ds(n0 + h * 448, 448)
                            psm = psG.tile([128, 448], FP32, tag="gi1",
                                           name=f"psm{g}{m}{h}")
                            nc.tensor.matmul(psm, cennT[:, m * 128:(m + 1) * 128],
                                             pfng[:, hl], start=True, stop=True)
                            if m == 0:
                                nc.vector.tensor_copy(simT[:, m, hs], psm)
                            else:
                                nc.scalar.activation(simT[:, m, hs], psm,
                                                     AF.Identity)
            # group pools freed
            if debug:
                nc.sync.dma_start(out=dbg['simT'][:, :, :], in_=simT)


        # (sim_node/nm_node filled per group above)

        with tc.tile_pool(name="slv", bufs=1) as slv, \
                tc.tile_pool(name="psC", bufs=1, space="PSUM") as psC:
            sim_node = slv.tile([128, L // 128, 256], BF16)
            for m in range(2):
                nc.sync.dma_start_transpose(
                    sim_node[:, :, m * 128:(m + 1) * 128], simT[:, m, :])
            # ===== nc secant (tau over C=256 per node) =====
            # g-eval: relu(x - t) == max(x, t) - t; accum_out reduces with
            # op1 (add) -> per-partition sum.
            junk_n = slv.tile([128, C], BF16)
            junk_n2 = slv.tile([128, C], BF16)
            zer_n = slv.tile([128, C], BF16)
            nc.vector.memset(zer_n, 0.0)
            NW = L // 128
            tau = slv.tile([128, NW], FP32)
            tau_p = slv.tile([128, NW], FP32)
            g_c = slv.tile([128, NW], FP32)
            g_p = slv.tile([128, NW], FP32)

            def nc_eval(tau_tile, g_tile):
                for ch in range(NW):
                    jt = junk_n if ch % 2 == 0 else junk_n2
                    nc.vector.scalar_tensor_tensor(
                        jt, sim_node[:, ch, :], tau_tile[:, ch:ch + 1], zer_n,
                        op0=ALU.subtract, op1=ALU.max,
                        accum_out=g_tile[:, ch:ch + 1])

            nc.vector.memset(tau_p, -2.0)
            nc_eval(tau_p, g_p)
            st1 = wk.tile([128, NW], FP32, tag="st1")
            nc.vector.tensor_scalar(st1, g_p, -1.0, 1.0 / 256.0,
                                    op0=ALU.add, op1=ALU.mult)
            nc.vector.tensor_add(tau, tau_p, st1)

            def secant_update(tt, tp, gg, gp, wtag, shape):
                num = wk.tile(shape, FP32, tag=wtag + "n")
                nc.vector.tensor_sub(num, tt, tp)
                gm1 = wk.tile(shape, FP32, tag=wtag + "g")
                nc.vector.tensor_scalar_add(gm1, gg, -1.0)
                nc.vector.tensor_mul(num, num, gm1)
                den = wk.tile(shape, FP32, tag=wtag + "d")
                nc.vector.tensor_sub(den, gp, gg)
                nc.vector.tensor_scalar_max(den, den, 1e-12)
                rden = wk.tile(shape, FP32, tag=wtag + "r")
                nc.vector.reciprocal(rden, den)
                nc.vector.tensor_copy(tp, tt)
                nc.vector.tensor_copy(gp, gg)
                stp = wk.tile(shape, FP32, tag=wtag + "s")
                nc.vector.tensor_mul(stp, num, rden)
                nc.vector.tensor_scalar(stp, stp, 0.0, 1.0,
                                        op0=ALU.max, op1=ALU.min)
                nc.vector.tensor_add(tt, tt, stp)

            def nc_iter(k):
                nc_eval(tau, g_c)
                secant_update(tau, tau_p, g_c, g_p, "ncs", [128, NW])

            # ===== cn bracket solver =====
            pos = slv.tile([128, 2, KX], FP32)
            gv = slv.tile([128, 2, KX], FP32)
            lo = slv.tile([128, 2], FP32)
            hi = slv.tile([128, 2], FP32)
            glo = slv.tile([128, 2], FP32)
            ghi = slv.tile([128, 2], FP32)
            junkL = slv.tile([128, L], BF16)
            junkL2 = junkL
            neg = slv.tile([128, 2, KX], FP32)

            def cn_probes(round_idx, nk=CN_K):
                nc.vector.tensor_scalar_mul(neg, pos, -1.0)
                for m in range(2):
                    for k in range(nk):
                        jt = junkL if k % 2 == 0 else junkL2
                        nc.scalar.activation(
                            jt, simT[:, m, :], AF.Relu,
                            bias=neg[:, m, 1 + k:2 + k],
                            accum_out=gv[:, m, 1 + k:2 + k])

            def cn_select():
                # shift masked (g>=1) positions by +8; the bracket ends are
                # argmax/argmin in shifted space; g values extracted by
                # bitwise-consistent is_equal one-hots (no magnitude tricks).
                msk = wk.tile([128, 2, KX], FP32, tag="msk")
                nc.vector.tensor_scalar(msk, gv, 1.0, None, op0=ALU.is_ge)
                tsel = wk.tile([128, 2, KX], FP32, tag="tsel")
                nc.vector.scalar_tensor_tensor(tsel, msk, 8.0, pos,
                                               op0=ALU.mult, op1=ALU.add)
                lo8 = wk.tile([128, 2], FP32, tag="lo8")
                nc.vector.tensor_reduce(lo8, tsel, axis=AX.X, op=ALU.max)
                hi8 = wk.tile([128, 2], FP32, tag="hi8")
                nc.vector.tensor_reduce(hi8, tsel, axis=AX.X, op=ALU.min)
                ohl = wk.tile([128, 2, KX], FP32, tag="ohl")
                sel = wk.tile([128, 2, KX], FP32, tag="sel")
                for m in range(2):
                    nc.vector.tensor_scalar(ohl[:, m, :], tsel[:, m, :],
                                            lo8[:, m:m + 1], None,
                                            op0=ALU.is_equal)
                nc.vector.tensor_mul(sel, gv, ohl)
                nc.vector.tensor_reduce(glo, sel, axis=AX.X, op=ALU.max)
                for m in range(2):
                    nc.vector.tensor_scalar(ohl[:, m, :], tsel[:, m, :],
                                            hi8[:, m:m + 1], None,
                                            op0=ALU.is_equal)
                nc.vector.tensor_mul(sel, gv, ohl)
                nc.vector.tensor_reduce(ghi, sel, axis=AX.X, op=ALU.max)
                nc.vector.tensor_scalar_add(lo, lo8, -8.0)
                nc.vector.tensor_copy(hi, hi8)

            # round 1 positions: fixed probes (identical on all cores, no
            # rowmax AllReduce); ends at the cosine-similarity bounds
            for k in range(CN_K):
                nc.vector.memset(pos[:, :, 1 + k:2 + k], CN_P1[k])
            nc.vector.memset(pos[:, :, 0:1], -1.0)
            nc.vector.memset(pos[:, :, KX - 1:KX], 1.0)
            nc.vector.memset(gv[:, :, 0:1], 2.0)
            nc.vector.memset(gv[:, :, KX - 1:KX], 0.0)

            cn_probes(0)
            nc_iter(0)
            nc_iter(1)
            gvg = wk.tile([128, 2, CN_K], FP32, tag="gvg")
            allreduce(ALU.add, gp_l[0], gp_a[0], gv[:, :, 1:KX - 1], gvg)
            nc.vector.tensor_copy(gv[:, :, 1:KX - 1], gvg)
            cn_select()
            # round 2: 5 uniform probes; entries 6,7 duplicate hi (g=0,
            # is_equal ties in the select resolve via max)
            K2 = 5
            w2 = wk.tile([128, 2], FP32, tag="w2")
            nc.vector.tensor_sub(w2, hi, lo)
            for k in range(K2):
                nc.vector.scalar_tensor_tensor(
                    pos[:, :, 1 + k:2 + k], w2, (k + 1.0) / (K2 + 1), lo,
                    op0=ALU.mult, op1=ALU.add)
            for k in range(K2, CN_K):
                nc.vector.tensor_copy(pos[:, :, 1 + k:2 + k], hi)
            nc.vector.memset(gv[:, :, 1 + K2:KX - 1], 0.0)
            nc.vector.tensor_copy(pos[:, :, 0:1], lo)
            nc.vector.tensor_copy(pos[:, :, KX - 1:KX], hi)
            nc.vector.tensor_copy(gv[:, :, 0:1], glo)
            nc.vector.tensor_copy(gv[:, :, KX - 1:KX], ghi)

            cn_probes(1, nk=K2)
            nc_iter(2)
            nc_iter(3)
            allreduce(ALU.add, gp_l[1], gp_a[1], gv[:, :, 1:KX - 1], gvg)
            nc.vector.tensor_copy(gv[:, :, 1:KX - 1], gvg)
            cn_select()
            nc_iter(4)
            nc_iter(5)
            # secant interpolation: ctau = lo + clip((glo-1)/(glo-ghi)) * (hi-lo)
            ctau = slv.tile([128, 2], FP32)
            num2 = wk.tile([128, 2], FP32, tag="num2")
            nc.vector.tensor_scalar_add(num2, glo, -1.0)
            den2 = wk.tile([128, 2], FP32, tag="den2")
            nc.vector.tensor_sub(den2, glo, ghi)
            nc.vector.tensor_scalar_max(den2, den2, 1e-9)
            rd2 = wk.tile([128, 2], FP32, tag="rd2")
            nc.vector.reciprocal(rd2, den2)
            frac = wk.tile([128, 2], FP32, tag="frac")
            nc.vector.tensor_mul(frac, num2, rd2)
            nc.vector.tensor_scalar(frac, frac, 0.0, 1.0, op0=ALU.max, op1=ALU.min)
            nc.vector.tensor_sub(w2, hi, lo)
            nc.vector.tensor_mul(frac, frac, w2)
            nc.vector.tensor_add(ctau, lo, frac)
            if debug:
                nc.sync.dma_start(out=dbg['taucn'][:, :], in_=ctau)
                nc.sync.dma_start(out=dbg['taunc'][:, :], in_=tau)

            # taunc -> DRAM row for the phase-8 broadcast
            tau_b = wk.tile([128, NW], BF16, tag="tau_b")
            nc.vector.tensor_copy(tau_b, tau)
            nc.sync.dma_start(
                out=taunc_dram.ap().rearrange("w p -> p w"), in_=tau_b)

            # ===== c_memory: relu in simT layout, xbar transpose, matmul =====
            ps_cms = [psC.tile([128, 128], FP32, tag=f"cm{m}", name=f"pscm{m}")
                      for m in range(2)]
            for m in range(2):
                jt = junkL if m == 0 else junkL2
                nc.vector.tensor_scalar(
                    jt, simT[:, m, :], ctau[:, m:m + 1], 0.0,
                    op0=ALU.subtract, op1=ALU.max)
                nc.sync.dma_start_transpose(
                    sim_node[:, :, m * 128:(m + 1) * 128], jt)
            for ch in range(NW):
                for m in range(2):
                    nc.tensor.matmul(
                        ps_cms[m], sim_node[:, ch, m * 128:(m + 1) * 128],
                        nm_node[:, ch, :], start=(ch == 0), stop=(ch == NW - 1))
            cmf = wk.tile([128, 2, 128], FP32, tag="cmf")
            for m in range(2):
                nc.vector.tensor_copy(cmf[:, m, :], ps_cms[m])
            cmgf = wk.tile([128, 2, 128], FP32, tag="cmgf")
            allreduce(ALU.add, cm_local, cm_all, cmf, cmgf,
                      in_ap=cm_local.ap().rearrange("(m p) d -> p m d", p=128),
                      out_ap=cm_all.ap().rearrange("(m p) d -> p m d", p=128))
            cmg = scr.tile([128, 2, 128], BF16, tag="cmg")
            nc.vector.tensor_copy(cmg, cmgf)
            if debug:
                nc.sync.dma_start(
                    out=dbg['cmem'].ap().rearrange("(m p) d -> p m d", p=128),
                    in_=cmgf)
        # sim_node freed after c_memory (slv pool closed; nodep closes below)

        # ===== embedding =====
        with tc.tile_pool(name="embp", bufs=2) as embp, \
                tc.tile_pool(name="psZ", bufs=2, space="PSUM") as psZ:
            NW = L // 128
            tnc = const.tile([128, L], BF16)
            nc.sync.dma_start(out=tnc, in_=_bcast_row(taunc_dram, L))
            batches = []
            wb = 0
            while wb < NW:
                nwin = min(4, NW - wb)
                batches.append((wb, nwin))
                wb += nwin
            ncm_t = {}

            def emit_ncm(i):
                wbi, nwi = batches[i]
                bsl = bass.ds(wbi * 128, nwi * 128)
                ncm = embp.tile([128, 2, 512], BF16, tag="ncm",
                                name=f"ncm{i}")
                for m in range(2):
                    nc.vector.tensor_sub(ncm[:, m, 0:nwi * 128],
                                         simT[:, m, bsl], tnc[:, bsl])
                nc.vector.tensor_scalar_max(ncm[:, :, 0:nwi * 128],
                                            ncm[:, :, 0:nwi * 128], 0.0)
                ncm_t[i] = ncm

            emit_ncm(0)
            for i, (wbi, nwin) in enumerate(batches):
                if i + 1 < len(batches):
                    emit_ncm(i + 1)
                ncm = ncm_t.pop(i)
                ps_z = psZ.tile([128, 4, 128], FP32, tag="z")
                for k in range(nwin):
                    w = wbi + k
                    for m in range(2):
                        nc.tensor.matmul(
                            ps_z[:, k, :], ncm[:, m, k * 128:(k + 1) * 128],
                            cmg[:, m, :], start=(m == 0), stop=False)
                    nc.tensor.matmul(ps_z[:, k, :], identb, nm_node[:, w, :],
                                     start=False, stop=True)
                emb_c = embp.tile([128, 4, 128], FP32, tag="emb_c")
                nc.vector.tensor_copy(emb_c[:, 0:nwin, :], ps_z[:, 0:nwin, :])
                nc.sync.dma_start(
                    out=emb_out[wbi * 128:(wbi + nwin) * 128, :].rearrange(
                        "(k p) d -> p k d", p=128),
                    in_=emb_c[:, 0:nwin, :])

    split_waits(nc)
    return nc


# ----------------------------------------------------------------------------
# host side
# ----------------------------------------------------------------------------

_CACHE = {}


def _route(L, src, dst, t):
    idx = np.concatenate([src, dst]).astype(np.int64)
    other = np.concatenate([dst, src]).astype(np.int64)
    tt = np.concatenate([t, t])
    eidx = np.concatenate([np.arange(len(src)), np.arange(len(src))])
    NW = L // 128
    order = np.argsort(idx, kind='stable')
    idx_s, other_s, tt_s, eidx_s = idx[order], other[order], tt[order], eidx[order]
    owner = idx_s // L
    cores = []
    for c in range(NCORES):
        msk = owner == c
        li = idx_s[msk] - c * L
        win = li // 128
        col = li % 128
        wcount = np.bincount(win, minlength=NW)
        assert wcount.max() <= 256, f"window overflow: {wcount.max()}"
        woff = np.zeros(NW + 1, np.int64)
        woff[1:] = np.cumsum(wcount)
        within = np.arange(len(li)) - woff[win]
        slot = win * 256 + within
        cores.append(dict(slot=slot, col=col, li=li, other=other_s[msk],
                          tt=tt_s[msk], eidx=eidx_s[msk]))
    return cores


def kernel(**inputs):
    node_memory = np.asarray(inputs['node_memory'])
    last_update = np.asarray(inputs['last_update'])
    node_features = np.asarray(inputs['node_features'])
    event_feat = np.asarray(inputs['event_feat'])
    t = np.asarray(inputs['t'])
    src = np.asarray(inputs['src']).astype(np.int64)
    dst = np.asarray(inputs['dst']).astype(np.int64)
    time_w = np.asarray(inputs['time_w'])
    time_b = np.asarray(inputs['time_b'])
    W_ih = np.asarray(inputs['W_ih'])
    b_ih = np.asarray(inputs['b_ih'])
    W_hh = np.asarray(inputs['W_hh'])
    b_hh = np.asarray(inputs['b_hh'])
    proj_W = np.asarray(inputs['proj_W'])
    proj_b = np.asarray(inputs['proj_b'])
    centroids = np.asarray(inputs['centroids'])

    Nn = node_memory.shape[0]
    gran = 128 * GW * NCORES          # L must be a multiple of 128*GW
    NP = -(-Nn // gran) * gran
    L = NP // NCORES
    SLOTS = 2 * L
    NT = SLOTS // 128
    NG = L // GN

    nmp = np.zeros((NP, D), np.float32); nmp[:Nn] = node_memory
    nfp = np.zeros((NP, D), np.float32); nfp[:Nn] = node_features
    lup = np.zeros(NP, np.float32); lup[:Nn] = last_update

    idx_full = np.concatenate([src, dst])
    cnt_full = np.bincount(idx_full, minlength=NP).astype(np.float32)
    icnt_full = 1.0 / np.maximum(cnt_full, 1.0)
    nothas_full = (cnt_full == 0).astype(np.float32)

    cores = _route(L, src, dst, t)
    bsum_h = f32c(np.stack([(b_ih + b_hh)[0:128], (b_ih + b_hh)[128:256]], 1))
    wih_h = bfc(W_ih.T.reshape(4, 128, 384).transpose(1, 0, 2))

    in_maps = []
    for c in range(NCORES):
        r = cores[c]
        sl = r['slot']
        p_i = sl % 128
        t_i = sl // 128
        ev_x = np.zeros((128, NT, 256), ml_dtypes.bfloat16)
        ev_x[p_i, t_i, 0:128] = nmp[r['other']].astype(ml_dtypes.bfloat16)
        ev_x[p_i, t_i, 128:256] = event_feat[r['eidx']].astype(ml_dtypes.bfloat16)
        ev_meta = np.zeros((128, 2, NT), np.float32)
        ev_meta[:, 0, :] = -1.0
        ev_meta[p_i, 0, t_i] = r['col'].astype(np.float32)
        ev_meta[p_i, 1, t_i] = icnt_full[r['li'] + c * L]
        dt_flat = np.zeros(SLOTS, np.float32)
        dt_flat[sl] = r['tt'] - lup[r['li'] + c * L]
        nsl = slice(c * L, (c + 1) * L)
        in_maps.append({
            'ev_x': ev_x,
            'ev_meta': ev_meta,
            'dt_row': f32c(dt_flat.reshape(NG, GS)),
            'mem_bf': bfc(nmp[nsl].T),
            'nf_bf': bfc(nfp[nsl].T),
            'nothas': bfc(nothas_full[nsl].reshape(NG, GN)),
            'W_ihT': wih_h,
            'W_hhT': bfc(W_hh.T),
            'bsum': bsum_h,
            'b_hh2': f32c(b_hh[256:384].reshape(128, 1)),
            'b_ih2': f32c(b_ih[256:384].reshape(128, 1)),
            'pWt': bfc(proj_W),
            'pb': f32c(proj_b.reshape(128, 1)),
            'cenT': f32c(centroids.T),
            'w_row': f32c(time_w.reshape(1, 128)),
            'bpi_col': f32c((time_b + HALF_PI).reshape(128, 1)),
            'iota_t': bfc(np.tile(np.arange(128, dtype=np.float32)[None, :],
                                  (128, 1))),
        })

    debug = int(os.environ.get("KERNEL_DEBUG", "0"))
    key = (L, debug)
    if key not in _CACHE:
        _CACHE[key] = build_program(L, debug=debug)
    nc = _CACHE[key]
    res = run_bass_kernel_spmd(nc, in_maps, list(range(NCORES)))
    emb = np.concatenate([res.results[c]['emb'] for c in range(NCORES)], 0)
    kernel._last_exec_ns = getattr(res, 'exec_time_ns', None)
    if debug:
        kernel._last_results = res.results
    return emb[:Nn].astype(np.float32)
